# revision 1
# baseline (speedup 1.0000x reference)
"""2-layer GCN (PyG GCNConv x2 + sigmoid) on 8 TRN2 NeuronCores, single fused NEFF.

Design (memory-regime, gather-bound):
- All GCN normalization is folded out of the per-edge path:
  out = sigmoid(dinv_dst * segsum(M'[src]) + b), with M' = W1^T @ (x * dinv)^T
  built on-device by the PE. Per-edge work is pure gather + sum.
- Layer 1: dst-sharded across cores; feature-major source tables built in 4
  pipelined node-quarters (12.5K nodes each, fits int16 gather indices);
  GPSIMD ap_gather streams per-edge rows (~1.4ns/edge); exact segment sums
  via degree-ladder tensor_reduce with compile-time-uniform budgets across
  cores; perm-gather reassembles node order; finalize is sliced to overlap
  the last quarter's assembly.
- z' = h'@W2 shards are AllGathered on-device (DRAM bounce, Shared output);
  falls back to a two-launch host-crossing variant if collectives fail.
- Layer 2: scalar gathers use the 16-partition-group structure of ap_gather
  (8 independent edge groups per instruction -> 8x throughput); tiny tables
  live on stride-16 partitions only.
- Host does only index/layout preprocessing: degrees, ladder packing with
  degree bucketing, a src-table permutation that dealigns self-loops (keeps
  shared max-over-core budgets ~13% instead of ~34% over actual edges),
  int16 index wrapping (16B-aligned slices), output unpermutation.
"""

import sys

sys.path.insert(0, "/opt/trn_rl_repo")
import numpy as np
from contextlib import ExitStack

from concourse import bacc, mybir
from concourse.tile import TileContext
from concourse.bass_utils import run_bass_kernel_spmd

MEASURE = False  # when True, run the cost-model simulator and fill LAST_SIM_NS
LAST_SIM_NS = None

N = 50000
E = 800000
F = 128
P = 128
NCORES = 8
NSH = N // NCORES  # 6250 dst nodes per core
NQ = 4  # src quarters
QN = N // NQ  # 12500 nodes per quarter
QCOLS = 12800  # quarter table cols: [zero, 12500 nodes, pad] -> 25 chunks of 512
MMCH = 512  # matmul chunk
G1 = 2560  # k1 gather chunk (slots; /16 cols stays 16B-aligned)
NGROUP = 8  # k2: 16-partition groups


def _wrap16(idx_flat):
    """logical sequence -> [16, n/16] wrapped layout"""
    n = idx_flat.shape[0]
    assert n % 16 == 0
    return np.ascontiguousarray(idx_flat.reshape(n // 16, 16).T)


def _pad16(n, mult=16):
    return ((n + mult - 1) // mult) * mult


def _pad128(n):
    return ((n + 127) // 128) * 128


def host_prep(x, edge_index, W1, b1, W2, b2):
    """All index/layout preprocessing. Returns per-core input maps + metadata."""
    src = np.concatenate([edge_index[0], np.arange(N, dtype=np.int64)]).astype(np.int32)
    dst = np.concatenate([edge_index[1], np.arange(N, dtype=np.int64)]).astype(np.int32)
    deg = np.bincount(dst, minlength=N).astype(np.float32)
    dinv = 1.0 / np.sqrt(np.maximum(deg, 1e-12))
    dinv[deg <= 0] = 0.0

    # Permute the src-table node order so each node's self-loop lands in a
    # pseudo-random quarter: keeps per-(core,quarter) degree distributions
    # aligned across cores, which keeps the shared max-over-cores ladder
    # budgets tight. dst-side layout is unaffected.
    psrc = np.random.default_rng(12345).permutation(N)  # node -> table position
    pinv = np.argsort(psrc)  # table position -> node

    # xT_pre[f, pos] = x[node, f] * dinv[node]; layout per quarter: [zero, nodes, pad]
    xtp = (x * dinv[:, None]).T.astype(np.float32)[:, pinv]  # [128, N] position order
    xt = np.zeros((P, NQ * QCOLS), dtype=np.float32)
    for q in range(NQ):
        xt[:, q * QCOLS + 1 : q * QCOLS + 1 + QN] = xtp[:, q * QN : (q + 1) * QN]

    core = dst // NSH  # [Etot]
    dstl = dst % NSH
    pos = psrc[src]
    quarter = pos // QN
    srcl = (pos % QN).astype(np.int32) + 1  # 0 = zero col

    # per (core, quarter): kappa counts per local dst node
    kap = np.zeros((NCORES, NQ, NSH), dtype=np.int32)
    for c in range(NCORES):
        mc = core == c
        for q in range(NQ):
            m = mc & (quarter == q)
            kap[c, q] = np.bincount(dstl[m], minlength=NSH)

    kmax = int(kap.max())
    # bucketed ladder: exact for small degrees, coarse above (pools the sparse
    # tail so the max-over-cores budget inflation stays small)
    lut = np.arange(kmax + 1)
    for kk in range(9, kmax + 1):
        for bb in (10, 12, 14, 17, 21, 26, 32, 40, 48, 64, 96, 128, 192, 256):
            if kk <= bb:
                lut[kk] = bb
                break
    kapb = lut[kap]
    # ladder budgets per quarter: n_b = max over cores of #nodes with bucket==b
    budgets = []  # budgets[q] = {bucket: n_b}
    for q in range(NQ):
        b = {}
        for k in np.unique(kapb[:, q, :]):
            k = int(k)
            if k == 0:
                continue
            nk = int((kapb[:, q, :] == k).sum(axis=1).max())
            if nk > 0:
                b[k] = nk
        budgets.append(b)

    # pack ladder rows into G1-slot chunks; shared layout per quarter
    # descriptors: (chunk_idx, slot_off_in_chunk, n_rows, k, accp_col_off)
    layouts = []  # layouts[q] = (n_chunks, [descr], accp_cols, {k: col_off})
    for q in range(NQ):
        descr = []
        kbase = {}
        col = 1  # col 0 = zero col
        ch, off = 0, 0
        for k in sorted(budgets[q]):
            nk = budgets[q][k]
            kbase[k] = col
            left = nk
            while left > 0:
                fit = min(left, (G1 - off) // k)
                if fit == 0:
                    ch += 1
                    off = 0
                    fit = min(left, G1 // k)
                descr.append((ch, off, fit, k, col))
                off += fit * k
                col += fit
                left -= fit
            # next k continues filling same chunk
        n_chunks = ch + 1
        layouts.append((n_chunks, descr, col, kbase))

    SQ = [layouts[q][0] * G1 for q in range(NQ)]  # slots per quarter
    PQ = max(layouts[q][2] for q in range(NQ))  # accP col budget
    PQ = _pad16(PQ)

    # build per-core slot-index arrays + perms
    eidx = np.zeros((NCORES, sum(SQ)), dtype=np.int16)
    PERM_NI = _pad128(NSH)  # 6272; multiple of 128 so idx slices stay 16B-aligned
    perms = np.zeros((NCORES, NQ, PERM_NI), dtype=np.int16)
    order = np.lexsort((dstl, quarter, core))  # edges grouped by (core, quarter, dst)
    so, do_, qo, co = srcl[order], dstl[order], quarter[order], core[order]
    for c in range(NCORES):
        qbase = 0
        for q in range(NQ):
            m = (co == c) & (qo == q)
            s_cq, d_cq = so[m], do_[m]  # sorted by dst
            kv = kap[c, q]
            kvb = lut[kv]
            # nodes with kappa>0, bucket-grouped: rank within bucket-section
            nodes = np.nonzero(kv)[0]
            kn = kv[nodes]  # actual degree (slots filled)
            knb = kvb[nodes]  # bucket (row width)
            nd_order = np.lexsort((nodes, knb))  # sort nodes by (bucket, node)
            nodes_s = nodes[nd_order]
            kn_s = kn[nd_order]
            knb_s = knb[nd_order]
            # row start slot for each node, following the shared layout
            _, descr, _, kbase = layouts[q]
            # per-bucket: rank of node among same-bucket nodes
            rank = np.zeros(len(nodes_s), dtype=np.int64)
            colof = np.zeros(len(nodes_s), dtype=np.int64)
            for k in np.unique(knb_s):
                mk = knb_s == k
                rank[mk] = np.arange(mk.sum())
                colof[mk] = kbase[int(k)]
            node_col = colof + rank  # accP column of each node
            perms[c, q, : len(nodes)] = 0
            pm = np.zeros(NSH, dtype=np.int16)
            pm[nodes_s] = node_col.astype(np.int16)
            perms[c, q, :NSH] = pm
            # slot position of each (row=node_col, lane): need chunk/slot map per accP col
            col2slot = np.full(layouts[q][2], -1, dtype=np.int64)
            for ch, off, n_rows, k, col in descr:
                cols = np.arange(n_rows)
                col2slot[col + cols] = ch * G1 + off + cols * k
            # edges of node appear consecutively (sorted by dst within (c,q))
            # slot of edge j of node n = col2slot[node_col[n]] + j
            # build via repeat
            starts = col2slot[node_col]
            eslots = np.repeat(starts, kn_s) + _concat_aranges(kn_s)
            # values: srcl of edges, grouped per node ascending-dst...
            # s_cq is sorted by dst; nodes_s is sorted by (k,node) -> reorder edges
            edge_node_ptr = np.zeros(NSH + 1, dtype=np.int64)
            edge_node_ptr[1:] = np.cumsum(kv)
            ev = np.concatenate(
                [s_cq[edge_node_ptr[n] : edge_node_ptr[n + 1]] for n in nodes_s]
            ) if len(nodes_s) else np.zeros(0, dtype=np.int32)
            eidx[c, qbase + eslots] = ev.astype(np.int16)
            qbase += SQ[q]

    # wrap idx arrays
    eidx_w = np.zeros((NCORES, P, sum(SQ) // 16), dtype=np.int16)
    perm_w = np.zeros((NCORES, P, NQ * (PERM_NI // 16)), dtype=np.int16)
    for c in range(NCORES):
        eidx_w[c] = np.tile(_wrap16(eidx[c]), (NGROUP, 1))
        pw = np.concatenate([_wrap16(perms[c, q]) for q in range(NQ)], axis=1)
        perm_w[c] = np.tile(pw, (NGROUP, 1))

    dinvb = np.stack([np.tile(dinv[c * NSH : (c + 1) * NSH], (P, 1)) for c in range(NCORES)])

    meta = dict(layouts=layouts, SQ=SQ, PQ=PQ, PERM_NI=PERM_NI, dinv=dinv)
    k1_inputs = []
    for c in range(NCORES):
        k1_inputs.append(
            {
                "xt": xt,
                "w1": W1.astype(np.float32),
                "b1": b1.astype(np.float32).reshape(P, 1),
                "w2": W2.astype(np.float32),
                "eidx": np.ascontiguousarray(eidx_w[c]),
                "perm": np.ascontiguousarray(perm_w[c]),
                "dinvb": np.ascontiguousarray(dinvb[c].astype(np.float32)),
            }
        )
    return k1_inputs, meta, (src, dst, dinv)


def _concat_aranges(lens):
    """[2,3] -> [0,1,0,1,2]"""
    if len(lens) == 0:
        return np.zeros(0, dtype=np.int64)
    total = int(lens.sum())
    out = np.ones(total, dtype=np.int64)
    ends = np.cumsum(lens)
    out[0] = 0
    out[ends[:-1]] = -(lens[:-1] - 1)
    return np.cumsum(out)


def build_k1(meta, debug_acc=False):
    layouts, SQ, PQ, PERM_NI = meta["layouts"], meta["SQ"], meta["PQ"], meta["PERM_NI"]
    nc = bacc.Bacc(None, target_bir_lowering=False)
    f32, i16 = mybir.dt.float32, mybir.dt.int16
    xt_d = nc.dram_tensor("xt", [P, NQ * QCOLS], f32, kind="ExternalInput")
    w1_d = nc.dram_tensor("w1", [P, P], f32, kind="ExternalInput")
    b1_d = nc.dram_tensor("b1", [P, 1], f32, kind="ExternalInput")
    w2_d = nc.dram_tensor("w2", [P, 1], f32, kind="ExternalInput")
    eidx_d = nc.dram_tensor("eidx", [P, sum(SQ) // 16], i16, kind="ExternalInput")
    perm_d = nc.dram_tensor("perm", [P, NQ * (PERM_NI // 16)], i16, kind="ExternalInput")
    dinvb_d = nc.dram_tensor("dinvb", [P, NSH], f32, kind="ExternalInput")
    zout_d = nc.dram_tensor("zout", [1, NSH], f32, kind="ExternalOutput")
    accout_d = (
        nc.dram_tensor("accout", [P, NSH], f32, kind="ExternalOutput") if debug_acc else None
    )

    with ExitStack() as ctx:
        tc = ctx.enter_context(TileContext(nc))
        cpool = ctx.enter_context(tc.tile_pool(name="cpool", bufs=1))
        apool = ctx.enter_context(tc.tile_pool(name="apool", bufs=1))
        w1 = cpool.tile([P, P], f32)
        b1 = cpool.tile([P, 1], f32)
        w2 = cpool.tile([P, 1], f32)
        eidx = cpool.tile([P, sum(SQ) // 16], i16)
        perm = cpool.tile([P, NQ * (PERM_NI // 16)], i16)
        acc = apool.tile([P, NSH], f32)
        accp = apool.tile([P, PQ], f32)
        nc.sync.dma_start(out=w1[:], in_=w1_d[:])
        nc.sync.dma_start(out=b1[:], in_=b1_d[:])
        nc.sync.dma_start(out=w2[:], in_=w2_d[:])
        nc.sync.dma_start(out=eidx[:], in_=eidx_d[:])
        nc.sync.dma_start(out=perm[:], in_=perm_d[:])
        nc.vector.memset(accp[:, 0:1], 0.0)

        with (
            tc.tile_pool(name="tabs", bufs=2) as tabs,
            tc.tile_pool(name="xpool", bufs=3) as xpool,
            tc.tile_pool(name="gpool", bufs=2) as gpool,
            tc.tile_pool(name="pspool", bufs=2, space="PSUM") as pspool,
        ):
            sq_base = 0
            for q in range(NQ):
                n_chunks, descr, _, _ = layouts[q]
                tab = tabs.tile([P, QCOLS], f32, tag="tab")
                # build quarter table: tab = W1^T @ xt[:, quarter]
                XB = 2 * MMCH  # 1024-col x loads (524KB DMAs)
                for x0 in range(0, QCOLS, XB):
                    xw = min(XB, QCOLS - x0)
                    xc = xpool.tile([P, XB], f32, tag="x")
                    nc.sync.dma_start(
                        out=xc[:, :xw], in_=xt_d[:, q * QCOLS + x0 : q * QCOLS + x0 + xw]
                    )
                    for m0 in range(0, xw, MMCH):
                        ps = pspool.tile([P, MMCH], f32, tag="ps")
                        nc.tensor.matmul(ps[:], w1[:], xc[:, m0 : m0 + MMCH], start=True, stop=True)
                        nc.scalar.activation(
                            tab[:, x0 + m0 : x0 + m0 + MMCH], ps[:],
                            mybir.ActivationFunctionType.Copy,
                        )
                # gather + ladder reduces
                by_chunk = {}
                for d in descr:
                    by_chunk.setdefault(d[0], []).append(d)
                for ch in range(n_chunks):
                    g = gpool.tile([P, G1], f32, tag="g")
                    i0 = (sq_base + ch * G1) // 16
                    nc.gpsimd.ap_gather(
                        g[:], tab[:], eidx[:, i0 : i0 + G1 // 16],
                        channels=P, num_elems=QCOLS, d=1, num_idxs=G1,
                    )
                    for (_, off, n_rows, k, col) in by_chunk.get(ch, []):
                        nc.vector.tensor_reduce(
                            accp[:, col : col + n_rows],
                            g[:, off : off + n_rows * k].rearrange(
                                "p (a b) -> p a b", a=n_rows, b=k
                            ),
                            axis=mybir.AxisListType.X, op=mybir.AluOpType.add,
                        )
                # assemble: acc (+)= accp[perm] in G1-col pieces
                pbase = q * (PERM_NI // 16)
                for s0 in range(0, PERM_NI, G1):
                    w = min(G1, PERM_NI - s0)
                    w = min(w, NSH - s0) if s0 < NSH else 0
                    if w <= 0:
                        break
                    wp = _pad16(w)
                    t = gpool.tile([P, G1], f32, tag="g")
                    nc.gpsimd.ap_gather(
                        t[:, :wp], accp[:], perm[:, pbase + s0 // 16 : pbase + (s0 + wp) // 16],
                        channels=P, num_elems=PQ, d=1, num_idxs=wp,
                    )
                    if q == 0:
                        nc.scalar.activation(
                            acc[:, s0 : s0 + w], t[:, :w], mybir.ActivationFunctionType.Copy
                        )
                    else:
                        nc.vector.tensor_add(acc[:, s0 : s0 + w], acc[:, s0 : s0 + w], t[:, :w])
                sq_base += SQ[q]

        if debug_acc:
            nc.sync.dma_start(out=accout_d[:], in_=acc[:])
        # finalize: h' = dinv*sigmoid(dinv*acc + b1); z' = W2^T @ h'
        with (
            tc.tile_pool(name="fin", bufs=1) as fin,
            tc.tile_pool(name="zpspool", bufs=2, space="PSUM") as zps,
        ):
            dinvb = fin.tile([P, NSH], f32)
            zrow = fin.tile([1, NSH], f32)
            nc.sync.dma_start(out=dinvb[:], in_=dinvb_d[:])
            nc.vector.tensor_mul(acc[:], acc[:], dinvb[:])
            nc.scalar.activation(acc[:], acc[:], mybir.ActivationFunctionType.Sigmoid, bias=b1[:, 0:1])
            nc.vector.tensor_mul(acc[:], acc[:], dinvb[:])
            for m0 in range(0, NSH, MMCH):
                w = min(MMCH, NSH - m0)
                ps = zps.tile([1, MMCH], f32, tag="zps")
                nc.tensor.matmul(ps[:, :w], w2[:], acc[:, m0 : m0 + w], start=True, stop=True)
                nc.scalar.activation(zrow[:, m0 : m0 + w], ps[:, :w], mybir.ActivationFunctionType.Copy)
            nc.sync.dma_start(out=zout_d[:], in_=zrow[:])
    nc.finalize()
    return nc


def host_prep_k2(zfull, src, dst, dinv, b2):
    """Layer-2: scalar gather with 8 independent 16-partition groups."""
    core = dst // NSH
    dstl = dst % NSH
    quarter = src // QN
    srcl = (src % QN).astype(np.int32) + 1
    grp = dstl % NGROUP  # node -> group

    # kappa per (core, quarter, group, node-within-group)
    GN = NSH // NGROUP  # 781.25 -> careful: use dstl//NGROUP as local id (0..781)
    gid = dstl // NGROUP
    GNN = (NSH + NGROUP - 1) // NGROUP  # 782
    kap = np.zeros((NCORES, NQ, NGROUP, GNN), dtype=np.int32)
    for c in range(NCORES):
        mc = core == c
        for q in range(NQ):
            mq = mc & (quarter == q)
            for g in range(NGROUP):
                m = mq & (grp == g)
                kap[c, q, g] = np.bincount(gid[m], minlength=GNN)

    kmax = int(kap.max())
    lut = np.arange(kmax + 1)
    for kk in range(5, kmax + 1):
        for bb in (6, 8, 10, 12, 15, 19, 24, 30, 38, 48, 64, 96, 128, 192, 256):
            if kk <= bb:
                lut[kk] = bb
                break
    kapb = lut[kap]
    budgets, layouts = [], []
    for q in range(NQ):
        b = {}
        for k in np.unique(kapb[:, q, :, :]):
            k = int(k)
            if k == 0:
                continue
            nk = int((kapb[:, q, :, :] == k).sum(axis=2).max())
            if nk > 0:
                b[k] = nk
        budgets.append(b)
        descr, kbase = [], {}
        col = 1
        slots = 0
        for k in sorted(b):
            kbase[k] = col
            descr.append((slots, b[k], k, col))
            slots += b[k] * k
            col += b[k]
        slots = _pad128(slots)
        layouts.append((slots, descr, col, kbase))

    P2 = _pad128(max(l[2] for l in layouts) if layouts else 128)
    SQ2 = [l[0] for l in layouts]

    # z tables: [8, QCOLS2] per quarter, col0=0
    QC2 = QN + 1
    ztab = None
    if zfull is not None:
        ztab = np.zeros((NQ, NGROUP, QC2), dtype=np.float32)
        for q in range(NQ):
            ztab[q, :, 1:] = zfull[q * QN : (q + 1) * QN][None, :]

    eidx2 = np.zeros((NCORES, NGROUP, sum(SQ2)), dtype=np.int16)
    perm2 = np.zeros((NCORES, NGROUP, P2), dtype=np.int16)
    nodemap = np.full((NCORES, NGROUP, P2), -1, dtype=np.int64)  # -> global node
    order = np.lexsort((gid, grp, quarter, core))
    so, go_, qo, co, gi = srcl[order], grp[order], quarter[order], core[order], gid[order]
    for c in range(NCORES):
        for g in range(NGROUP):
            qbase = 0
            for q in range(NQ):
                m = (co == c) & (go_ == g) & (qo == q)
                s_e, gi_e = so[m], gi[m]
                kv = kap[c, q, g]
                kvb = lut[kv]
                nodes = np.nonzero(kv)[0]
                kn = kv[nodes]
                knb = kvb[nodes]
                nd = np.lexsort((nodes, knb))
                nodes_s, kn_s, knb_s = nodes[nd], kn[nd], knb[nd]
                _, descr, _, kbase = layouts[q]
                rank = np.zeros(len(nodes_s), dtype=np.int64)
                colof = np.zeros(len(nodes_s), dtype=np.int64)
                for k in np.unique(knb_s):
                    mk = knb_s == k
                    rank[mk] = np.arange(mk.sum())
                    colof[mk] = kbase[int(k)]
                node_col = colof + rank
                col2slot = np.full(layouts[q][2], -1, dtype=np.int64)
                for soff, n_rows, k, col in descr:
                    cols = np.arange(n_rows)
                    col2slot[col + cols] = soff + cols * k
                starts = col2slot[node_col]
                eslots = np.repeat(starts, kn_s) + _concat_aranges(kn_s)
                ptr = np.zeros(GNN + 1, dtype=np.int64)
                ptr[1:] = np.cumsum(kv)
                ev = (
                    np.concatenate([s_e[ptr[n] : ptr[n + 1]] for n in nodes_s])
                    if len(nodes_s)
                    else np.zeros(0, dtype=np.int32)
                )
                eidx2[c, g, qbase + eslots] = ev.astype(np.int16)
                qbase += SQ2[q]
                # perm for this quarter accumulates into same node cols later;
                # here: node n (local gid) col in accp_q
                # we need per-quarter perms; store packed later
            # perms built per quarter below

    # per-quarter perms + final node mapping
    perm2q = np.zeros((NCORES, NGROUP, NQ, P2), dtype=np.int16)
    for c in range(NCORES):
        for g in range(NGROUP):
            for q in range(NQ):
                kv = kap[c, q, g]
                kvb = lut[kv]
                nodes = np.nonzero(kv)[0]
                knb = kvb[nodes]
                nd = np.lexsort((nodes, knb))
                nodes_s, knb_s = nodes[nd], knb[nd]
                _, _, _, kbase = layouts[q]
                rank = np.zeros(len(nodes_s), dtype=np.int64)
                colof = np.zeros(len(nodes_s), dtype=np.int64)
                for k in np.unique(knb_s):
                    mk = knb_s == k
                    rank[mk] = np.arange(mk.sum())
                    colof[mk] = kbase[int(k)]
                pm = np.zeros(GNN, dtype=np.int16)
                pm[nodes_s] = (colof + rank).astype(np.int16)
                perm2q[c, g, q, :GNN] = pm
            for j in range(GNN):
                n_global = (c * NSH) + (j * NGROUP + g)
                if j * NGROUP + g < NSH:
                    nodemap[c, g, j] = n_global

    # wrapped arrays
    eidx2_w = np.zeros((NCORES, P, sum(SQ2) // 16), dtype=np.int16)
    perm2_w = np.zeros((NCORES, P, NQ * (P2 // 16)), dtype=np.int16)
    for c in range(NCORES):
        for g in range(NGROUP):
            eidx2_w[c, g * 16 : (g + 1) * 16] = _wrap16(eidx2[c, g])
            perm2_w[c, g * 16 : (g + 1) * 16] = np.concatenate(
                [_wrap16(perm2q[c, g, q]) for q in range(NQ)], axis=1
            )

    dinvP = np.zeros((NCORES, NGROUP, P2), dtype=np.float32)
    for c in range(NCORES):
        for g in range(NGROUP):
            for j in range(GNN):
                n = j * NGROUP + g
                if n < NSH:
                    dinvP[c, g, j] = dinv[c * NSH + n]

    meta2 = dict(layouts=layouts, SQ2=SQ2, P2=P2, QC2=QC2, nodemap=nodemap, b2=float(b2[0]))
    k2_inputs = []
    for c in range(NCORES):
        d = {
            "eidx2": np.ascontiguousarray(eidx2_w[c]),
            "perm2": np.ascontiguousarray(perm2_w[c]),
            "dinvp": np.ascontiguousarray(dinvP[c]),
        }
        if ztab is not None:
            d["ztab"] = np.ascontiguousarray(ztab.reshape(NQ * NGROUP, QC2))
        k2_inputs.append(d)
    return k2_inputs, meta2


def build_k2(meta2):
    layouts, SQ2, P2, QC2 = meta2["layouts"], meta2["SQ2"], meta2["P2"], meta2["QC2"]
    b2 = meta2["b2"]
    nc = bacc.Bacc(None, target_bir_lowering=False)
    f32, i16 = mybir.dt.float32, mybir.dt.int16
    ztab_d = nc.dram_tensor("ztab", [NQ * NGROUP, QC2], f32, kind="ExternalInput")
    eidx_d = nc.dram_tensor("eidx2", [P, sum(SQ2) // 16], i16, kind="ExternalInput")
    perm_d = nc.dram_tensor("perm2", [P, NQ * (P2 // 16)], i16, kind="ExternalInput")
    dinvp_d = nc.dram_tensor("dinvp", [NGROUP, P2], f32, kind="ExternalInput")
    out_d = nc.dram_tensor("out2", [NGROUP, P2], f32, kind="ExternalOutput")

    with ExitStack() as ctx:
        tc = ctx.enter_context(TileContext(nc))
        pool = ctx.enter_context(tc.tile_pool(name="pool", bufs=1))
        gpool = ctx.enter_context(tc.tile_pool(name="g2", bufs=3))
        eidx = pool.tile([P, sum(SQ2) // 16], i16)
        perm = pool.tile([P, NQ * (P2 // 16)], i16)
        acc = pool.tile([P, P2], f32)
        accp = pool.tile([P, P2], f32)
        dinvp = pool.tile([P, P2], f32)
        nc.sync.dma_start(out=eidx[:], in_=eidx_d[:])
        nc.sync.dma_start(out=perm[:], in_=perm_d[:])
        nc.sync.dma_start(out=dinvp[0:NGROUP * 16:16, :], in_=dinvp_d[:])
        nc.vector.memset(accp[:, 0:1], 0.0)

        with tc.tile_pool(name="ztabs", bufs=2) as ztabs:
            sq_base = 0
            for q in range(NQ):
                slots, descr, _, _ = layouts[q]
                zt = ztabs.tile([P, QC2], f32, tag="zt")
                nc.sync.dma_start(
                    out=zt[0:NGROUP * 16:16, :], in_=ztab_d[q * NGROUP : (q + 1) * NGROUP, :]
                )
                g = gpool.tile([P, max(_pad16(max(SQ2)), 16)], f32, tag="g")
                nc.gpsimd.ap_gather(
                    g[:, :slots], zt[:], eidx[:, sq_base // 16 : (sq_base + slots) // 16],
                    channels=P, num_elems=QC2, d=1, num_idxs=slots,
                )
                for soff, n_rows, k, col in descr:
                    nc.vector.tensor_reduce(
                        accp[:, col : col + n_rows],
                        g[:, soff : soff + n_rows * k].rearrange("p (a b) -> p a b", a=n_rows, b=k),
                        axis=mybir.AxisListType.X, op=mybir.AluOpType.add,
                    )
                t = gpool.tile([P, max(_pad16(max(SQ2)), 16)], f32, tag="g")
                nc.gpsimd.ap_gather(
                    t[:, :P2], accp[:], perm[:, q * (P2 // 16) : (q + 1) * (P2 // 16)],
                    channels=P, num_elems=P2, d=1, num_idxs=P2,
                )
                if q == 0:
                    nc.vector.tensor_copy(acc[:], t[:, :P2])
                else:
                    nc.vector.tensor_add(acc[:], acc[:], t[:, :P2])
                sq_base += slots

        nc.vector.tensor_mul(acc[:], acc[:], dinvp[:])
        nc.scalar.activation(acc[:], acc[:], mybir.ActivationFunctionType.Sigmoid, bias=b2)
        nc.sync.dma_start(out=out_d[:], in_=acc[0:NGROUP * 16:16, :])
    nc.finalize()
    return nc


def _sim_ns(nc):
    from concourse import bass_interp

    sim = bass_interp.CoreSim(nc, no_exec=True, publish_trace=False)
    sim.simulate()
    return int(sim.time)


def build_fused(meta, meta2):
    """Single-launch: layer 1 + on-device AllGather of z' + layer 2."""
    layouts, SQ, PQ, PERM_NI = meta["layouts"], meta["SQ"], meta["PQ"], meta["PERM_NI"]
    layouts2, SQ2, P2, QC2 = meta2["layouts"], meta2["SQ2"], meta2["P2"], meta2["QC2"]
    b2 = meta2["b2"]
    nc = bacc.Bacc(None, target_bir_lowering=False)
    f32, i16 = mybir.dt.float32, mybir.dt.int16
    xt_d = nc.dram_tensor("xt", [P, NQ * QCOLS], f32, kind="ExternalInput")
    w1_d = nc.dram_tensor("w1", [P, P], f32, kind="ExternalInput")
    b1_d = nc.dram_tensor("b1", [P, 1], f32, kind="ExternalInput")
    w2_d = nc.dram_tensor("w2", [P, 1], f32, kind="ExternalInput")
    eidx_d = nc.dram_tensor("eidx", [P, sum(SQ) // 16], i16, kind="ExternalInput")
    perm_d = nc.dram_tensor("perm", [P, NQ * (PERM_NI // 16)], i16, kind="ExternalInput")
    dinvb_d = nc.dram_tensor("dinvb", [P, NSH], f32, kind="ExternalInput")
    eidx2_d = nc.dram_tensor("eidx2", [P, sum(SQ2) // 16], i16, kind="ExternalInput")
    perm2_d = nc.dram_tensor("perm2", [P, NQ * (P2 // 16)], i16, kind="ExternalInput")
    dinvp_d = nc.dram_tensor("dinvp", [NGROUP, P2], f32, kind="ExternalInput")
    out_d = nc.dram_tensor("out2", [NGROUP, P2], f32, kind="ExternalOutput")

    with ExitStack() as ctx:
        tc = ctx.enter_context(TileContext(nc))
        cpool = ctx.enter_context(tc.tile_pool(name="cpool", bufs=1))
        dram = ctx.enter_context(tc.tile_pool(name="dram", bufs=1, space="DRAM"))
        w1 = cpool.tile([P, P], f32)
        b1 = cpool.tile([P, 1], f32)
        w2 = cpool.tile([P, 1], f32)
        eidx = cpool.tile([P, sum(SQ) // 16], i16)
        perm = cpool.tile([P, NQ * (PERM_NI // 16)], i16)
        zin = nc.dram_tensor("zin_cc", [NGROUP, NSH], f32, kind="Internal")
        zall = nc.dram_tensor("zall_cc", [NGROUP * NCORES, NSH], f32, kind="Internal", addr_space="Shared")
        nc.sync.dma_start(out=w1[:], in_=w1_d[:])
        nc.sync.dma_start(out=b1[:], in_=b1_d[:])
        nc.sync.dma_start(out=w2[:], in_=w2_d[:])
        nc.sync.dma_start(out=eidx[:], in_=eidx_d[:])
        nc.sync.dma_start(out=perm[:], in_=perm_d[:])

        with tc.tile_pool(name="apool", bufs=1) as apool:
            acc = apool.tile([P, NSH], f32)
            accp = apool.tile([P, PQ], f32)
            nc.vector.memset(accp[:, 0:1], 0.0)
            with (
                tc.tile_pool(name="tabs", bufs=2) as tabs,
                tc.tile_pool(name="xpool", bufs=2) as xpool,
                tc.tile_pool(name="gpool", bufs=2) as gpool,
                tc.tile_pool(name="pspool", bufs=2, space="PSUM") as pspool,
            ):
                sq_base = 0
                for q in range(NQ):
                    n_chunks, descr, _, _ = layouts[q]
                    tab = tabs.tile([P, QCOLS], f32, tag="tab")
                    XB = 2 * MMCH
                    for x0 in range(0, QCOLS, XB):
                        xw = min(XB, QCOLS - x0)
                        xc = xpool.tile([P, XB], f32, tag="x")
                        nc.sync.dma_start(
                            out=xc[:, :xw], in_=xt_d[:, q * QCOLS + x0 : q * QCOLS + x0 + xw]
                        )
                        for m0 in range(0, xw, MMCH):
                            ps = pspool.tile([P, MMCH], f32, tag="ps")
                            nc.tensor.matmul(ps[:], w1[:], xc[:, m0 : m0 + MMCH], start=True, stop=True)
                            nc.scalar.activation(
                                tab[:, x0 + m0 : x0 + m0 + MMCH], ps[:],
                                mybir.ActivationFunctionType.Copy,
                            )
                    by_chunk = {}
                    for d_ in descr:
                        by_chunk.setdefault(d_[0], []).append(d_)
                    for ch in range(n_chunks):
                        g = gpool.tile([P, G1], f32, tag="g")
                        i0 = (sq_base + ch * G1) // 16
                        nc.gpsimd.ap_gather(
                            g[:], tab[:], eidx[:, i0 : i0 + G1 // 16],
                            channels=P, num_elems=QCOLS, d=1, num_idxs=G1,
                        )
                        for (_, off, n_rows, k, col) in by_chunk.get(ch, []):
                            nc.vector.tensor_reduce(
                                accp[:, col : col + n_rows],
                                g[:, off : off + n_rows * k].rearrange(
                                    "p (a b) -> p a b", a=n_rows, b=k
                                ),
                                axis=mybir.AxisListType.X, op=mybir.AluOpType.add,
                            )
                    pbase = q * (PERM_NI // 16)
                    for s0 in range(0, PERM_NI, G1):
                        w = min(G1, PERM_NI - s0)
                        w = min(w, NSH - s0) if s0 < NSH else 0
                        if w <= 0:
                            break
                        wp = _pad16(w)
                        t = gpool.tile([P, G1], f32, tag="g")
                        nc.gpsimd.ap_gather(
                            t[:, :wp], accp[:], perm[:, pbase + s0 // 16 : pbase + (s0 + wp) // 16],
                            channels=P, num_elems=PQ, d=1, num_idxs=wp,
                        )
                        if q == 0:
                            nc.scalar.activation(
                                acc[:, s0 : s0 + w], t[:, :w], mybir.ActivationFunctionType.Copy
                            )
                        else:
                            nc.vector.tensor_add(acc[:, s0 : s0 + w], acc[:, s0 : s0 + w], t[:, :w])
                    sq_base += SQ[q]

            with (
                tc.tile_pool(name="fin", bufs=1) as fin,
                tc.tile_pool(name="zpspool", bufs=2, space="PSUM") as zps,
            ):
                dinvb = fin.tile([P, NSH], f32)
                zrow = fin.tile([1, NSH], f32)
                nc.sync.dma_start(out=dinvb[:], in_=dinvb_d[:])
                for f0 in range(0, NSH, G1):
                    fw = min(G1, NSH - f0)
                    sl = slice(f0, f0 + fw)
                    nc.vector.tensor_mul(acc[:, sl], acc[:, sl], dinvb[:, sl])
                    nc.scalar.activation(
                        acc[:, sl], acc[:, sl], mybir.ActivationFunctionType.Sigmoid, bias=b1[:, 0:1]
                    )
                    nc.vector.tensor_mul(acc[:, sl], acc[:, sl], dinvb[:, sl])
                for m0 in range(0, NSH, MMCH):
                    w = min(MMCH, NSH - m0)
                    ps = zps.tile([1, MMCH], f32, tag="zps")
                    nc.tensor.matmul(ps[:, :w], w2[:], acc[:, m0 : m0 + w], start=True, stop=True)
                    nc.scalar.activation(zrow[:, m0 : m0 + w], ps[:, :w], mybir.ActivationFunctionType.Copy)
                for g_ in range(NGROUP):
                    nc.sync.dma_start(out=zin[g_ : g_ + 1, :], in_=zrow[:])

        nc.gpsimd.collective_compute(
            "AllGather", mybir.AluOpType.bypass,
            replica_groups=[list(range(NCORES))],
            ins=[zin[:].opt()], outs=[zall[:].opt()],
        )

        # ---- layer 2 ----
        with (
            tc.tile_pool(name="k2pool", bufs=1) as pool2,
            tc.tile_pool(name="ztabs", bufs=2) as ztabs,
            tc.tile_pool(name="g2", bufs=3) as gpool2,
        ):
            eidx2 = pool2.tile([P, sum(SQ2) // 16], i16)
            perm2 = pool2.tile([P, NQ * (P2 // 16)], i16)
            acc2 = pool2.tile([P, P2], f32)
            accp2 = pool2.tile([P, P2], f32)
            dinvp = pool2.tile([P, P2], f32)
            nc.sync.dma_start(out=eidx2[:], in_=eidx2_d[:])
            nc.sync.dma_start(out=perm2[:], in_=perm2_d[:])
            nc.sync.dma_start(out=dinvp[0 : NGROUP * 16 : 16, :], in_=dinvp_d[:])
            nc.vector.memset(accp2[:, 0:1], 0.0)
            GSZ = max(_pad16(max(SQ2)), P2, 16)
            half = NSH
            sq_base = 0
            for q in range(NQ):
                slots, descr, _, _ = layouts2[q]
                zt = ztabs.tile([P, QC2], f32, tag="zt")
                nc.vector.memset(zt[:, 0:1], 0.0)
                nc.sync.dma_start(
                    out=zt[0 : NGROUP * 16 : 16, 1 : 1 + half],
                    in_=zall[NGROUP * (2 * q) : NGROUP * (2 * q) + NGROUP, :],
                )
                nc.sync.dma_start(
                    out=zt[0 : NGROUP * 16 : 16, 1 + half : 1 + 2 * half],
                    in_=zall[NGROUP * (2 * q + 1) : NGROUP * (2 * q + 1) + NGROUP, :],
                )
                g = gpool2.tile([P, GSZ], f32, tag="g")
                nc.gpsimd.ap_gather(
                    g[:, :slots], zt[:], eidx2[:, sq_base // 16 : (sq_base + slots) // 16],
                    channels=P, num_elems=QC2, d=1, num_idxs=slots,
                )
                for soff, n_rows, k, col in descr:
                    nc.vector.tensor_reduce(
                        accp2[:, col : col + n_rows],
                        g[:, soff : soff + n_rows * k].rearrange("p (a b) -> p a b", a=n_rows, b=k),
                        axis=mybir.AxisListType.X, op=mybir.AluOpType.add,
                    )
                t = gpool2.tile([P, GSZ], f32, tag="g")
                nc.gpsimd.ap_gather(
                    t[:, :P2], accp2[:], perm2[:, q * (P2 // 16) : (q + 1) * (P2 // 16)],
                    channels=P, num_elems=P2, d=1, num_idxs=P2,
                )
                if q == 0:
                    nc.scalar.activation(acc2[:], t[:, :P2], mybir.ActivationFunctionType.Copy)
                else:
                    nc.vector.tensor_add(acc2[:], acc2[:], t[:, :P2])
                sq_base += slots

            nc.vector.tensor_mul(acc2[:], acc2[:], dinvp[:])
            nc.scalar.activation(acc2[:], acc2[:], mybir.ActivationFunctionType.Sigmoid, bias=b2)
            nc.sync.dma_start(out=out_d[:], in_=acc2[0 : NGROUP * 16 : 16, :])
    nc.finalize()
    return nc


def _assemble_out(results, meta2):
    out = np.zeros((N, 1), dtype=np.float32)
    nodemap = meta2["nodemap"]
    for c in range(NCORES):
        o = results[c]["out2"]  # [8, P2]
        valid = nodemap[c] >= 0
        out[nodemap[c][valid], 0] = o[valid]
    return out


def kernel(x, edge_index, W1, b1, W2, b2):
    global LAST_SIM_NS
    x = np.asarray(x, dtype=np.float32)
    edge_index = np.asarray(edge_index)
    k1_inputs, meta, (src, dst, dinv) = host_prep(x, edge_index, W1, b1, W2, b2)
    b2np = np.asarray(b2, dtype=np.float32)
    try:
        # single launch: layer1 + AllGather(z') + layer2 fused in one NEFF
        k2_inputs, meta2 = host_prep_k2(None, src, dst, dinv, b2np)
        nc = build_fused(meta, meta2)
        if MEASURE:
            LAST_SIM_NS = _sim_ns(nc)
        in_maps = [dict(k1_inputs[c], **k2_inputs[c]) for c in range(NCORES)]
        res = run_bass_kernel_spmd(nc, in_maps, list(range(NCORES)))
        return _assemble_out(res.results, meta2)
    except Exception:
        import traceback

        traceback.print_exc()

    # fallback: two launches with z' crossing via host
    nc1 = build_k1(meta)
    sim1 = _sim_ns(nc1) if MEASURE else 0
    res1 = run_bass_kernel_spmd(nc1, k1_inputs, list(range(NCORES)))
    zfull = np.concatenate([res1.results[c]["zout"][0, :NSH] for c in range(NCORES)])
    k2_inputs, meta2 = host_prep_k2(zfull, src, dst, dinv, b2np)
    nc2 = build_k2(meta2)
    if MEASURE:
        LAST_SIM_NS = sim1 + _sim_ns(nc2)
    res2 = run_bass_kernel_spmd(nc2, k2_inputs, list(range(NCORES)))
    return _assemble_out(res2.results, meta2)



# revision 43
# speedup vs baseline: 2.7098x; 2.7098x over previous
"""2-layer GCN (PyG GCNConv x2 + sigmoid) on 8 TRN2 NeuronCores, single fused NEFF.

Design notes (cost-model driven):
- ap_gather costs max(table_cols, num_idxs)*0.833ns -> tables and gather
  chunks must be size-matched. 7 src-quarters (table=7144 cols) with 2
  ~8K-slot chunks each keeps L1 gathers slot-optimal (~0.84ns/edge).
- Exact-degree shared (max-over-core) ladders; k=1 rows are Act-engine
  copies instead of DVE reduces; accumulator acc and both dinv_dst
  multiplies run in bf16 on DVE (2x mode); tables built by PE in bf16.
- Layer 2: the 8 GPSIMD 16-partition groups each own one SRC CORE RANGE so
  per-group z tables are 6256 wide (table-cost-minimal); self-loops are
  excluded from the edge stream (their term is zrow itself, added at the
  end); cross-group partial sums contract on the PE via a ones vector.
- z' AllGather on-device (DRAM bounce); finalize sliced to shrink the
  serial tail into the collective.
"""

import sys

sys.path.insert(0, "/opt/trn_rl_repo")
import numpy as np
import ml_dtypes
from contextlib import ExitStack

from concourse import bacc, mybir
from concourse.tile import TileContext
from concourse.bass_utils import run_bass_kernel_spmd

MEASURE = False
LAST_SIM_NS = None

N = 50000
E = 800000
F = 128
P = 128
NCORES = 8
NSH = N // NCORES  # 6250
NQ = 7
QN = 7143  # nodes per quarter (last has 7142)
T = QN + 1  # 7144: [zero col, up to 7143 nodes]
NP_ = 6256  # padded per-core node count
MMCH = 512
XB = 2048


def _wrap16(idx_flat):
    n = idx_flat.shape[0]
    assert n % 16 == 0
    return np.ascontiguousarray(idx_flat.reshape(n // 16, 16).T)


def _pad16(n):
    return ((n + 15) // 16) * 16


def _concat_aranges(lens):
    if len(lens) == 0:
        return np.zeros(0, dtype=np.int64)
    total = int(lens.sum())
    out = np.ones(total, dtype=np.int64)
    ends = np.cumsum(lens)
    out[0] = 0
    out[ends[:-1]] = -(lens[:-1] - 1)
    return np.cumsum(out)


def _ladder_layout(kap_by_unit, n_chunks_cap):
    """kap_by_unit: [n_units, n_nodes]. Shared exact-k ladder with row-aligned
    chunks. Returns (descr[(chunk, off, n_rows, k, col)], cols, kbase,
    chunk_sizes)."""
    kmax = int(kap_by_unit.max())
    budgets = {}
    for k in range(1, kmax + 1):
        nk = int((kap_by_unit == k).sum(axis=1).max())
        if nk > 0:
            budgets[k] = nk
    raw = sum(k * nk for k, nk in budgets.items())
    cap = raw + 64 if n_chunks_cap is None else (raw + n_chunks_cap - 1) // n_chunks_cap + 48
    descr, kbase = [], {}
    col = 1
    ch, off = 0, 0
    for k in sorted(budgets):
        nk = budgets[k]
        kbase[k] = col
        left = nk
        while left > 0:
            fit = min(left, (cap - off) // k)
            if fit == 0:
                ch += 1
                off = 0
                fit = min(left, cap // k)
            descr.append((ch, off, fit, k, col))
            off += fit * k
            col += fit
            left -= fit
    chunk_sizes = {}
    for c, o, nr, k, _ in descr:
        chunk_sizes[c] = max(chunk_sizes.get(c, 0), o + nr * k)
    sizes = [_pad16(chunk_sizes[c]) for c in sorted(chunk_sizes)]
    return descr, col, kbase, sizes


def _pack_slots(kap, srcl_by_dst, dstl_by_dst, descr, kbase, cols, chunk_offs):
    """kap: [n_nodes] this unit's degrees; srcl/dstl: this unit's edges sorted
    by dst. Returns (slot_positions, slot_values, perm[node->accp col])."""
    nodes = np.nonzero(kap)[0]
    kn = kap[nodes]
    nd = np.lexsort((nodes, kn))
    nodes_s, kn_s = nodes[nd], kn[nd]
    rank = np.zeros(len(nodes_s), dtype=np.int64)
    colof = np.zeros(len(nodes_s), dtype=np.int64)
    for k in np.unique(kn_s):
        mk = kn_s == k
        rank[mk] = np.arange(mk.sum())
        colof[mk] = kbase[int(k)]
    node_col = colof + rank
    col2slot = np.full(cols, -1, dtype=np.int64)
    for ch, off, n_rows, k, col in descr:
        cc = np.arange(n_rows)
        col2slot[col + cc] = chunk_offs[ch] + off + cc * k
    starts = col2slot[node_col]
    eslots = np.repeat(starts, kn_s) + _concat_aranges(kn_s)
    # edge values in (k, node) order: stable sort of dst-sorted edges by k
    eo = np.argsort(kap[dstl_by_dst], kind="stable")
    ev = srcl_by_dst[eo]
    pm = np.zeros(len(kap), dtype=np.int16)
    pm[nodes_s] = node_col.astype(np.int16)
    return eslots, ev, pm


def host_prep(x, edge_index, W1, b1, W2, b2):
    src = np.concatenate([edge_index[0], np.arange(N, dtype=np.int64)]).astype(np.int32)
    dst = np.concatenate([edge_index[1], np.arange(N, dtype=np.int64)]).astype(np.int32)
    deg = np.bincount(dst, minlength=N).astype(np.float32)
    dinv = 1.0 / np.sqrt(np.maximum(deg, 1e-12))
    dinv[deg <= 0] = 0.0

    # Node -> table position. Stratified round-robin: nodes with identical
    # per-core in-degree vectors spread evenly over quarters, which tightens
    # the shared (max-over-core) ladder budgets vs a random permutation.
    degc = np.zeros((N, NCORES), dtype=np.int32)
    dst_t = np.concatenate([edge_index[1], np.arange(N, dtype=np.int64)])
    src_t = np.concatenate([edge_index[0], np.arange(N, dtype=np.int64)])
    np.add.at(degc, (src_t, dst_t // NSH), 1)
    okey = np.lexsort(tuple(degc[:, c] for c in range(NCORES)))
    rank = np.empty(N, dtype=np.int64)
    rank[okey] = np.arange(N)
    psrc = (rank % NQ) * QN + rank // NQ  # node -> table position
    assert psrc.max() < NQ * QN
    pinv = np.argsort(psrc)

    xtp = (x * dinv[:, None]).T.astype(np.float32)[:, pinv]  # [128, N] pos order
    xt = np.zeros((P, NQ * T), dtype=ml_dtypes.bfloat16)
    for q in range(NQ):
        qn = min(QN, N - q * QN)
        xt[:, q * T + 1 : q * T + 1 + qn] = xtp[:, q * QN : q * QN + qn].astype(
            ml_dtypes.bfloat16
        )

    core = dst // NSH
    dstl = (dst % NSH).astype(np.int64)
    pos = psrc[src]
    quarter = pos // QN
    srcl = (pos % QN).astype(np.int64) + 1

    flat = (core.astype(np.int64) * NQ + quarter) * NSH + dstl
    kap = np.bincount(flat, minlength=NCORES * NQ * NSH).reshape(NCORES, NQ, NSH)

    layouts = []
    for q in range(NQ):
        descr, cols, kbase, sizes = _ladder_layout(kap[:, q, :], 2)
        offs = np.concatenate([[0], np.cumsum(sizes)]).astype(np.int64)
        layouts.append((descr, cols, kbase, sizes, offs))
    SQ = [int(l[4][-1]) for l in layouts]
    PQ = _pad16(max(l[1] for l in layouts))
    G0 = max(max(l[3]) for l in layouts)

    order = np.lexsort((dstl, quarter, core))
    so, do_, qo, co = srcl[order], dstl[order], quarter[order], core[order]

    # combined per-quarter index stream: [SQ[q] slot idxs | NP_ perm idxs]
    qoff = np.concatenate([[0], np.cumsum([s + NP_ for s in SQ])]).astype(np.int64)
    qbase = np.concatenate([[0], np.cumsum(SQ)]).astype(np.int64)
    eidx = np.zeros((NCORES, int(qoff[-1])), dtype=np.int16)
    for c in range(NCORES):
        mc = co == c
        for q in range(NQ):
            m = mc & (qo == q)
            descr, cols, kbase, sizes, offs = layouts[q]
            eslots, ev, pm = _pack_slots(
                kap[c, q], so[m], do_[m], descr, kbase, cols, offs
            )
            eidx[c, qoff[q] + eslots] = ev.astype(np.int16)
            eidx[c, qoff[q] + SQ[q] : qoff[q] + SQ[q] + NSH] = pm

    eidx_w = np.zeros((NCORES, P, int(qoff[-1]) // 16), dtype=np.int16)
    for c in range(NCORES):
        eidx_w[c] = np.tile(_wrap16(eidx[c]), (8, 1))

    dinvb = np.zeros((NCORES, P, NP_), dtype=ml_dtypes.bfloat16)
    dinvrow = np.zeros((NCORES, 1, NP_), dtype=np.float32)
    for c in range(NCORES):
        dv = dinv[c * NSH : (c + 1) * NSH]
        dinvb[c, :, :NSH] = np.tile(dv.astype(ml_dtypes.bfloat16)[None, :], (P, 1))
        dinvrow[c, 0, :NSH] = dv

    meta = dict(layouts=layouts, SQ=SQ, PQ=PQ, G0=G0, qbase=qbase, qoff=qoff)
    k1_inputs = []
    for c in range(NCORES):
        k1_inputs.append(
            {
                "xt": xt,
                "w1": np.asarray(W1, dtype=ml_dtypes.bfloat16),
                "b1": np.asarray(b1, dtype=np.float32).reshape(P, 1),
                "w2": np.asarray(W2, dtype=ml_dtypes.bfloat16).reshape(P, 1),
                "eidx": np.ascontiguousarray(eidx_w[c]),
                "dinvb": np.ascontiguousarray(dinvb[c]),
                "dinvrow": np.ascontiguousarray(dinvrow[c]),
            }
        )
    return k1_inputs, meta, (src, dst, dinv)


def host_prep_k2(src, dst):
    """Layer 2: 8 GPSIMD groups = 8 src core ranges; self-loops excluded."""
    m = src != dst
    s2, d2 = src[m].astype(np.int64), dst[m].astype(np.int64)
    c2 = d2 // NSH
    g2 = s2 // NSH
    dstl = d2 % NSH
    srcl = s2 % NSH + 1

    flat = (c2 * NCORES + g2) * NSH + dstl
    kap2 = np.bincount(flat, minlength=NCORES * NCORES * NSH).reshape(
        NCORES * NCORES, NSH
    )
    descr2, cols2, kbase2, sizes2 = _ladder_layout(kap2, 2)
    offs2 = np.concatenate([[0], np.cumsum(sizes2)]).astype(np.int64)
    slots2 = int(offs2[-1])
    P2 = _pad16(cols2)

    order = np.lexsort((dstl, g2, c2))
    so, do_, go, co = srcl[order], dstl[order], g2[order], c2[order]
    eidx2 = np.zeros((NCORES, NCORES, slots2), dtype=np.int16)
    perm2 = np.zeros((NCORES, NCORES, NP_), dtype=np.int16)
    for c in range(NCORES):
        mc = co == c
        for g in range(NCORES):
            mm = mc & (go == g)
            eslots, ev, pm = _pack_slots(
                kap2[c * NCORES + g], so[mm], do_[mm], descr2, kbase2, cols2, offs2
            )
            eidx2[c, g, eslots] = ev.astype(np.int16)
            perm2[c, g, :NSH] = pm

    eidx2_w = np.zeros((NCORES, P, slots2 // 16), dtype=np.int16)
    perm2_w = np.zeros((NCORES, P, NP_ // 16), dtype=np.int16)
    for c in range(NCORES):
        for g in range(NCORES):
            eidx2_w[c, g * 16 : (g + 1) * 16] = _wrap16(eidx2[c, g])
            perm2_w[c, g * 16 : (g + 1) * 16] = _wrap16(perm2[c, g])

    svec = np.zeros((P, 1), dtype=ml_dtypes.bfloat16)
    svec[0:P:16, 0] = 1.0  # sum the 8 group-partial rows

    meta2 = dict(descr2=descr2, P2=P2, slots2=slots2, sizes2=sizes2, offs2=offs2)
    k2_inputs = []
    for c in range(NCORES):
        k2_inputs.append(
            {
                "eidx2": np.ascontiguousarray(eidx2_w[c]),
                "perm2": np.ascontiguousarray(perm2_w[c]),
                "svec": svec,
            }
        )
    return k2_inputs, meta2


def build_fused(meta, meta2, b2val):
    layouts, SQ, PQ, G0, qbase, qoff = (
        meta["layouts"],
        meta["SQ"],
        meta["PQ"],
        meta["G0"],
        meta["qbase"],
        meta["qoff"],
    )
    ITW = (max(SQ) + NP_) // 16  # combined per-quarter idx tile width
    descr2, P2, slots2 = meta2["descr2"], meta2["P2"], meta2["slots2"]
    sizes2, offs2 = meta2["sizes2"], meta2["offs2"]
    G2 = max(sizes2)

    nc = bacc.Bacc(None, target_bir_lowering=False)
    f32, f32r, bf16, i16 = (
        mybir.dt.float32,
        mybir.dt.float32r,
        mybir.dt.bfloat16,
        mybir.dt.int16,
    )

    xt_d = nc.dram_tensor("xt", [P, NQ * T], bf16, kind="ExternalInput")
    w1_d = nc.dram_tensor("w1", [P, P], bf16, kind="ExternalInput")
    b1_d = nc.dram_tensor("b1", [P, 1], f32, kind="ExternalInput")
    w2_d = nc.dram_tensor("w2", [P, 1], bf16, kind="ExternalInput")
    eidx_d = nc.dram_tensor("eidx", [P, int(qoff[-1]) // 16], i16, kind="ExternalInput")
    dinvb_d = nc.dram_tensor("dinvb", [P, NP_], bf16, kind="ExternalInput")
    dinvrow_d = nc.dram_tensor("dinvrow", [1, NP_], f32, kind="ExternalInput")
    eidx2_d = nc.dram_tensor("eidx2", [P, slots2 // 16], i16, kind="ExternalInput")
    perm2_d = nc.dram_tensor("perm2", [P, NP_ // 16], i16, kind="ExternalInput")
    svec_d = nc.dram_tensor("svec", [P, 1], bf16, kind="ExternalInput")
    out_d = nc.dram_tensor("out", [1, NP_], f32, kind="ExternalOutput")

    zin = nc.dram_tensor("zin_cc", [1, NSH], f32, kind="Internal")
    zall = nc.dram_tensor(
        "zall_cc", [NCORES, NSH], f32, kind="Internal", addr_space="Shared"
    )

    Copy = mybir.ActivationFunctionType.Copy
    Sigmoid = mybir.ActivationFunctionType.Sigmoid
    ADD = mybir.AluOpType.add

    with ExitStack() as ctx:
        tc = ctx.enter_context(TileContext(nc))
        cpool = ctx.enter_context(tc.tile_pool(name="cpool", bufs=1))
        w1 = cpool.tile([P, P], bf16)
        b1 = cpool.tile([P, 1], f32)
        w2 = cpool.tile([P, 1], bf16)
        acc = cpool.tile([P, NP_], bf16)
        warm = cpool.tile([1, 16], f32)
        nc.sync.dma_start(out=w1[:], in_=w1_d[:])
        nc.sync.dma_start(out=b1[:], in_=b1_d[:])
        nc.sync.dma_start(out=w2[:], in_=w2_d[:])
        # preload the sigmoid activation table off the critical path
        nc.vector.memset(warm[:], 0.0)
        nc.scalar.activation(warm[:], warm[:], Sigmoid, bias=0.0)

        with (
            tc.tile_pool(name="tabs", bufs=2) as tabs,
            tc.tile_pool(name="xpool", bufs=2) as xpool,
            tc.tile_pool(name="gpool", bufs=2) as gpool,
            tc.tile_pool(name="tpool", bufs=1) as tpool,
            tc.tile_pool(name="accpool", bufs=2) as accpool,
            tc.tile_pool(name="epool", bufs=2) as epool,
            tc.tile_pool(name="pspool", bufs=2, space="PSUM") as pspool,
        ):
            accps = {}

            def assemble(q):
                # perm-gather quarter q's partials to node order and fold
                # into acc (emitted one quarter late to keep Pool saturated)
                accp_q, it_q = accps.pop(q)
                p0 = SQ[q] // 16
                tt = gpool.tile([P, G0], f32, tag="g")
                nc.gpsimd.ap_gather(
                    tt[:, :NP_],
                    accp_q[:],
                    it_q[:, p0 : p0 + NP_ // 16],
                    channels=P,
                    num_elems=PQ,
                    d=1,
                    num_idxs=NP_,
                )
                if q == 0:
                    nc.scalar.activation(acc[:], tt[:, :NP_], Copy)
                elif q < NQ - 1:
                    tb = tpool.tile([P, NP_], bf16, tag="tb")
                    nc.scalar.activation(tb[:], tt[:, :NP_], Copy)
                    nc.vector.tensor_add(acc[:], acc[:], tb[:])
                else:
                    # last quarter: slice so finalize can start per-slice
                    tb = tpool.tile([P, NP_], bf16, tag="tb")
                    for s0 in range(0, NP_, 1564):
                        sl = slice(s0, s0 + 1564)
                        nc.scalar.activation(tb[:, sl], tt[:, sl], Copy)
                        nc.vector.tensor_add(acc[:, sl], acc[:, sl], tb[:, sl])

            for q in range(NQ):
                descr, cols, kbase, sizes, offs = layouts[q]
                tab = tabs.tile([P, T], f32, tag="tab")
                chunks = [512, 512, 1024] if q == 0 else []
                x0 = sum(chunks)
                while x0 < T:
                    chunks.append(min(XB, T - x0))
                    x0 += chunks[-1]
                x0 = 0
                for xw in chunks:
                    xc = xpool.tile([P, XB], bf16, tag="x")
                    nc.sync.dma_start(
                        out=xc[:, :xw], in_=xt_d[:, q * T + x0 : q * T + x0 + xw]
                    )
                    ps = pspool.tile([P, XB], f32, tag="ps")
                    for m0 in range(0, xw, MMCH):
                        mw = min(MMCH, xw - m0)
                        nc.tensor.matmul(
                            ps[:, m0 : m0 + mw],
                            w1[:],
                            xc[:, m0 : m0 + mw],
                            start=True,
                            stop=True,
                        )
                    nc.scalar.activation(tab[:, x0 : x0 + xw], ps[:, :xw], Copy)
                    x0 += xw
                accp = accpool.tile([P, PQ], f32, tag="accp")
                it = epool.tile([P, ITW], i16, tag="it")
                qw = (SQ[q] + NP_) // 16
                i0 = int(qoff[q]) // 16
                nc.sync.dma_start(out=it[:, :qw], in_=eidx_d[:, i0 : i0 + qw])
                accps[q] = (accp, it)
                nc.vector.memset(accp[:, 0:1], 0.0)
                by_chunk = {}
                for d_ in descr:
                    by_chunk.setdefault(d_[0], []).append(d_)
                for ci, ch in enumerate(sorted(by_chunk)):
                    sz = sizes[ch]
                    c0 = int(offs[ch]) // 16
                    g = gpool.tile([P, G0], f32, tag="g")
                    nc.gpsimd.ap_gather(
                        g[:, :sz],
                        tab[:],
                        it[:, c0 : c0 + sz // 16],
                        channels=P,
                        num_elems=T,
                        d=1,
                        num_idxs=sz,
                    )
                    if ci == 0 and q > 0:
                        assemble(q - 1)
                    for _, off, n_rows, k, col in by_chunk[ch]:
                        if k == 1 and ci == 0:
                            # Act handles chunk-0 k=1 rows; later chunks go to
                            # DVE so Act isn't blocked ahead of next tab build
                            nc.scalar.activation(
                                accp[:, col : col + n_rows],
                                g[:, off : off + n_rows],
                                Copy,
                            )
                        elif k == 1:
                            nc.vector.tensor_copy(
                                accp[:, col : col + n_rows], g[:, off : off + n_rows]
                            )
                        else:
                            nc.vector.tensor_reduce(
                                accp[:, col : col + n_rows],
                                g[:, off : off + n_rows * k].rearrange(
                                    "p (a b) -> p a b", a=n_rows, b=k
                                ),
                                axis=mybir.AxisListType.X,
                                op=ADD,
                            )
                if q == NQ - 1:
                    assemble(q)

        # finalize (4 column slices): h' = dinv*sigmoid(dinv*acc+b1); z=W2^T h'
        with (
            tc.tile_pool(name="fin", bufs=1) as fin,
            tc.tile_pool(name="zps", bufs=2, space="PSUM") as zps,
        ):
            zrow = fin.tile([1, NP_], f32)
            dinvrow2 = fin.tile([1, NP_], f32)
            nc.sync.dma_start(out=dinvrow2[:], in_=dinvrow_d[:])
            with tc.tile_pool(name="finb", bufs=1) as finb:
                dinvb = finb.tile([P, NP_], bf16)
                nc.sync.dma_start(out=dinvb[:], in_=dinvb_d[:])
                bounds = [0, 1536, 3072, 4608, NP_]
                for si in range(4):
                    sl = slice(bounds[si], bounds[si + 1])
                    nc.vector.tensor_mul(acc[:, sl], acc[:, sl], dinvb[:, sl])
                    nc.scalar.activation(
                        acc[:, sl], acc[:, sl], Sigmoid, bias=b1[:, 0:1]
                    )
                    nc.vector.tensor_mul(acc[:, sl], acc[:, sl], dinvb[:, sl])
                    for m0 in range(bounds[si], bounds[si + 1], MMCH):
                        mw = min(MMCH, bounds[si + 1] - m0)
                        ps = zps.tile([1, MMCH], f32, tag="zp")
                        nc.tensor.matmul(
                            ps[:, :mw],
                            w2[:],
                            acc[:, m0 : m0 + mw],
                            start=True,
                            stop=True,
                        )
                        nc.scalar.activation(zrow[:, m0 : m0 + mw], ps[:, :mw], Copy)
                nc.sync.dma_start(out=zin[:, : NSH // 2], in_=zrow[:, : NSH // 2])
                nc.sync.dma_start(out=zin[:, NSH // 2 :], in_=zrow[:, NSH // 2 : NSH])

            nc.gpsimd.collective_compute(
                "AllGather",
                mybir.AluOpType.bypass,
                replica_groups=[list(range(NCORES))],
                ins=[zin[:].opt()],
                outs=[zall[:].opt()],
            )

            # ---- layer 2 ----
            with (
                tc.tile_pool(name="k2pool", bufs=1) as pool2,
                tc.tile_pool(name="zps2", bufs=2, space="PSUM") as zps2,
            ):
                eidx2 = pool2.tile([P, slots2 // 16], i16)
                perm2 = pool2.tile([P, NP_ // 16], i16)
                accp2 = pool2.tile([P, P2], f32)
                svec = pool2.tile([P, 1], bf16)
                zfin = pool2.tile([1, NP_], f32)
                nc.sync.dma_start(out=eidx2[:], in_=eidx2_d[:])
                nc.sync.dma_start(out=perm2[:], in_=perm2_d[:])
                nc.sync.dma_start(out=svec[:], in_=svec_d[:])
                nc.vector.memset(accp2[:, 0:1], 0.0)
                with tc.tile_pool(name="ztpool", bufs=1) as ztpool, tc.tile_pool(
                    name="g2pool", bufs=2
                ) as g2pool:
                    zt = ztpool.tile([P, NP_], f32)
                    nc.vector.memset(zt[:, 0:1], 0.0)
                    nc.sync.dma_start(out=zt[0:P:16, 1 : 1 + NSH], in_=zall[:, :])
                    by_chunk2 = {}
                    for d_ in descr2:
                        by_chunk2.setdefault(d_[0], []).append(d_)
                    for ch in sorted(by_chunk2):
                        sz = sizes2[ch]
                        g2 = g2pool.tile([P, G2], f32, tag="g2")
                        i0 = int(offs2[ch]) // 16
                        nc.gpsimd.ap_gather(
                            g2[:, :sz],
                            zt[:],
                            eidx2[:, i0 : i0 + sz // 16],
                            channels=P,
                            num_elems=NP_,
                            d=1,
                            num_idxs=sz,
                        )
                        for _, off, n_rows, k, col in by_chunk2[ch]:
                            if k == 1:
                                nc.scalar.activation(
                                    accp2[:, col : col + n_rows],
                                    g2[:, off : off + n_rows],
                                    Copy,
                                )
                            elif k == 2:
                                # split pair-adds between GPSIMD and DVE
                                nh = _pad16(n_rows * 2 // 5)
                                nc.gpsimd.tensor_add(
                                    accp2[:, col : col + nh],
                                    g2[:, off : off + 2 * nh].rearrange(
                                        "p (a b) -> p a b", a=nh, b=2
                                    )[:, :, 0],
                                    g2[:, off : off + 2 * nh].rearrange(
                                        "p (a b) -> p a b", a=nh, b=2
                                    )[:, :, 1],
                                )
                                nc.vector.tensor_reduce(
                                    accp2[:, col + nh : col + n_rows],
                                    g2[:, off + 2 * nh : off + 2 * n_rows].rearrange(
                                        "p (a b) -> p a b", a=n_rows - nh, b=2
                                    ),
                                    axis=mybir.AxisListType.X,
                                    op=ADD,
                                )
                            else:
                                nc.vector.tensor_reduce(
                                    accp2[:, col : col + n_rows],
                                    g2[:, off : off + n_rows * k].rearrange(
                                        "p (a b) -> p a b", a=n_rows, b=k
                                    ),
                                    axis=mybir.AxisListType.X,
                                    op=ADD,
                                )
                with tc.tile_pool(name="gp2", bufs=1) as gp2pool:
                    g2p = gp2pool.tile([P, NP_], f32)
                    g2pb = gp2pool.tile([P, NP_], bf16)
                    nc.gpsimd.ap_gather(
                        g2p[:],
                        accp2[:],
                        perm2[:],
                        channels=P,
                        num_elems=P2,
                        d=1,
                        num_idxs=NP_,
                    )
                    HB = NP_ // 2
                    for s0 in (0, HB):
                        nc.scalar.activation(
                            g2pb[:, s0 : s0 + HB], g2p[:, s0 : s0 + HB], Copy
                        )
                        for m0 in range(s0, s0 + HB, MMCH):
                            mw = min(MMCH, s0 + HB - m0)
                            ps = zps2.tile([1, MMCH], f32, tag="zp2")
                            nc.tensor.matmul(
                                ps[:, :mw],
                                svec[:],
                                g2pb[:, m0 : m0 + mw],
                                start=True,
                                stop=True,
                            )
                            nc.scalar.activation(zfin[:, m0 : m0 + mw], ps[:, :mw], Copy)
                            # += self-loop term, then * dinv_dst
                            nc.vector.tensor_add(
                                zfin[:, m0 : m0 + mw],
                                zfin[:, m0 : m0 + mw],
                                zrow[:, m0 : m0 + mw],
                            )
                            nc.vector.tensor_mul(
                                zfin[:, m0 : m0 + mw],
                                zfin[:, m0 : m0 + mw],
                                dinvrow2[:, m0 : m0 + mw],
                            )
                        nc.scalar.activation(
                            zfin[:, s0 : s0 + HB],
                            zfin[:, s0 : s0 + HB],
                            Sigmoid,
                            bias=float(b2val),
                        )
                        nc.sync.dma_start(
                            out=out_d[:, s0 : s0 + HB], in_=zfin[:, s0 : s0 + HB]
                        )
    nc.finalize()
    return nc


def _sim_ns(nc):
    from concourse import bass_interp

    sim = bass_interp.CoreSim(nc, no_exec=True, publish_trace=False)
    sim.simulate()
    return int(sim.time)


def kernel(x, edge_index, W1, b1, W2, b2):
    global LAST_SIM_NS
    x = np.asarray(x, dtype=np.float32)
    edge_index = np.asarray(edge_index)
    k1_inputs, meta, (src, dst, dinv) = host_prep(x, edge_index, W1, b1, W2, b2)
    k2_inputs, meta2 = host_prep_k2(src, dst)
    b2val = float(np.asarray(b2, dtype=np.float32).reshape(-1)[0])
    nc = build_fused(meta, meta2, b2val)
    if MEASURE:
        LAST_SIM_NS = _sim_ns(nc)
    in_maps = [dict(k1_inputs[c], **k2_inputs[c]) for c in range(NCORES)]
    res = run_bass_kernel_spmd(nc, in_maps, list(range(NCORES)))
    out = np.zeros((N, 1), dtype=np.float32)
    for c in range(NCORES):
        out[c * NSH : (c + 1) * NSH, 0] = res.results[c]["out"][0, :NSH]
    return out


# revision 47
# speedup vs baseline: 2.7169x; 1.0026x over previous
"""2-layer GCN (PyG GCNConv x2 + sigmoid) on 8 TRN2 NeuronCores, single fused NEFF.

Design notes (cost-model driven):
- ap_gather costs max(table_cols, num_idxs)*0.833ns -> tables and gather
  chunks must be size-matched. 7 src-quarters (table=7144 cols) with 2
  ~8K-slot chunks each keeps L1 gathers slot-optimal (~0.84ns/edge).
- Exact-degree shared (max-over-core) ladders; k=1 rows are Act-engine
  copies instead of DVE reduces; accumulator acc and both dinv_dst
  multiplies run in bf16 on DVE (2x mode); tables built by PE in bf16.
- Layer 2: the 8 GPSIMD 16-partition groups each own one SRC CORE RANGE so
  per-group z tables are 6256 wide (table-cost-minimal); self-loops are
  excluded from the edge stream (their term is zrow itself, added at the
  end); cross-group partial sums contract on the PE via a ones vector.
- z' AllGather on-device (DRAM bounce); finalize sliced to shrink the
  serial tail into the collective.
"""

import sys

sys.path.insert(0, "/opt/trn_rl_repo")
import numpy as np
import ml_dtypes
from contextlib import ExitStack

from concourse import bacc, mybir
from concourse.tile import TileContext
from concourse.bass_utils import run_bass_kernel_spmd

MEASURE = False
LAST_SIM_NS = None

N = 50000
E = 800000
F = 128
P = 128
NCORES = 8
NSH = N // NCORES  # 6250
NQ = 7
QN = 7143  # nodes per quarter (last has 7142)
T = QN + 1  # 7144: [zero col, up to 7143 nodes]
NP_ = 6256  # padded per-core node count
MMCH = 512
XB = 2048


def _wrap16(idx_flat):
    n = idx_flat.shape[0]
    assert n % 16 == 0
    return np.ascontiguousarray(idx_flat.reshape(n // 16, 16).T)


def _pad16(n):
    return ((n + 15) // 16) * 16


def _concat_aranges(lens):
    if len(lens) == 0:
        return np.zeros(0, dtype=np.int64)
    total = int(lens.sum())
    out = np.ones(total, dtype=np.int64)
    ends = np.cumsum(lens)
    out[0] = 0
    out[ends[:-1]] = -(lens[:-1] - 1)
    return np.cumsum(out)


def _ladder_layout(kap_by_unit, n_chunks_cap, kdesc=False):
    """kap_by_unit: [n_units, n_nodes]. Shared exact-k ladder with row-aligned
    chunks, big k first (heavy reduces overlap the next chunk's gather).
    Returns (descr[(chunk, off, n_rows, k, col)], cols, kbase, chunk_sizes)."""
    kmax = int(kap_by_unit.max())
    budgets = {}
    for k in range(1, kmax + 1):
        nk = int((kap_by_unit == k).sum(axis=1).max())
        if nk > 0:
            budgets[k] = nk
    raw = sum(k * nk for k, nk in budgets.items())
    cap = raw + 64 if n_chunks_cap is None else (raw + n_chunks_cap - 1) // n_chunks_cap + 48
    descr, kbase = [], {}
    col = 1
    ch, off = 0, 0
    for k in sorted(budgets, reverse=kdesc):
        nk = budgets[k]
        kbase[k] = col
        left = nk
        while left > 0:
            fit = min(left, (cap - off) // k)
            if fit == 0:
                ch += 1
                off = 0
                fit = min(left, cap // k)
            descr.append((ch, off, fit, k, col))
            off += fit * k
            col += fit
            left -= fit
    chunk_sizes = {}
    for c, o, nr, k, _ in descr:
        chunk_sizes[c] = max(chunk_sizes.get(c, 0), o + nr * k)
    sizes = [_pad16(chunk_sizes[c]) for c in sorted(chunk_sizes)]
    return descr, col, kbase, sizes


def _pack_slots(kap, srcl_by_dst, dstl_by_dst, descr, kbase, cols, chunk_offs):
    """kap: [n_nodes] this unit's degrees; srcl/dstl: this unit's edges sorted
    by dst. Returns (slot_positions, slot_values, perm[node->accp col])."""
    nodes = np.nonzero(kap)[0]
    kn = kap[nodes]
    nd = np.lexsort((nodes, kn))
    nodes_s, kn_s = nodes[nd], kn[nd]
    rank = np.zeros(len(nodes_s), dtype=np.int64)
    colof = np.zeros(len(nodes_s), dtype=np.int64)
    for k in np.unique(kn_s):
        mk = kn_s == k
        rank[mk] = np.arange(mk.sum())
        colof[mk] = kbase[int(k)]
    node_col = colof + rank
    col2slot = np.full(cols, -1, dtype=np.int64)
    for ch, off, n_rows, k, col in descr:
        cc = np.arange(n_rows)
        col2slot[col + cc] = chunk_offs[ch] + off + cc * k
    starts = col2slot[node_col]
    eslots = np.repeat(starts, kn_s) + _concat_aranges(kn_s)
    # edge values in (k, node) order: stable sort of dst-sorted edges by k
    eo = np.argsort(kap[dstl_by_dst], kind="stable")
    ev = srcl_by_dst[eo]
    pm = np.zeros(len(kap), dtype=np.int16)
    pm[nodes_s] = node_col.astype(np.int16)
    return eslots, ev, pm


def host_prep(x, edge_index, W1, b1, W2, b2):
    src = np.concatenate([edge_index[0], np.arange(N, dtype=np.int64)]).astype(np.int32)
    dst = np.concatenate([edge_index[1], np.arange(N, dtype=np.int64)]).astype(np.int32)
    deg = np.bincount(dst, minlength=N).astype(np.float32)
    dinv = 1.0 / np.sqrt(np.maximum(deg, 1e-12))
    dinv[deg <= 0] = 0.0

    # Node -> table position. Stratified round-robin: nodes with identical
    # per-core in-degree vectors spread evenly over quarters, which tightens
    # the shared (max-over-core) ladder budgets vs a random permutation.
    degc = np.zeros((N, NCORES), dtype=np.int32)
    dst_t = np.concatenate([edge_index[1], np.arange(N, dtype=np.int64)])
    src_t = np.concatenate([edge_index[0], np.arange(N, dtype=np.int64)])
    np.add.at(degc, (src_t, dst_t // NSH), 1)
    okey = np.lexsort(tuple(degc[:, c] for c in range(NCORES)))
    rank = np.empty(N, dtype=np.int64)
    rank[okey] = np.arange(N)
    psrc = (rank % NQ) * QN + rank // NQ  # node -> table position
    assert psrc.max() < NQ * QN
    pinv = np.argsort(psrc)

    xtp = (x * dinv[:, None]).T.astype(np.float32)[:, pinv]  # [128, N] pos order
    xt = np.zeros((P, NQ * T), dtype=ml_dtypes.bfloat16)
    for q in range(NQ):
        qn = min(QN, N - q * QN)
        xt[:, q * T + 1 : q * T + 1 + qn] = xtp[:, q * QN : q * QN + qn].astype(
            ml_dtypes.bfloat16
        )

    core = dst // NSH
    dstl = (dst % NSH).astype(np.int64)
    pos = psrc[src]
    quarter = pos // QN
    srcl = (pos % QN).astype(np.int64) + 1

    flat = (core.astype(np.int64) * NQ + quarter) * NSH + dstl
    kap = np.bincount(flat, minlength=NCORES * NQ * NSH).reshape(NCORES, NQ, NSH)

    layouts = []
    for q in range(NQ):
        descr, cols, kbase, sizes = _ladder_layout(kap[:, q, :], 2)
        offs = np.concatenate([[0], np.cumsum(sizes)]).astype(np.int64)
        layouts.append((descr, cols, kbase, sizes, offs))
    SQ = [int(l[4][-1]) for l in layouts]
    PQ = _pad16(max(l[1] for l in layouts))
    G0 = max(max(l[3]) for l in layouts)

    order = np.lexsort((dstl, quarter, core))
    so, do_, qo, co = srcl[order], dstl[order], quarter[order], core[order]

    # combined per-quarter index stream: [SQ[q] slot idxs | NP_ perm idxs]
    qoff = np.concatenate([[0], np.cumsum([s + NP_ for s in SQ])]).astype(np.int64)
    qbase = np.concatenate([[0], np.cumsum(SQ)]).astype(np.int64)
    eidx = np.zeros((NCORES, int(qoff[-1])), dtype=np.int16)
    for c in range(NCORES):
        mc = co == c
        for q in range(NQ):
            m = mc & (qo == q)
            descr, cols, kbase, sizes, offs = layouts[q]
            eslots, ev, pm = _pack_slots(
                kap[c, q], so[m], do_[m], descr, kbase, cols, offs
            )
            eidx[c, qoff[q] + eslots] = ev.astype(np.int16)
            eidx[c, qoff[q] + SQ[q] : qoff[q] + SQ[q] + NSH] = pm

    eidx_w = np.zeros((NCORES, P, int(qoff[-1]) // 16), dtype=np.int16)
    for c in range(NCORES):
        eidx_w[c] = np.tile(_wrap16(eidx[c]), (8, 1))

    dinvb = np.zeros((NCORES, P, NP_), dtype=ml_dtypes.bfloat16)
    dinvrow = np.zeros((NCORES, 1, NP_), dtype=np.float32)
    for c in range(NCORES):
        dv = dinv[c * NSH : (c + 1) * NSH]
        dinvb[c, :, :NSH] = np.tile(dv.astype(ml_dtypes.bfloat16)[None, :], (P, 1))
        dinvrow[c, 0, :NSH] = dv

    meta = dict(layouts=layouts, SQ=SQ, PQ=PQ, G0=G0, qbase=qbase, qoff=qoff)
    k1_inputs = []
    for c in range(NCORES):
        k1_inputs.append(
            {
                "xt": xt,
                "w1": np.asarray(W1, dtype=ml_dtypes.bfloat16),
                "b1": np.asarray(b1, dtype=np.float32).reshape(P, 1),
                "w2": np.asarray(W2, dtype=ml_dtypes.bfloat16).reshape(P, 1),
                "eidx": np.ascontiguousarray(eidx_w[c]),
                "dinvb": np.ascontiguousarray(dinvb[c]),
                "dinvrow": np.ascontiguousarray(dinvrow[c]),
            }
        )
    return k1_inputs, meta, (src, dst, dinv)


def host_prep_k2(src, dst):
    """Layer 2: 8 GPSIMD groups = 8 src core ranges; self-loops excluded."""
    m = src != dst
    s2, d2 = src[m].astype(np.int64), dst[m].astype(np.int64)
    c2 = d2 // NSH
    g2 = s2 // NSH
    dstl = d2 % NSH
    srcl = s2 % NSH + 1

    flat = (c2 * NCORES + g2) * NSH + dstl
    kap2 = np.bincount(flat, minlength=NCORES * NCORES * NSH).reshape(
        NCORES * NCORES, NSH
    )
    descr2, cols2, kbase2, sizes2 = _ladder_layout(kap2, 2, kdesc=True)
    offs2 = np.concatenate([[0], np.cumsum(sizes2)]).astype(np.int64)
    slots2 = int(offs2[-1])
    P2 = _pad16(cols2)

    order = np.lexsort((dstl, g2, c2))
    so, do_, go, co = srcl[order], dstl[order], g2[order], c2[order]
    eidx2 = np.zeros((NCORES, NCORES, slots2), dtype=np.int16)
    perm2 = np.zeros((NCORES, NCORES, NP_), dtype=np.int16)
    for c in range(NCORES):
        mc = co == c
        for g in range(NCORES):
            mm = mc & (go == g)
            eslots, ev, pm = _pack_slots(
                kap2[c * NCORES + g], so[mm], do_[mm], descr2, kbase2, cols2, offs2
            )
            eidx2[c, g, eslots] = ev.astype(np.int16)
            perm2[c, g, :NSH] = pm

    eidx2_w = np.zeros((NCORES, P, slots2 // 16), dtype=np.int16)
    perm2_w = np.zeros((NCORES, P, NP_ // 16), dtype=np.int16)
    for c in range(NCORES):
        for g in range(NCORES):
            eidx2_w[c, g * 16 : (g + 1) * 16] = _wrap16(eidx2[c, g])
            perm2_w[c, g * 16 : (g + 1) * 16] = _wrap16(perm2[c, g])

    svec = np.zeros((P, 1), dtype=ml_dtypes.bfloat16)
    svec[0:P:16, 0] = 1.0  # sum the 8 group-partial rows

    meta2 = dict(descr2=descr2, P2=P2, slots2=slots2, sizes2=sizes2, offs2=offs2)
    k2_inputs = []
    for c in range(NCORES):
        k2_inputs.append(
            {
                "eidx2": np.ascontiguousarray(eidx2_w[c]),
                "perm2": np.ascontiguousarray(perm2_w[c]),
                "svec": svec,
            }
        )
    return k2_inputs, meta2


def build_fused(meta, meta2, b2val):
    layouts, SQ, PQ, G0, qbase, qoff = (
        meta["layouts"],
        meta["SQ"],
        meta["PQ"],
        meta["G0"],
        meta["qbase"],
        meta["qoff"],
    )
    ITW = (max(SQ) + NP_) // 16  # combined per-quarter idx tile width
    descr2, P2, slots2 = meta2["descr2"], meta2["P2"], meta2["slots2"]
    sizes2, offs2 = meta2["sizes2"], meta2["offs2"]
    G2 = max(sizes2)

    nc = bacc.Bacc(None, target_bir_lowering=False)
    f32, f32r, bf16, i16 = (
        mybir.dt.float32,
        mybir.dt.float32r,
        mybir.dt.bfloat16,
        mybir.dt.int16,
    )

    xt_d = nc.dram_tensor("xt", [P, NQ * T], bf16, kind="ExternalInput")
    w1_d = nc.dram_tensor("w1", [P, P], bf16, kind="ExternalInput")
    b1_d = nc.dram_tensor("b1", [P, 1], f32, kind="ExternalInput")
    w2_d = nc.dram_tensor("w2", [P, 1], bf16, kind="ExternalInput")
    eidx_d = nc.dram_tensor("eidx", [P, int(qoff[-1]) // 16], i16, kind="ExternalInput")
    dinvb_d = nc.dram_tensor("dinvb", [P, NP_], bf16, kind="ExternalInput")
    dinvrow_d = nc.dram_tensor("dinvrow", [1, NP_], f32, kind="ExternalInput")
    eidx2_d = nc.dram_tensor("eidx2", [P, slots2 // 16], i16, kind="ExternalInput")
    perm2_d = nc.dram_tensor("perm2", [P, NP_ // 16], i16, kind="ExternalInput")
    svec_d = nc.dram_tensor("svec", [P, 1], bf16, kind="ExternalInput")
    out_d = nc.dram_tensor("out", [1, NP_], f32, kind="ExternalOutput")

    zin = nc.dram_tensor("zin_cc", [1, NSH], f32, kind="Internal")
    zall = nc.dram_tensor(
        "zall_cc", [NCORES, NSH], f32, kind="Internal", addr_space="Shared"
    )

    Copy = mybir.ActivationFunctionType.Copy
    Sigmoid = mybir.ActivationFunctionType.Sigmoid
    ADD = mybir.AluOpType.add

    with ExitStack() as ctx:
        tc = ctx.enter_context(TileContext(nc))
        cpool = ctx.enter_context(tc.tile_pool(name="cpool", bufs=1))
        w1 = cpool.tile([P, P], bf16)
        b1 = cpool.tile([P, 1], f32)
        w2 = cpool.tile([P, 1], bf16)
        acc = cpool.tile([P, NP_], bf16)
        warm = cpool.tile([1, 16], f32)
        nc.sync.dma_start(out=w1[:], in_=w1_d[:])
        nc.sync.dma_start(out=b1[:], in_=b1_d[:])
        nc.sync.dma_start(out=w2[:], in_=w2_d[:])
        # preload the sigmoid activation table off the critical path
        nc.vector.memset(warm[:], 0.0)
        nc.scalar.activation(warm[:], warm[:], Sigmoid, bias=0.0)

        with (
            tc.tile_pool(name="tabs", bufs=2) as tabs,
            tc.tile_pool(name="xpool", bufs=2) as xpool,
            tc.tile_pool(name="gpool", bufs=2) as gpool,
            tc.tile_pool(name="tpool", bufs=1) as tpool,
            tc.tile_pool(name="accpool", bufs=2) as accpool,
            tc.tile_pool(name="epool", bufs=2) as epool,
            tc.tile_pool(name="pspool", bufs=2, space="PSUM") as pspool,
        ):
            accps = {}

            def assemble(q):
                # perm-gather quarter q's partials to node order and fold
                # into acc (emitted one quarter late to keep Pool saturated)
                accp_q, it_q = accps.pop(q)
                p0 = SQ[q] // 16
                tt = gpool.tile([P, G0], f32, tag="g")
                nc.gpsimd.ap_gather(
                    tt[:, :NP_],
                    accp_q[:],
                    it_q[:, p0 : p0 + NP_ // 16],
                    channels=P,
                    num_elems=PQ,
                    d=1,
                    num_idxs=NP_,
                )
                if q == 0:
                    nc.scalar.activation(acc[:], tt[:, :NP_], Copy)
                elif q < NQ - 1:
                    tb = tpool.tile([P, NP_], bf16, tag="tb")
                    nc.scalar.activation(tb[:], tt[:, :NP_], Copy)
                    nc.vector.tensor_add(acc[:], acc[:], tb[:])
                else:
                    # last quarter: slice so finalize can start per-slice
                    tb = tpool.tile([P, NP_], bf16, tag="tb")
                    for s0 in range(0, NP_, 1564):
                        sl = slice(s0, s0 + 1564)
                        nc.scalar.activation(tb[:, sl], tt[:, sl], Copy)
                        nc.vector.tensor_add(acc[:, sl], acc[:, sl], tb[:, sl])

            for q in range(NQ):
                descr, cols, kbase, sizes, offs = layouts[q]
                it = epool.tile([P, ITW], i16, tag="it")
                qw = (SQ[q] + NP_) // 16
                i0 = int(qoff[q]) // 16
                nc.sync.dma_start(out=it[:, :qw], in_=eidx_d[:, i0 : i0 + qw])
                tab = tabs.tile([P, T], f32, tag="tab")
                chunks = [512, 512, 1024] if q == 0 else []
                x0 = sum(chunks)
                while x0 < T:
                    chunks.append(min(XB, T - x0))
                    x0 += chunks[-1]
                x0 = 0
                for xw in chunks:
                    xc = xpool.tile([P, XB], bf16, tag="x")
                    nc.sync.dma_start(
                        out=xc[:, :xw], in_=xt_d[:, q * T + x0 : q * T + x0 + xw]
                    )
                    ps = pspool.tile([P, XB], f32, tag="ps")
                    for m0 in range(0, xw, MMCH):
                        mw = min(MMCH, xw - m0)
                        nc.tensor.matmul(
                            ps[:, m0 : m0 + mw],
                            w1[:],
                            xc[:, m0 : m0 + mw],
                            start=True,
                            stop=True,
                        )
                    nc.scalar.activation(tab[:, x0 : x0 + xw], ps[:, :xw], Copy)
                    x0 += xw
                accp = accpool.tile([P, PQ], f32, tag="accp")
                accps[q] = (accp, it)
                nc.vector.memset(accp[:, 0:1], 0.0)
                by_chunk = {}
                for d_ in descr:
                    by_chunk.setdefault(d_[0], []).append(d_)
                for ci, ch in enumerate(sorted(by_chunk)):
                    sz = sizes[ch]
                    c0 = int(offs[ch]) // 16
                    g = gpool.tile([P, G0], f32, tag="g")
                    nc.gpsimd.ap_gather(
                        g[:, :sz],
                        tab[:],
                        it[:, c0 : c0 + sz // 16],
                        channels=P,
                        num_elems=T,
                        d=1,
                        num_idxs=sz,
                    )
                    if ci == 0 and q > 0:
                        assemble(q - 1)
                    for _, off, n_rows, k, col in by_chunk[ch]:
                        if k == 1 and ci == 0:
                            # Act handles chunk-0 k=1 rows; later chunks go to
                            # DVE so Act isn't blocked ahead of next tab build
                            nc.scalar.activation(
                                accp[:, col : col + n_rows],
                                g[:, off : off + n_rows],
                                Copy,
                            )
                        elif k == 1:
                            nc.vector.tensor_copy(
                                accp[:, col : col + n_rows], g[:, off : off + n_rows]
                            )
                        else:
                            nc.vector.tensor_reduce(
                                accp[:, col : col + n_rows],
                                g[:, off : off + n_rows * k].rearrange(
                                    "p (a b) -> p a b", a=n_rows, b=k
                                ),
                                axis=mybir.AxisListType.X,
                                op=ADD,
                            )
                if q == NQ - 1:
                    assemble(q)

        # finalize (4 column slices): h' = dinv*sigmoid(dinv*acc+b1); z=W2^T h'
        with (
            tc.tile_pool(name="fin", bufs=1) as fin,
            tc.tile_pool(name="zps", bufs=2, space="PSUM") as zps,
        ):
            zrow = fin.tile([1, NP_], f32)
            dinvrow2 = fin.tile([1, NP_], f32)
            nc.sync.dma_start(out=dinvrow2[:], in_=dinvrow_d[:])
            with tc.tile_pool(name="finb", bufs=1) as finb:
                dinvb = finb.tile([P, NP_], bf16)
                nc.sync.dma_start(out=dinvb[:], in_=dinvb_d[:])
                bounds = [0, 1536, 3072, 4608, NP_]
                for si in range(4):
                    sl = slice(bounds[si], bounds[si + 1])
                    nc.vector.tensor_mul(acc[:, sl], acc[:, sl], dinvb[:, sl])
                    nc.scalar.activation(
                        acc[:, sl], acc[:, sl], Sigmoid, bias=b1[:, 0:1]
                    )
                    nc.vector.tensor_mul(acc[:, sl], acc[:, sl], dinvb[:, sl])
                    for m0 in range(bounds[si], bounds[si + 1], MMCH):
                        mw = min(MMCH, bounds[si + 1] - m0)
                        ps = zps.tile([1, MMCH], f32, tag="zp")
                        nc.tensor.matmul(
                            ps[:, :mw],
                            w2[:],
                            acc[:, m0 : m0 + mw],
                            start=True,
                            stop=True,
                        )
                        nc.scalar.activation(zrow[:, m0 : m0 + mw], ps[:, :mw], Copy)
                nc.sync.dma_start(out=zin[:, : NSH // 2], in_=zrow[:, : NSH // 2])
                nc.sync.dma_start(out=zin[:, NSH // 2 :], in_=zrow[:, NSH // 2 : NSH])

            nc.gpsimd.collective_compute(
                "AllGather",
                mybir.AluOpType.bypass,
                replica_groups=[list(range(NCORES))],
                ins=[zin[:].opt()],
                outs=[zall[:].opt()],
            )

            # ---- layer 2 ----
            with (
                tc.tile_pool(name="k2pool", bufs=1) as pool2,
                tc.tile_pool(name="zps2", bufs=2, space="PSUM") as zps2,
            ):
                eidx2 = pool2.tile([P, slots2 // 16], i16)
                perm2 = pool2.tile([P, NP_ // 16], i16)
                accp2 = pool2.tile([P, P2], f32)
                svec = pool2.tile([P, 1], bf16)
                zfin = pool2.tile([1, NP_], f32)
                nc.sync.dma_start(out=eidx2[:], in_=eidx2_d[:])
                nc.sync.dma_start(out=perm2[:], in_=perm2_d[:])
                nc.sync.dma_start(out=svec[:], in_=svec_d[:])
                nc.vector.memset(accp2[:, 0:1], 0.0)
                with tc.tile_pool(name="ztpool", bufs=1) as ztpool, tc.tile_pool(
                    name="g2pool", bufs=2
                ) as g2pool:
                    zt = ztpool.tile([P, NP_], f32)
                    nc.vector.memset(zt[:, 0:1], 0.0)
                    nc.sync.dma_start(out=zt[0:P:16, 1 : 1 + NSH], in_=zall[:, :])
                    by_chunk2 = {}
                    for d_ in descr2:
                        by_chunk2.setdefault(d_[0], []).append(d_)
                    for ch in sorted(by_chunk2):
                        sz = sizes2[ch]
                        g2 = g2pool.tile([P, G2], f32, tag="g2")
                        i0 = int(offs2[ch]) // 16
                        nc.gpsimd.ap_gather(
                            g2[:, :sz],
                            zt[:],
                            eidx2[:, i0 : i0 + sz // 16],
                            channels=P,
                            num_elems=NP_,
                            d=1,
                            num_idxs=sz,
                        )
                        for _, off, n_rows, k, col in by_chunk2[ch]:
                            if k == 1:
                                nc.scalar.activation(
                                    accp2[:, col : col + n_rows],
                                    g2[:, off : off + n_rows],
                                    Copy,
                                )
                            elif k == 2:
                                # split pair-adds between GPSIMD and DVE
                                nh = _pad16(n_rows * 2 // 5)
                                nc.gpsimd.tensor_add(
                                    accp2[:, col : col + nh],
                                    g2[:, off : off + 2 * nh].rearrange(
                                        "p (a b) -> p a b", a=nh, b=2
                                    )[:, :, 0],
                                    g2[:, off : off + 2 * nh].rearrange(
                                        "p (a b) -> p a b", a=nh, b=2
                                    )[:, :, 1],
                                )
                                nc.vector.tensor_reduce(
                                    accp2[:, col + nh : col + n_rows],
                                    g2[:, off + 2 * nh : off + 2 * n_rows].rearrange(
                                        "p (a b) -> p a b", a=n_rows - nh, b=2
                                    ),
                                    axis=mybir.AxisListType.X,
                                    op=ADD,
                                )
                            else:
                                nc.vector.tensor_reduce(
                                    accp2[:, col : col + n_rows],
                                    g2[:, off : off + n_rows * k].rearrange(
                                        "p (a b) -> p a b", a=n_rows, b=k
                                    ),
                                    axis=mybir.AxisListType.X,
                                    op=ADD,
                                )
                with tc.tile_pool(name="gp2", bufs=1) as gp2pool:
                    g2p = gp2pool.tile([P, NP_], f32)
                    g2pb = gp2pool.tile([P, NP_], bf16)
                    nc.gpsimd.ap_gather(
                        g2p[:],
                        accp2[:],
                        perm2[:],
                        channels=P,
                        num_elems=P2,
                        d=1,
                        num_idxs=NP_,
                    )
                    HB = NP_ // 2
                    for s0 in (0, HB):
                        nc.scalar.activation(
                            g2pb[:, s0 : s0 + HB], g2p[:, s0 : s0 + HB], Copy
                        )
                        for m0 in range(s0, s0 + HB, MMCH):
                            mw = min(MMCH, s0 + HB - m0)
                            ps = zps2.tile([1, MMCH], f32, tag="zp2")
                            nc.tensor.matmul(
                                ps[:, :mw],
                                svec[:],
                                g2pb[:, m0 : m0 + mw],
                                start=True,
                                stop=True,
                            )
                            nc.scalar.activation(zfin[:, m0 : m0 + mw], ps[:, :mw], Copy)
                            # += self-loop term, then * dinv_dst
                            nc.vector.tensor_add(
                                zfin[:, m0 : m0 + mw],
                                zfin[:, m0 : m0 + mw],
                                zrow[:, m0 : m0 + mw],
                            )
                            nc.vector.tensor_mul(
                                zfin[:, m0 : m0 + mw],
                                zfin[:, m0 : m0 + mw],
                                dinvrow2[:, m0 : m0 + mw],
                            )
                        nc.scalar.activation(
                            zfin[:, s0 : s0 + HB],
                            zfin[:, s0 : s0 + HB],
                            Sigmoid,
                            bias=float(b2val),
                        )
                        nc.sync.dma_start(
                            out=out_d[:, s0 : s0 + HB], in_=zfin[:, s0 : s0 + HB]
                        )
    nc.finalize()
    return nc


def _sim_ns(nc):
    from concourse import bass_interp

    sim = bass_interp.CoreSim(nc, no_exec=True, publish_trace=False)
    sim.simulate()
    return int(sim.time)


def kernel(x, edge_index, W1, b1, W2, b2):
    global LAST_SIM_NS
    x = np.asarray(x, dtype=np.float32)
    edge_index = np.asarray(edge_index)
    k1_inputs, meta, (src, dst, dinv) = host_prep(x, edge_index, W1, b1, W2, b2)
    k2_inputs, meta2 = host_prep_k2(src, dst)
    b2val = float(np.asarray(b2, dtype=np.float32).reshape(-1)[0])
    nc = build_fused(meta, meta2, b2val)
    if MEASURE:
        LAST_SIM_NS = _sim_ns(nc)
    in_maps = [dict(k1_inputs[c], **k2_inputs[c]) for c in range(NCORES)]
    res = run_bass_kernel_spmd(nc, in_maps, list(range(NCORES)))
    out = np.zeros((N, 1), dtype=np.float32)
    for c in range(NCORES):
        out[c * NSH : (c + 1) * NSH, 0] = res.results[c]["out"][0, :NSH]
    return out


# revision 52
# speedup vs baseline: 2.7685x; 1.0190x over previous
"""2-layer GCN (PyG GCNConv x2 + sigmoid) on 8 TRN2 NeuronCores, single fused NEFF.

Design notes (cost-model driven):
- ap_gather costs max(table_cols, num_idxs)*0.833ns -> tables and gather
  chunks must be size-matched. 7 src-quarters (table=7144 cols) with 2
  ~8K-slot chunks each keeps L1 gathers slot-optimal (~0.84ns/edge).
- Exact-degree shared (max-over-core) ladders; k=1 rows are Act-engine
  copies instead of DVE reduces; accumulator acc and both dinv_dst
  multiplies run in bf16 on DVE (2x mode); tables built by PE in bf16.
- Layer 2: the 8 GPSIMD 16-partition groups each own one SRC CORE RANGE so
  per-group z tables are 6256 wide (table-cost-minimal); self-loops are
  excluded from the edge stream (their term is zrow itself, added at the
  end); cross-group partial sums contract on the PE via a ones vector.
- z' AllGather on-device (DRAM bounce); finalize sliced to shrink the
  serial tail into the collective.
"""

import sys

sys.path.insert(0, "/opt/trn_rl_repo")
import numpy as np
import ml_dtypes
from contextlib import ExitStack

from concourse import bacc, mybir
from concourse.tile import TileContext
from concourse.bass_utils import run_bass_kernel_spmd

MEASURE = False
LAST_SIM_NS = None

N = 50000
E = 800000
F = 128
P = 128
NCORES = 8
NSH = N // NCORES  # 6250
NQ = 7
QN = 7143  # nodes per quarter (last has 7142)
T = QN + 1  # 7144: [zero col, up to 7143 nodes]
NP_ = 6256  # padded per-core node count
MMCH = 512
XB = 2048


def _wrap16(idx_flat):
    n = idx_flat.shape[0]
    assert n % 16 == 0
    return np.ascontiguousarray(idx_flat.reshape(n // 16, 16).T)


def _pad16(n):
    return ((n + 15) // 16) * 16


def _concat_aranges(lens):
    if len(lens) == 0:
        return np.zeros(0, dtype=np.int64)
    total = int(lens.sum())
    out = np.ones(total, dtype=np.int64)
    ends = np.cumsum(lens)
    out[0] = 0
    out[ends[:-1]] = -(lens[:-1] - 1)
    return np.cumsum(out)


def _ladder_layout(kap_by_unit, n_chunks_cap, kdesc=False):
    """kap_by_unit: [n_units, n_nodes]. Shared exact-k ladder with row-aligned
    chunks, big k first (heavy reduces overlap the next chunk's gather).
    Returns (descr[(chunk, off, n_rows, k, col)], cols, kbase, chunk_sizes)."""
    kmax = int(kap_by_unit.max())
    budgets = {}
    for k in range(1, kmax + 1):
        nk = int((kap_by_unit == k).sum(axis=1).max())
        if nk > 0:
            budgets[k] = nk
    raw = sum(k * nk for k, nk in budgets.items())
    cap = raw + 64 if n_chunks_cap is None else (raw + n_chunks_cap - 1) // n_chunks_cap + 48
    descr, kbase = [], {}
    col = 1
    ch, off = 0, 0
    for k in sorted(budgets, reverse=kdesc):
        nk = budgets[k]
        kbase[k] = col
        left = nk
        while left > 0:
            fit = min(left, (cap - off) // k)
            if fit == 0:
                ch += 1
                off = 0
                fit = min(left, cap // k)
            descr.append((ch, off, fit, k, col))
            off += fit * k
            col += fit
            left -= fit
    chunk_sizes = {}
    for c, o, nr, k, _ in descr:
        chunk_sizes[c] = max(chunk_sizes.get(c, 0), o + nr * k)
    sizes = [_pad16(chunk_sizes[c]) for c in sorted(chunk_sizes)]
    return descr, col, kbase, sizes


def _pack_slots(kap, srcl_by_dst, dstl_by_dst, descr, kbase, cols, chunk_offs):
    """kap: [n_nodes] this unit's degrees; srcl/dstl: this unit's edges sorted
    by dst. Returns (slot_positions, slot_values, perm[node->accp col])."""
    nodes = np.nonzero(kap)[0]
    kn = kap[nodes]
    nd = np.lexsort((nodes, kn))
    nodes_s, kn_s = nodes[nd], kn[nd]
    rank = np.zeros(len(nodes_s), dtype=np.int64)
    colof = np.zeros(len(nodes_s), dtype=np.int64)
    for k in np.unique(kn_s):
        mk = kn_s == k
        rank[mk] = np.arange(mk.sum())
        colof[mk] = kbase[int(k)]
    node_col = colof + rank
    col2slot = np.full(cols, -1, dtype=np.int64)
    for ch, off, n_rows, k, col in descr:
        cc = np.arange(n_rows)
        col2slot[col + cc] = chunk_offs[ch] + off + cc * k
    starts = col2slot[node_col]
    eslots = np.repeat(starts, kn_s) + _concat_aranges(kn_s)
    # edge values in (k, node) order: stable sort of dst-sorted edges by k
    eo = np.argsort(kap[dstl_by_dst], kind="stable")
    ev = srcl_by_dst[eo]
    pm = np.zeros(len(kap), dtype=np.int16)
    pm[nodes_s] = node_col.astype(np.int16)
    return eslots, ev, pm


def host_prep(x, edge_index, W1, b1, W2, b2):
    src = np.concatenate([edge_index[0], np.arange(N, dtype=np.int64)]).astype(np.int32)
    dst = np.concatenate([edge_index[1], np.arange(N, dtype=np.int64)]).astype(np.int32)
    deg = np.bincount(dst, minlength=N).astype(np.float32)
    dinv = 1.0 / np.sqrt(np.maximum(deg, 1e-12))
    dinv[deg <= 0] = 0.0

    # Node -> table position. Stratified round-robin: nodes with identical
    # per-core in-degree vectors spread evenly over quarters, which tightens
    # the shared (max-over-core) ladder budgets vs a random permutation.
    degc = np.zeros((N, NCORES), dtype=np.int32)
    dst_t = np.concatenate([edge_index[1], np.arange(N, dtype=np.int64)])
    src_t = np.concatenate([edge_index[0], np.arange(N, dtype=np.int64)])
    np.add.at(degc, (src_t, dst_t // NSH), 1)
    okey = np.lexsort(tuple(degc[:, c] for c in range(NCORES)))
    rank = np.empty(N, dtype=np.int64)
    rank[okey] = np.arange(N)
    psrc = (rank % NQ) * QN + rank // NQ  # node -> table position
    assert psrc.max() < NQ * QN
    pinv = np.argsort(psrc)

    xtp = (x * dinv[:, None]).T.astype(np.float32)[:, pinv]  # [128, N] pos order
    xt = np.zeros((P, NQ * T), dtype=ml_dtypes.bfloat16)
    for q in range(NQ):
        qn = min(QN, N - q * QN)
        xt[:, q * T + 1 : q * T + 1 + qn] = xtp[:, q * QN : q * QN + qn].astype(
            ml_dtypes.bfloat16
        )

    core = dst // NSH
    dstl = (dst % NSH).astype(np.int64)
    pos = psrc[src]
    quarter = pos // QN
    srcl = (pos % QN).astype(np.int64) + 1

    flat = (core.astype(np.int64) * NQ + quarter) * NSH + dstl
    kap = np.bincount(flat, minlength=NCORES * NQ * NSH).reshape(NCORES, NQ, NSH)

    layouts = []
    for q in range(NQ):
        descr, cols, kbase, sizes = _ladder_layout(kap[:, q, :], 2)
        offs = np.concatenate([[0], np.cumsum(sizes)]).astype(np.int64)
        layouts.append((descr, cols, kbase, sizes, offs))
    SQ = [int(l[4][-1]) for l in layouts]
    PQ = _pad16(max(l[1] for l in layouts))
    G0 = max(max(l[3]) for l in layouts)

    order = np.lexsort((dstl, quarter, core))
    so, do_, qo, co = srcl[order], dstl[order], quarter[order], core[order]

    # combined per-quarter index stream: [SQ[q] slot idxs | NP_ perm idxs]
    qoff = np.concatenate([[0], np.cumsum([s + NP_ for s in SQ])]).astype(np.int64)
    qbase = np.concatenate([[0], np.cumsum(SQ)]).astype(np.int64)
    eidx = np.zeros((NCORES, int(qoff[-1])), dtype=np.int16)
    for c in range(NCORES):
        mc = co == c
        for q in range(NQ):
            m = mc & (qo == q)
            descr, cols, kbase, sizes, offs = layouts[q]
            eslots, ev, pm = _pack_slots(
                kap[c, q], so[m], do_[m], descr, kbase, cols, offs
            )
            eidx[c, qoff[q] + eslots] = ev.astype(np.int16)
            eidx[c, qoff[q] + SQ[q] : qoff[q] + SQ[q] + NSH] = pm

    eidx_w = np.zeros((NCORES, P, int(qoff[-1]) // 16), dtype=np.int16)
    for c in range(NCORES):
        eidx_w[c] = np.tile(_wrap16(eidx[c]), (8, 1))

    dinvb = np.zeros((NCORES, P, NP_), dtype=ml_dtypes.bfloat16)
    dinvrow = np.zeros((NCORES, 1, NP_), dtype=np.float32)
    for c in range(NCORES):
        dv = dinv[c * NSH : (c + 1) * NSH]
        dinvb[c, :, :NSH] = np.tile(dv.astype(ml_dtypes.bfloat16)[None, :], (P, 1))
        dinvrow[c, 0, :NSH] = dv

    meta = dict(layouts=layouts, SQ=SQ, PQ=PQ, G0=G0, qbase=qbase, qoff=qoff)
    k1_inputs = []
    for c in range(NCORES):
        k1_inputs.append(
            {
                "xt": xt,
                "w1": np.asarray(W1, dtype=ml_dtypes.bfloat16),
                "b1": np.asarray(b1, dtype=np.float32).reshape(P, 1),
                "w2": np.asarray(W2, dtype=ml_dtypes.bfloat16).reshape(P, 1),
                "eidx": np.ascontiguousarray(eidx_w[c]),
                "dinvb": np.ascontiguousarray(dinvb[c]),
                "dinvrow": np.ascontiguousarray(dinvrow[c]),
            }
        )
    return k1_inputs, meta, (src, dst, dinv)


def host_prep_k2(src, dst):
    """Layer 2: 8 GPSIMD groups = 8 src core ranges; self-loops excluded."""
    m = src != dst
    s2, d2 = src[m].astype(np.int64), dst[m].astype(np.int64)
    c2 = d2 // NSH
    g2 = s2 // NSH
    dstl = d2 % NSH
    srcl = s2 % NSH + 1

    flat = (c2 * NCORES + g2) * NSH + dstl
    kap2 = np.bincount(flat, minlength=NCORES * NCORES * NSH).reshape(
        NCORES * NCORES, NSH
    )
    descr2, cols2, kbase2, sizes2 = _ladder_layout(kap2, 2, kdesc=True)
    offs2 = np.concatenate([[0], np.cumsum(sizes2)]).astype(np.int64)
    slots2 = int(offs2[-1])
    P2 = _pad16(cols2)

    order = np.lexsort((dstl, g2, c2))
    so, do_, go, co = srcl[order], dstl[order], g2[order], c2[order]
    eidx2 = np.zeros((NCORES, NCORES, slots2), dtype=np.int16)
    perm2 = np.zeros((NCORES, NCORES, NP_), dtype=np.int16)
    for c in range(NCORES):
        mc = co == c
        for g in range(NCORES):
            mm = mc & (go == g)
            eslots, ev, pm = _pack_slots(
                kap2[c * NCORES + g], so[mm], do_[mm], descr2, kbase2, cols2, offs2
            )
            eidx2[c, g, eslots] = ev.astype(np.int16)
            perm2[c, g, :NSH] = pm

    eidx2_w = np.zeros((NCORES, P, slots2 // 16), dtype=np.int16)
    perm2_w = np.zeros((NCORES, P, NP_ // 16), dtype=np.int16)
    for c in range(NCORES):
        for g in range(NCORES):
            eidx2_w[c, g * 16 : (g + 1) * 16] = _wrap16(eidx2[c, g])
            perm2_w[c, g * 16 : (g + 1) * 16] = _wrap16(perm2[c, g])

    svec = np.zeros((P, 1), dtype=ml_dtypes.bfloat16)
    svec[0:P:16, 0] = 1.0  # sum the 8 group-partial rows

    meta2 = dict(descr2=descr2, P2=P2, slots2=slots2, sizes2=sizes2, offs2=offs2)
    k2_inputs = []
    for c in range(NCORES):
        k2_inputs.append(
            {
                "eidx2": np.ascontiguousarray(eidx2_w[c]),
                "perm2": np.ascontiguousarray(perm2_w[c]),
                "svec": svec,
            }
        )
    return k2_inputs, meta2


def build_fused(meta, meta2, b2val):
    layouts, SQ, PQ, G0, qbase, qoff = (
        meta["layouts"],
        meta["SQ"],
        meta["PQ"],
        meta["G0"],
        meta["qbase"],
        meta["qoff"],
    )
    ITW = (max(SQ) + NP_) // 16  # combined per-quarter idx tile width
    descr2, P2, slots2 = meta2["descr2"], meta2["P2"], meta2["slots2"]
    sizes2, offs2 = meta2["sizes2"], meta2["offs2"]
    G2 = max(sizes2)

    nc = bacc.Bacc(None, target_bir_lowering=False)
    f32, f32r, bf16, i16 = (
        mybir.dt.float32,
        mybir.dt.float32r,
        mybir.dt.bfloat16,
        mybir.dt.int16,
    )

    xt_d = nc.dram_tensor("xt", [P, NQ * T], bf16, kind="ExternalInput")
    w1_d = nc.dram_tensor("w1", [P, P], bf16, kind="ExternalInput")
    b1_d = nc.dram_tensor("b1", [P, 1], f32, kind="ExternalInput")
    w2_d = nc.dram_tensor("w2", [P, 1], bf16, kind="ExternalInput")
    eidx_d = nc.dram_tensor("eidx", [P, int(qoff[-1]) // 16], i16, kind="ExternalInput")
    dinvb_d = nc.dram_tensor("dinvb", [P, NP_], bf16, kind="ExternalInput")
    dinvrow_d = nc.dram_tensor("dinvrow", [1, NP_], f32, kind="ExternalInput")
    eidx2_d = nc.dram_tensor("eidx2", [P, slots2 // 16], i16, kind="ExternalInput")
    perm2_d = nc.dram_tensor("perm2", [P, NP_ // 16], i16, kind="ExternalInput")
    svec_d = nc.dram_tensor("svec", [P, 1], bf16, kind="ExternalInput")
    out_d = nc.dram_tensor("out", [1, NP_], f32, kind="ExternalOutput")

    zin = nc.dram_tensor("zin_cc", [1, NSH], f32, kind="Internal")
    zall = nc.dram_tensor(
        "zall_cc", [NCORES, NSH], f32, kind="Internal", addr_space="Shared"
    )

    Copy = mybir.ActivationFunctionType.Copy
    Sigmoid = mybir.ActivationFunctionType.Sigmoid
    ADD = mybir.AluOpType.add

    with ExitStack() as ctx:
        tc = ctx.enter_context(TileContext(nc))
        cpool = ctx.enter_context(tc.tile_pool(name="cpool", bufs=1))
        w1 = cpool.tile([P, P], bf16)
        b1 = cpool.tile([P, 1], f32)
        w2 = cpool.tile([P, 1], bf16)
        acc = cpool.tile([P, NP_], bf16)
        warm = cpool.tile([1, 16], f32)
        nc.sync.dma_start(out=w1[:], in_=w1_d[:])
        nc.sync.dma_start(out=b1[:], in_=b1_d[:])
        nc.sync.dma_start(out=w2[:], in_=w2_d[:])
        # preload the sigmoid activation table off the critical path
        nc.vector.memset(warm[:], 0.0)
        nc.scalar.activation(warm[:], warm[:], Sigmoid, bias=0.0)

        with (
            tc.tile_pool(name="tabs", bufs=2) as tabs,
            tc.tile_pool(name="xpool", bufs=2) as xpool,
            tc.tile_pool(name="gpool", bufs=2) as gpool,
            tc.tile_pool(name="tpool", bufs=1) as tpool,
            tc.tile_pool(name="accpool", bufs=2) as accpool,
            tc.tile_pool(name="epool", bufs=2) as epool,
            tc.tile_pool(name="pspool", bufs=2, space="PSUM") as pspool,
        ):
            accps = {}

            def assemble(q):
                # perm-gather quarter q's partials to node order and fold
                # into acc (emitted one quarter late to keep Pool saturated)
                accp_q, it_q = accps.pop(q)
                p0 = SQ[q] // 16
                tt = gpool.tile([P, G0], f32, tag="g")
                nc.gpsimd.ap_gather(
                    tt[:, :NP_],
                    accp_q[:],
                    it_q[:, p0 : p0 + NP_ // 16],
                    channels=P,
                    num_elems=PQ,
                    d=1,
                    num_idxs=NP_,
                )
                if q == 0:
                    nc.scalar.activation(acc[:], tt[:, :NP_], Copy)
                elif q < NQ - 1:
                    tb = tpool.tile([P, NP_], bf16, tag="tb")
                    nc.scalar.activation(tb[:], tt[:, :NP_], Copy)
                    nc.vector.tensor_add(acc[:], acc[:], tb[:])
                else:
                    # last quarter: slice so finalize can start per-slice
                    tb = tpool.tile([P, NP_], bf16, tag="tb")
                    for s0 in range(0, NP_, 1564):
                        sl = slice(s0, s0 + 1564)
                        nc.scalar.activation(tb[:, sl], tt[:, sl], Copy)
                        nc.vector.tensor_add(acc[:, sl], acc[:, sl], tb[:, sl])

            for q in range(NQ):
                descr, cols, kbase, sizes, offs = layouts[q]
                tab = tabs.tile([P, T], f32, tag="tab")
                chunks = []
                x0 = sum(chunks)
                while x0 < T:
                    chunks.append(min(XB, T - x0))
                    x0 += chunks[-1]
                x0 = 0
                for xw in chunks:
                    xc = xpool.tile([P, XB], bf16, tag="x")
                    nc.sync.dma_start(
                        out=xc[:, :xw], in_=xt_d[:, q * T + x0 : q * T + x0 + xw]
                    )
                    ps = pspool.tile([P, XB], f32, tag="ps")
                    for m0 in range(0, xw, MMCH):
                        mw = min(MMCH, xw - m0)
                        nc.tensor.matmul(
                            ps[:, m0 : m0 + mw],
                            w1[:],
                            xc[:, m0 : m0 + mw],
                            start=True,
                            stop=True,
                        )
                    nc.scalar.activation(tab[:, x0 : x0 + xw], ps[:, :xw], Copy)
                    x0 += xw
                accp = accpool.tile([P, PQ], f32, tag="accp")
                it = epool.tile([P, ITW], i16, tag="it")
                qw = (SQ[q] + NP_) // 16
                i0 = int(qoff[q]) // 16
                nc.sync.dma_start(out=it[:, :qw], in_=eidx_d[:, i0 : i0 + qw])
                accps[q] = (accp, it)
                nc.vector.memset(accp[:, 0:1], 0.0)
                by_chunk = {}
                for d_ in descr:
                    by_chunk.setdefault(d_[0], []).append(d_)
                for ci, ch in enumerate(sorted(by_chunk)):
                    sz = sizes[ch]
                    c0 = int(offs[ch]) // 16
                    g = gpool.tile([P, G0], f32, tag="g")
                    nc.gpsimd.ap_gather(
                        g[:, :sz],
                        tab[:],
                        it[:, c0 : c0 + sz // 16],
                        channels=P,
                        num_elems=T,
                        d=1,
                        num_idxs=sz,
                    )
                    if ci == 0 and q > 0:
                        assemble(q - 1)
                    for _, off, n_rows, k, col in by_chunk[ch]:
                        if k == 1 and ci == 0:
                            # Act handles chunk-0 k=1 rows; later chunks go to
                            # DVE so Act isn't blocked ahead of next tab build
                            nc.scalar.activation(
                                accp[:, col : col + n_rows],
                                g[:, off : off + n_rows],
                                Copy,
                            )
                        elif k == 1:
                            nc.vector.tensor_copy(
                                accp[:, col : col + n_rows], g[:, off : off + n_rows]
                            )
                        elif k == 2 and n_rows >= 48:
                            # rebalance: ~1/3 of pair-adds on GPSIMD
                            nh = _pad16(n_rows // 3)
                            pr = g[:, off : off + 2 * nh].rearrange(
                                "p (a b) -> p a b", a=nh, b=2
                            )
                            nc.gpsimd.tensor_add(
                                accp[:, col : col + nh], pr[:, :, 0], pr[:, :, 1]
                            )
                            nc.vector.tensor_reduce(
                                accp[:, col + nh : col + n_rows],
                                g[:, off + 2 * nh : off + 2 * n_rows].rearrange(
                                    "p (a b) -> p a b", a=n_rows - nh, b=2
                                ),
                                axis=mybir.AxisListType.X,
                                op=ADD,
                            )
                        else:
                            nc.vector.tensor_reduce(
                                accp[:, col : col + n_rows],
                                g[:, off : off + n_rows * k].rearrange(
                                    "p (a b) -> p a b", a=n_rows, b=k
                                ),
                                axis=mybir.AxisListType.X,
                                op=ADD,
                            )
                if q == NQ - 1:
                    assemble(q)

        # finalize (4 column slices): h' = dinv*sigmoid(dinv*acc+b1); z=W2^T h'
        with (
            tc.tile_pool(name="fin", bufs=1) as fin,
            tc.tile_pool(name="zps", bufs=2, space="PSUM") as zps,
        ):
            zrow = fin.tile([1, NP_], f32)
            dinvrow2 = fin.tile([1, NP_], f32)
            nc.sync.dma_start(out=dinvrow2[:], in_=dinvrow_d[:])
            with tc.tile_pool(name="finb", bufs=1) as finb:
                dinvb = finb.tile([P, NP_], bf16)
                nc.sync.dma_start(out=dinvb[:], in_=dinvb_d[:])
                bounds = [0, 1536, 3072, 4608, NP_]
                for si in range(4):
                    sl = slice(bounds[si], bounds[si + 1])
                    nc.vector.tensor_mul(acc[:, sl], acc[:, sl], dinvb[:, sl])
                    nc.scalar.activation(
                        acc[:, sl], acc[:, sl], Sigmoid, bias=b1[:, 0:1]
                    )
                    nc.vector.tensor_mul(acc[:, sl], acc[:, sl], dinvb[:, sl])
                    for m0 in range(bounds[si], bounds[si + 1], MMCH):
                        mw = min(MMCH, bounds[si + 1] - m0)
                        ps = zps.tile([1, MMCH], f32, tag="zp")
                        nc.tensor.matmul(
                            ps[:, :mw],
                            w2[:],
                            acc[:, m0 : m0 + mw],
                            start=True,
                            stop=True,
                        )
                        nc.scalar.activation(zrow[:, m0 : m0 + mw], ps[:, :mw], Copy)
                nc.sync.dma_start(out=zin[:, : NSH // 2], in_=zrow[:, : NSH // 2])
                nc.sync.dma_start(out=zin[:, NSH // 2 :], in_=zrow[:, NSH // 2 : NSH])

            nc.gpsimd.collective_compute(
                "AllGather",
                mybir.AluOpType.bypass,
                replica_groups=[list(range(NCORES))],
                ins=[zin[:].opt()],
                outs=[zall[:].opt()],
            )

            # ---- layer 2 ----
            with (
                tc.tile_pool(name="k2pool", bufs=1) as pool2,
                tc.tile_pool(name="zps2", bufs=2, space="PSUM") as zps2,
            ):
                eidx2 = pool2.tile([P, slots2 // 16], i16)
                perm2 = pool2.tile([P, NP_ // 16], i16)
                accp2 = pool2.tile([P, P2], f32)
                svec = pool2.tile([P, 1], bf16)
                zfin = pool2.tile([1, NP_], f32)
                nc.sync.dma_start(out=eidx2[:], in_=eidx2_d[:])
                nc.sync.dma_start(out=perm2[:], in_=perm2_d[:])
                nc.sync.dma_start(out=svec[:], in_=svec_d[:])
                nc.vector.memset(accp2[:, 0:1], 0.0)
                with tc.tile_pool(name="ztpool", bufs=1) as ztpool, tc.tile_pool(
                    name="g2pool", bufs=2
                ) as g2pool:
                    zt = ztpool.tile([P, NP_], f32)
                    nc.vector.memset(zt[:, 0:1], 0.0)
                    nc.sync.dma_start(out=zt[0:P:16, 1 : 1 + NSH], in_=zall[:, :])
                    by_chunk2 = {}
                    for d_ in descr2:
                        by_chunk2.setdefault(d_[0], []).append(d_)
                    for ch in sorted(by_chunk2):
                        sz = sizes2[ch]
                        g2 = g2pool.tile([P, G2], f32, tag="g2")
                        i0 = int(offs2[ch]) // 16
                        nc.gpsimd.ap_gather(
                            g2[:, :sz],
                            zt[:],
                            eidx2[:, i0 : i0 + sz // 16],
                            channels=P,
                            num_elems=NP_,
                            d=1,
                            num_idxs=sz,
                        )
                        for _, off, n_rows, k, col in by_chunk2[ch]:
                            if k == 1:
                                nc.scalar.activation(
                                    accp2[:, col : col + n_rows],
                                    g2[:, off : off + n_rows],
                                    Copy,
                                )
                            elif k == 2:
                                # pair-adds on GPSIMD (Pool idles post-gather)
                                pairs = g2[:, off : off + 2 * n_rows].rearrange(
                                    "p (a b) -> p a b", a=n_rows, b=2
                                )
                                nc.gpsimd.tensor_add(
                                    accp2[:, col : col + n_rows],
                                    pairs[:, :, 0],
                                    pairs[:, :, 1],
                                )
                            elif k == 3:
                                trip = g2[:, off : off + 3 * n_rows].rearrange(
                                    "p (a b) -> p a b", a=n_rows, b=3
                                )
                                nc.gpsimd.tensor_add(
                                    accp2[:, col : col + n_rows],
                                    trip[:, :, 0],
                                    trip[:, :, 1],
                                )
                                nc.vector.tensor_add(
                                    accp2[:, col : col + n_rows],
                                    accp2[:, col : col + n_rows],
                                    trip[:, :, 2],
                                )
                            else:
                                nc.vector.tensor_reduce(
                                    accp2[:, col : col + n_rows],
                                    g2[:, off : off + n_rows * k].rearrange(
                                        "p (a b) -> p a b", a=n_rows, b=k
                                    ),
                                    axis=mybir.AxisListType.X,
                                    op=ADD,
                                )
                with tc.tile_pool(name="gp2", bufs=1) as gp2pool:
                    g2p = gp2pool.tile([P, NP_], f32)
                    g2pb = gp2pool.tile([P, NP_], bf16)
                    nc.gpsimd.ap_gather(
                        g2p[:],
                        accp2[:],
                        perm2[:],
                        channels=P,
                        num_elems=P2,
                        d=1,
                        num_idxs=NP_,
                    )
                    HB = NP_ // 2
                    for s0 in (0, HB):
                        nc.scalar.activation(
                            g2pb[:, s0 : s0 + HB], g2p[:, s0 : s0 + HB], Copy
                        )
                        for m0 in range(s0, s0 + HB, MMCH):
                            mw = min(MMCH, s0 + HB - m0)
                            ps = zps2.tile([1, MMCH], f32, tag="zp2")
                            nc.tensor.matmul(
                                ps[:, :mw],
                                svec[:],
                                g2pb[:, m0 : m0 + mw],
                                start=True,
                                stop=True,
                            )
                            nc.scalar.activation(zfin[:, m0 : m0 + mw], ps[:, :mw], Copy)
                            # += self-loop term, then * dinv_dst
                            nc.vector.tensor_add(
                                zfin[:, m0 : m0 + mw],
                                zfin[:, m0 : m0 + mw],
                                zrow[:, m0 : m0 + mw],
                            )
                            nc.vector.tensor_mul(
                                zfin[:, m0 : m0 + mw],
                                zfin[:, m0 : m0 + mw],
                                dinvrow2[:, m0 : m0 + mw],
                            )
                        nc.scalar.activation(
                            zfin[:, s0 : s0 + HB],
                            zfin[:, s0 : s0 + HB],
                            Sigmoid,
                            bias=float(b2val),
                        )
                        nc.sync.dma_start(
                            out=out_d[:, s0 : s0 + HB], in_=zfin[:, s0 : s0 + HB]
                        )
    nc.finalize()
    return nc


def _sim_ns(nc):
    from concourse import bass_interp

    sim = bass_interp.CoreSim(nc, no_exec=True, publish_trace=False)
    sim.simulate()
    return int(sim.time)


def kernel(x, edge_index, W1, b1, W2, b2):
    global LAST_SIM_NS
    x = np.asarray(x, dtype=np.float32)
    edge_index = np.asarray(edge_index)
    k1_inputs, meta, (src, dst, dinv) = host_prep(x, edge_index, W1, b1, W2, b2)
    k2_inputs, meta2 = host_prep_k2(src, dst)
    b2val = float(np.asarray(b2, dtype=np.float32).reshape(-1)[0])
    nc = build_fused(meta, meta2, b2val)
    if MEASURE:
        LAST_SIM_NS = _sim_ns(nc)
    in_maps = [dict(k1_inputs[c], **k2_inputs[c]) for c in range(NCORES)]
    res = run_bass_kernel_spmd(nc, in_maps, list(range(NCORES)))
    out = np.zeros((N, 1), dtype=np.float32)
    for c in range(NCORES):
        out[c * NSH : (c + 1) * NSH, 0] = res.results[c]["out"][0, :NSH]
    return out


# revision 55
# speedup vs baseline: 2.7920x; 1.0085x over previous
"""2-layer GCN (PyG GCNConv x2 + sigmoid) on 8 TRN2 NeuronCores, single fused NEFF.

Sharding: dst-node ranges across the 8 cores (6250 nodes each); GCN weights
replicated; the layer-1->layer-2 halo exchange is an on-device AllGather of
each core's 6250 z'=W2^T h' values.

Design notes (cost-model driven):
- ap_gather costs max(table_cols, num_idxs)*0.833ns -> tables and gather
  chunks must be size-matched. 7 src-quarters (table=7144 cols) with 2
  ~8K-slot chunks each keeps L1 gathers slot-optimal (~0.84ns/edge).
- Edge segment sums via exact-degree ladders with layouts shared
  (max-over-core) so one SPMD program fits all cores; k=1 rows are Act
  copies, some k<=2/3 pair-adds go to GPSIMD to balance DVE; per-quarter
  partials are perm-gathered back to node order and accumulated in bf16
  (DVE 2x mode); assembly runs one quarter behind the gathers to keep
  GPSIMD saturated.
- Tables built by PE in bf16 (1 cyc/col); both dinv_dst multiplies fold
  into bf16 tensor ops; h'@W2 contracts on PE in bf16.
- Layer 2: the 8 GPSIMD 16-partition groups each own one SRC CORE RANGE so
  per-group z tables are 6256 wide (table-cost-minimal); self-loops are
  excluded from the edge stream (their term is zrow, added per chunk);
  cross-group partial sums contract on the PE via a stride-16 ones vector
  over the perm-gathered (bf16-converted) partials.
- Finalize and the last quarter's assembly are column-sliced so the
  z-row production chain into the collective stays pipelined.
"""

import sys

sys.path.insert(0, "/opt/trn_rl_repo")
import numpy as np
import ml_dtypes
from contextlib import ExitStack

from concourse import bacc, mybir
from concourse.tile import TileContext
from concourse.bass_utils import run_bass_kernel_spmd

MEASURE = False
LAST_SIM_NS = None

N = 50000
E = 800000
F = 128
P = 128
NCORES = 8
NSH = N // NCORES  # 6250
NQ = 7
QN = 7143  # nodes per quarter (last has 7142)
T = QN + 1  # 7144: [zero col, up to 7143 nodes]
NP_ = 6256  # padded per-core node count
MMCH = 512
XB = 2048


def _wrap16(idx_flat):
    n = idx_flat.shape[0]
    assert n % 16 == 0
    return np.ascontiguousarray(idx_flat.reshape(n // 16, 16).T)


def _pad16(n):
    return ((n + 15) // 16) * 16


def _concat_aranges(lens):
    if len(lens) == 0:
        return np.zeros(0, dtype=np.int64)
    total = int(lens.sum())
    out = np.ones(total, dtype=np.int64)
    ends = np.cumsum(lens)
    out[0] = 0
    out[ends[:-1]] = -(lens[:-1] - 1)
    return np.cumsum(out)


def _ladder_layout(kap_by_unit, n_chunks_cap, kdesc=False):
    """kap_by_unit: [n_units, n_nodes]. Shared exact-k ladder with row-aligned
    chunks, big k first (heavy reduces overlap the next chunk's gather).
    Returns (descr[(chunk, off, n_rows, k, col)], cols, kbase, chunk_sizes)."""
    kmax = int(kap_by_unit.max())
    budgets = {}
    for k in range(1, kmax + 1):
        nk = int((kap_by_unit == k).sum(axis=1).max())
        if nk > 0:
            budgets[k] = nk
    raw = sum(k * nk for k, nk in budgets.items())
    cap = raw + 64 if n_chunks_cap is None else (raw + n_chunks_cap - 1) // n_chunks_cap + 48
    descr, kbase = [], {}
    col = 1
    ch, off = 0, 0
    for k in sorted(budgets, reverse=kdesc):
        nk = budgets[k]
        kbase[k] = col
        left = nk
        while left > 0:
            fit = min(left, (cap - off) // k)
            if fit == 0:
                ch += 1
                off = 0
                fit = min(left, cap // k)
            descr.append((ch, off, fit, k, col))
            off += fit * k
            col += fit
            left -= fit
    chunk_sizes = {}
    for c, o, nr, k, _ in descr:
        chunk_sizes[c] = max(chunk_sizes.get(c, 0), o + nr * k)
    sizes = [_pad16(chunk_sizes[c]) for c in sorted(chunk_sizes)]
    return descr, col, kbase, sizes


def _pack_slots(kap, srcl_by_dst, dstl_by_dst, descr, kbase, cols, chunk_offs):
    """kap: [n_nodes] this unit's degrees; srcl/dstl: this unit's edges sorted
    by dst. Returns (slot_positions, slot_values, perm[node->accp col])."""
    nodes = np.nonzero(kap)[0]
    kn = kap[nodes]
    nd = np.lexsort((nodes, kn))
    nodes_s, kn_s = nodes[nd], kn[nd]
    rank = np.zeros(len(nodes_s), dtype=np.int64)
    colof = np.zeros(len(nodes_s), dtype=np.int64)
    for k in np.unique(kn_s):
        mk = kn_s == k
        rank[mk] = np.arange(mk.sum())
        colof[mk] = kbase[int(k)]
    node_col = colof + rank
    col2slot = np.full(cols, -1, dtype=np.int64)
    for ch, off, n_rows, k, col in descr:
        cc = np.arange(n_rows)
        col2slot[col + cc] = chunk_offs[ch] + off + cc * k
    starts = col2slot[node_col]
    eslots = np.repeat(starts, kn_s) + _concat_aranges(kn_s)
    # edge values in (k, node) order: stable sort of dst-sorted edges by k
    eo = np.argsort(kap[dstl_by_dst], kind="stable")
    ev = srcl_by_dst[eo]
    pm = np.zeros(len(kap), dtype=np.int16)
    pm[nodes_s] = node_col.astype(np.int16)
    return eslots, ev, pm


def host_prep(x, edge_index, W1, b1, W2, b2):
    src = np.concatenate([edge_index[0], np.arange(N, dtype=np.int64)]).astype(np.int32)
    dst = np.concatenate([edge_index[1], np.arange(N, dtype=np.int64)]).astype(np.int32)
    deg = np.bincount(dst, minlength=N).astype(np.float32)
    dinv = 1.0 / np.sqrt(np.maximum(deg, 1e-12))
    dinv[deg <= 0] = 0.0

    # Node -> table position. Stratified round-robin: nodes with identical
    # per-core in-degree vectors spread evenly over quarters, which tightens
    # the shared (max-over-core) ladder budgets vs a random permutation.
    degc = np.zeros((N, NCORES), dtype=np.int32)
    dst_t = np.concatenate([edge_index[1], np.arange(N, dtype=np.int64)])
    src_t = np.concatenate([edge_index[0], np.arange(N, dtype=np.int64)])
    np.add.at(degc, (src_t, dst_t // NSH), 1)
    okey = np.lexsort(tuple(degc[:, c] for c in range(NCORES)))
    rank = np.empty(N, dtype=np.int64)
    rank[okey] = np.arange(N)
    psrc = (rank % NQ) * QN + rank // NQ  # node -> table position
    assert psrc.max() < NQ * QN
    pinv = np.argsort(psrc)

    xtp = (x * dinv[:, None]).T.astype(np.float32)[:, pinv]  # [128, N] pos order
    xt = np.zeros((P, NQ * T), dtype=ml_dtypes.bfloat16)
    for q in range(NQ):
        qn = min(QN, N - q * QN)
        xt[:, q * T + 1 : q * T + 1 + qn] = xtp[:, q * QN : q * QN + qn].astype(
            ml_dtypes.bfloat16
        )

    core = dst // NSH
    dstl = (dst % NSH).astype(np.int64)
    pos = psrc[src]
    quarter = pos // QN
    srcl = (pos % QN).astype(np.int64) + 1

    flat = (core.astype(np.int64) * NQ + quarter) * NSH + dstl
    kap = np.bincount(flat, minlength=NCORES * NQ * NSH).reshape(NCORES, NQ, NSH)

    layouts = []
    for q in range(NQ):
        descr, cols, kbase, sizes = _ladder_layout(kap[:, q, :], 2)
        offs = np.concatenate([[0], np.cumsum(sizes)]).astype(np.int64)
        layouts.append((descr, cols, kbase, sizes, offs))
    SQ = [int(l[4][-1]) for l in layouts]
    PQ = _pad16(max(l[1] for l in layouts))
    G0 = max(max(l[3]) for l in layouts)

    order = np.lexsort((dstl, quarter, core))
    so, do_, qo, co = srcl[order], dstl[order], quarter[order], core[order]

    # combined per-quarter index stream: [SQ[q] slot idxs | NP_ perm idxs]
    qoff = np.concatenate([[0], np.cumsum([s + NP_ for s in SQ])]).astype(np.int64)
    qbase = np.concatenate([[0], np.cumsum(SQ)]).astype(np.int64)
    eidx = np.zeros((NCORES, int(qoff[-1])), dtype=np.int16)
    for c in range(NCORES):
        mc = co == c
        for q in range(NQ):
            m = mc & (qo == q)
            descr, cols, kbase, sizes, offs = layouts[q]
            eslots, ev, pm = _pack_slots(
                kap[c, q], so[m], do_[m], descr, kbase, cols, offs
            )
            eidx[c, qoff[q] + eslots] = ev.astype(np.int16)
            eidx[c, qoff[q] + SQ[q] : qoff[q] + SQ[q] + NSH] = pm

    eidx_w = np.zeros((NCORES, P, int(qoff[-1]) // 16), dtype=np.int16)
    for c in range(NCORES):
        eidx_w[c] = np.tile(_wrap16(eidx[c]), (8, 1))

    dinvb = np.zeros((NCORES, P, NP_), dtype=ml_dtypes.bfloat16)
    dinvrow = np.zeros((NCORES, 1, NP_), dtype=np.float32)
    for c in range(NCORES):
        dv = dinv[c * NSH : (c + 1) * NSH]
        dinvb[c, :, :NSH] = np.tile(dv.astype(ml_dtypes.bfloat16)[None, :], (P, 1))
        dinvrow[c, 0, :NSH] = dv

    meta = dict(layouts=layouts, SQ=SQ, PQ=PQ, G0=G0, qbase=qbase, qoff=qoff)
    k1_inputs = []
    for c in range(NCORES):
        k1_inputs.append(
            {
                "xt": xt,
                "w1": np.asarray(W1, dtype=ml_dtypes.bfloat16),
                "b1": np.asarray(b1, dtype=np.float32).reshape(P, 1),
                "w2": np.asarray(W2, dtype=ml_dtypes.bfloat16).reshape(P, 1),
                "eidx": np.ascontiguousarray(eidx_w[c]),
                "dinvb": np.ascontiguousarray(dinvb[c]),
                "dinvrow": np.ascontiguousarray(dinvrow[c]),
            }
        )
    return k1_inputs, meta, (src, dst, dinv)


def host_prep_k2(src, dst):
    """Layer 2: 8 GPSIMD groups = 8 src core ranges; self-loops excluded."""
    m = src != dst
    s2, d2 = src[m].astype(np.int64), dst[m].astype(np.int64)
    c2 = d2 // NSH
    g2 = s2 // NSH
    dstl = d2 % NSH
    srcl = s2 % NSH + 1

    flat = (c2 * NCORES + g2) * NSH + dstl
    kap2 = np.bincount(flat, minlength=NCORES * NCORES * NSH).reshape(
        NCORES * NCORES, NSH
    )
    descr2, cols2, kbase2, sizes2 = _ladder_layout(kap2, 2, kdesc=True)
    offs2 = np.concatenate([[0], np.cumsum(sizes2)]).astype(np.int64)
    slots2 = int(offs2[-1])
    P2 = _pad16(cols2)

    order = np.lexsort((dstl, g2, c2))
    so, do_, go, co = srcl[order], dstl[order], g2[order], c2[order]
    eidx2 = np.zeros((NCORES, NCORES, slots2), dtype=np.int16)
    perm2 = np.zeros((NCORES, NCORES, NP_), dtype=np.int16)
    for c in range(NCORES):
        mc = co == c
        for g in range(NCORES):
            mm = mc & (go == g)
            eslots, ev, pm = _pack_slots(
                kap2[c * NCORES + g], so[mm], do_[mm], descr2, kbase2, cols2, offs2
            )
            eidx2[c, g, eslots] = ev.astype(np.int16)
            perm2[c, g, :NSH] = pm

    eidx2_w = np.zeros((NCORES, P, slots2 // 16), dtype=np.int16)
    perm2_w = np.zeros((NCORES, P, NP_ // 16), dtype=np.int16)
    for c in range(NCORES):
        for g in range(NCORES):
            eidx2_w[c, g * 16 : (g + 1) * 16] = _wrap16(eidx2[c, g])
            perm2_w[c, g * 16 : (g + 1) * 16] = _wrap16(perm2[c, g])

    svec = np.zeros((P, 1), dtype=ml_dtypes.bfloat16)
    svec[0:P:16, 0] = 1.0  # sum the 8 group-partial rows

    meta2 = dict(descr2=descr2, P2=P2, slots2=slots2, sizes2=sizes2, offs2=offs2)
    k2_inputs = []
    for c in range(NCORES):
        k2_inputs.append(
            {
                "eidx2": np.ascontiguousarray(eidx2_w[c]),
                "perm2": np.ascontiguousarray(perm2_w[c]),
                "svec": svec,
            }
        )
    return k2_inputs, meta2


def build_fused(meta, meta2, b2val):
    layouts, SQ, PQ, G0, qbase, qoff = (
        meta["layouts"],
        meta["SQ"],
        meta["PQ"],
        meta["G0"],
        meta["qbase"],
        meta["qoff"],
    )
    ITW = (max(SQ) + NP_) // 16  # combined per-quarter idx tile width
    descr2, P2, slots2 = meta2["descr2"], meta2["P2"], meta2["slots2"]
    sizes2, offs2 = meta2["sizes2"], meta2["offs2"]
    G2 = max(sizes2)

    nc = bacc.Bacc(None, target_bir_lowering=False)
    f32, f32r, bf16, i16 = (
        mybir.dt.float32,
        mybir.dt.float32r,
        mybir.dt.bfloat16,
        mybir.dt.int16,
    )

    xt_d = nc.dram_tensor("xt", [P, NQ * T], bf16, kind="ExternalInput")
    w1_d = nc.dram_tensor("w1", [P, P], bf16, kind="ExternalInput")
    b1_d = nc.dram_tensor("b1", [P, 1], f32, kind="ExternalInput")
    w2_d = nc.dram_tensor("w2", [P, 1], bf16, kind="ExternalInput")
    eidx_d = nc.dram_tensor("eidx", [P, int(qoff[-1]) // 16], i16, kind="ExternalInput")
    dinvb_d = nc.dram_tensor("dinvb", [P, NP_], bf16, kind="ExternalInput")
    dinvrow_d = nc.dram_tensor("dinvrow", [1, NP_], f32, kind="ExternalInput")
    eidx2_d = nc.dram_tensor("eidx2", [P, slots2 // 16], i16, kind="ExternalInput")
    perm2_d = nc.dram_tensor("perm2", [P, NP_ // 16], i16, kind="ExternalInput")
    svec_d = nc.dram_tensor("svec", [P, 1], bf16, kind="ExternalInput")
    out_d = nc.dram_tensor("out", [1, NP_], f32, kind="ExternalOutput")

    zin = nc.dram_tensor("zin_cc", [1, NSH], f32, kind="Internal")
    zall = nc.dram_tensor(
        "zall_cc", [NCORES, NSH], f32, kind="Internal", addr_space="Shared"
    )

    Copy = mybir.ActivationFunctionType.Copy
    Sigmoid = mybir.ActivationFunctionType.Sigmoid
    ADD = mybir.AluOpType.add

    with ExitStack() as ctx:
        tc = ctx.enter_context(TileContext(nc))
        cpool = ctx.enter_context(tc.tile_pool(name="cpool", bufs=1))
        w1 = cpool.tile([P, P], bf16)
        b1 = cpool.tile([P, 1], f32)
        w2 = cpool.tile([P, 1], bf16)
        acc = cpool.tile([P, NP_], bf16)
        warm = cpool.tile([1, 16], f32)
        nc.sync.dma_start(out=w1[:], in_=w1_d[:])
        nc.sync.dma_start(out=b1[:], in_=b1_d[:])
        nc.sync.dma_start(out=w2[:], in_=w2_d[:])
        # preload the sigmoid activation table off the critical path
        nc.vector.memset(warm[:], 0.0)
        nc.scalar.activation(warm[:], warm[:], Sigmoid, bias=0.0)

        with (
            tc.tile_pool(name="tabs", bufs=2) as tabs,
            tc.tile_pool(name="xpool", bufs=2) as xpool,
            tc.tile_pool(name="gpool", bufs=2) as gpool,
            tc.tile_pool(name="tpool", bufs=1) as tpool,
            tc.tile_pool(name="accpool", bufs=2) as accpool,
            tc.tile_pool(name="epool", bufs=2) as epool,
            tc.tile_pool(name="pspool", bufs=2, space="PSUM") as pspool,
        ):
            accps = {}

            def assemble(q):
                # perm-gather quarter q's partials to node order and fold
                # into acc (emitted one quarter late to keep Pool saturated)
                accp_q, it_q = accps.pop(q)
                p0 = SQ[q] // 16
                tt = gpool.tile([P, G0], f32, tag="g")
                nc.gpsimd.ap_gather(
                    tt[:, :NP_],
                    accp_q[:],
                    it_q[:, p0 : p0 + NP_ // 16],
                    channels=P,
                    num_elems=PQ,
                    d=1,
                    num_idxs=NP_,
                )
                if q == 0:
                    nc.scalar.activation(acc[:], tt[:, :NP_], Copy)
                elif q < NQ - 1:
                    tb = tpool.tile([P, NP_], bf16, tag="tb")
                    nc.scalar.activation(tb[:], tt[:, :NP_], Copy)
                    nc.vector.tensor_add(acc[:], acc[:], tb[:])
                else:
                    # last quarter: slice so finalize can start per-slice
                    tb = tpool.tile([P, NP_], bf16, tag="tb")
                    for s0 in range(0, NP_, 1564):
                        sl = slice(s0, s0 + 1564)
                        nc.scalar.activation(tb[:, sl], tt[:, sl], Copy)
                        nc.vector.tensor_add(acc[:, sl], acc[:, sl], tb[:, sl])

            for q in range(NQ):
                descr, cols, kbase, sizes, offs = layouts[q]
                tab = tabs.tile([P, T], f32, tag="tab")
                chunks = []
                x0 = sum(chunks)
                while x0 < T:
                    chunks.append(min(XB, T - x0))
                    x0 += chunks[-1]
                x0 = 0
                for xw in chunks:
                    xc = xpool.tile([P, XB], bf16, tag="x")
                    nc.sync.dma_start(
                        out=xc[:, :xw], in_=xt_d[:, q * T + x0 : q * T + x0 + xw]
                    )
                    ps = pspool.tile([P, XB], f32, tag="ps")
                    for m0 in range(0, xw, MMCH):
                        mw = min(MMCH, xw - m0)
                        nc.tensor.matmul(
                            ps[:, m0 : m0 + mw],
                            w1[:],
                            xc[:, m0 : m0 + mw],
                            start=True,
                            stop=True,
                        )
                    nc.scalar.activation(tab[:, x0 : x0 + xw], ps[:, :xw], Copy)
                    x0 += xw
                accp = accpool.tile([P, PQ], f32, tag="accp")
                it = epool.tile([P, ITW], i16, tag="it")
                qw = (SQ[q] + NP_) // 16
                i0 = int(qoff[q]) // 16
                nc.sync.dma_start(out=it[:, :qw], in_=eidx_d[:, i0 : i0 + qw])
                accps[q] = (accp, it)
                nc.vector.memset(accp[:, 0:1], 0.0)
                by_chunk = {}
                for d_ in descr:
                    by_chunk.setdefault(d_[0], []).append(d_)
                for ci, ch in enumerate(sorted(by_chunk)):
                    sz = sizes[ch]
                    c0 = int(offs[ch]) // 16
                    g = gpool.tile([P, G0], f32, tag="g")
                    nc.gpsimd.ap_gather(
                        g[:, :sz],
                        tab[:],
                        it[:, c0 : c0 + sz // 16],
                        channels=P,
                        num_elems=T,
                        d=1,
                        num_idxs=sz,
                    )
                    if ci == 0 and q > 0:
                        assemble(q - 1)
                    for _, off, n_rows, k, col in by_chunk[ch]:
                        if k == 1 and ci == 0:
                            # Act handles chunk-0 k=1 rows; later chunks go to
                            # DVE so Act isn't blocked ahead of next tab build
                            nc.scalar.activation(
                                accp[:, col : col + n_rows],
                                g[:, off : off + n_rows],
                                Copy,
                            )
                        elif k == 1:
                            nc.vector.tensor_copy(
                                accp[:, col : col + n_rows], g[:, off : off + n_rows]
                            )
                        elif k == 2 and n_rows >= 48:
                            # rebalance: ~1/3 of pair-adds on GPSIMD
                            nh = _pad16(n_rows * 2 // 3)
                            pr = g[:, off : off + 2 * nh].rearrange(
                                "p (a b) -> p a b", a=nh, b=2
                            )
                            nc.gpsimd.tensor_add(
                                accp[:, col : col + nh], pr[:, :, 0], pr[:, :, 1]
                            )
                            nc.vector.tensor_reduce(
                                accp[:, col + nh : col + n_rows],
                                g[:, off + 2 * nh : off + 2 * n_rows].rearrange(
                                    "p (a b) -> p a b", a=n_rows - nh, b=2
                                ),
                                axis=mybir.AxisListType.X,
                                op=ADD,
                            )
                        else:
                            nc.vector.tensor_reduce(
                                accp[:, col : col + n_rows],
                                g[:, off : off + n_rows * k].rearrange(
                                    "p (a b) -> p a b", a=n_rows, b=k
                                ),
                                axis=mybir.AxisListType.X,
                                op=ADD,
                            )
                if q == NQ - 1:
                    assemble(q)

        # finalize (4 column slices): h' = dinv*sigmoid(dinv*acc+b1); z=W2^T h'
        with (
            tc.tile_pool(name="fin", bufs=1) as fin,
            tc.tile_pool(name="zps", bufs=2, space="PSUM") as zps,
        ):
            zrow = fin.tile([1, NP_], f32)
            dinvrow2 = fin.tile([1, NP_], f32)
            nc.sync.dma_start(out=dinvrow2[:], in_=dinvrow_d[:])
            with tc.tile_pool(name="finb", bufs=1) as finb:
                dinvb = finb.tile([P, NP_], bf16)
                nc.sync.dma_start(out=dinvb[:], in_=dinvb_d[:])
                bounds = [0, 1536, 3072, 4608, NP_]
                for si in range(4):
                    sl = slice(bounds[si], bounds[si + 1])
                    nc.vector.tensor_mul(acc[:, sl], acc[:, sl], dinvb[:, sl])
                    nc.scalar.activation(
                        acc[:, sl], acc[:, sl], Sigmoid, bias=b1[:, 0:1]
                    )
                    nc.vector.tensor_mul(acc[:, sl], acc[:, sl], dinvb[:, sl])
                    for m0 in range(bounds[si], bounds[si + 1], MMCH):
                        mw = min(MMCH, bounds[si + 1] - m0)
                        ps = zps.tile([1, MMCH], f32, tag="zp")
                        nc.tensor.matmul(
                            ps[:, :mw],
                            w2[:],
                            acc[:, m0 : m0 + mw],
                            start=True,
                            stop=True,
                        )
                        nc.scalar.activation(zrow[:, m0 : m0 + mw], ps[:, :mw], Copy)
                nc.sync.dma_start(out=zin[:, : NSH // 2], in_=zrow[:, : NSH // 2])
                nc.sync.dma_start(out=zin[:, NSH // 2 :], in_=zrow[:, NSH // 2 : NSH])

            nc.gpsimd.collective_compute(
                "AllGather",
                mybir.AluOpType.bypass,
                replica_groups=[list(range(NCORES))],
                ins=[zin[:].opt()],
                outs=[zall[:].opt()],
            )

            # ---- layer 2 ----
            with (
                tc.tile_pool(name="k2pool", bufs=1) as pool2,
                tc.tile_pool(name="zps2", bufs=2, space="PSUM") as zps2,
            ):
                eidx2 = pool2.tile([P, slots2 // 16], i16)
                perm2 = pool2.tile([P, NP_ // 16], i16)
                accp2 = pool2.tile([P, P2], f32)
                svec = pool2.tile([P, 1], bf16)
                zfin = pool2.tile([1, NP_], f32)
                nc.sync.dma_start(out=eidx2[:], in_=eidx2_d[:])
                nc.sync.dma_start(out=perm2[:], in_=perm2_d[:])
                nc.sync.dma_start(out=svec[:], in_=svec_d[:])
                nc.vector.memset(accp2[:, 0:1], 0.0)
                with tc.tile_pool(name="ztpool", bufs=1) as ztpool, tc.tile_pool(
                    name="g2pool", bufs=2
                ) as g2pool:
                    zt = ztpool.tile([P, NP_], f32)
                    nc.vector.memset(zt[:, 0:1], 0.0)
                    nc.sync.dma_start(out=zt[0:P:16, 1 : 1 + NSH], in_=zall[:, :])
                    by_chunk2 = {}
                    for d_ in descr2:
                        by_chunk2.setdefault(d_[0], []).append(d_)
                    for ch in sorted(by_chunk2):
                        sz = sizes2[ch]
                        g2 = g2pool.tile([P, G2], f32, tag="g2")
                        i0 = int(offs2[ch]) // 16
                        nc.gpsimd.ap_gather(
                            g2[:, :sz],
                            zt[:],
                            eidx2[:, i0 : i0 + sz // 16],
                            channels=P,
                            num_elems=NP_,
                            d=1,
                            num_idxs=sz,
                        )
                        for _, off, n_rows, k, col in by_chunk2[ch]:
                            if k == 1:
                                nc.scalar.activation(
                                    accp2[:, col : col + n_rows],
                                    g2[:, off : off + n_rows],
                                    Copy,
                                )
                            elif k == 2:
                                # pair-adds on GPSIMD (Pool idles post-gather)
                                pairs = g2[:, off : off + 2 * n_rows].rearrange(
                                    "p (a b) -> p a b", a=n_rows, b=2
                                )
                                nc.gpsimd.tensor_add(
                                    accp2[:, col : col + n_rows],
                                    pairs[:, :, 0],
                                    pairs[:, :, 1],
                                )
                            elif k == 3:
                                trip = g2[:, off : off + 3 * n_rows].rearrange(
                                    "p (a b) -> p a b", a=n_rows, b=3
                                )
                                nc.gpsimd.tensor_add(
                                    accp2[:, col : col + n_rows],
                                    trip[:, :, 0],
                                    trip[:, :, 1],
                                )
                                nc.vector.tensor_add(
                                    accp2[:, col : col + n_rows],
                                    accp2[:, col : col + n_rows],
                                    trip[:, :, 2],
                                )
                            else:
                                nc.vector.tensor_reduce(
                                    accp2[:, col : col + n_rows],
                                    g2[:, off : off + n_rows * k].rearrange(
                                        "p (a b) -> p a b", a=n_rows, b=k
                                    ),
                                    axis=mybir.AxisListType.X,
                                    op=ADD,
                                )
                with tc.tile_pool(name="gp2", bufs=1) as gp2pool:
                    g2p = gp2pool.tile([P, NP_], f32)
                    g2pb = gp2pool.tile([P, NP_], bf16)
                    nc.gpsimd.ap_gather(
                        g2p[:],
                        accp2[:],
                        perm2[:],
                        channels=P,
                        num_elems=P2,
                        d=1,
                        num_idxs=NP_,
                    )
                    HB = NP_ // 2
                    for s0 in (0, HB):
                        nc.scalar.activation(
                            g2pb[:, s0 : s0 + HB], g2p[:, s0 : s0 + HB], Copy
                        )
                        for m0 in range(s0, s0 + HB, MMCH):
                            mw = min(MMCH, s0 + HB - m0)
                            ps = zps2.tile([1, MMCH], f32, tag="zp2")
                            nc.tensor.matmul(
                                ps[:, :mw],
                                svec[:],
                                g2pb[:, m0 : m0 + mw],
                                start=True,
                                stop=True,
                            )
                            nc.scalar.activation(zfin[:, m0 : m0 + mw], ps[:, :mw], Copy)
                            # += self-loop term, then * dinv_dst
                            nc.vector.tensor_add(
                                zfin[:, m0 : m0 + mw],
                                zfin[:, m0 : m0 + mw],
                                zrow[:, m0 : m0 + mw],
                            )
                            nc.vector.tensor_mul(
                                zfin[:, m0 : m0 + mw],
                                zfin[:, m0 : m0 + mw],
                                dinvrow2[:, m0 : m0 + mw],
                            )
                        nc.scalar.activation(
                            zfin[:, s0 : s0 + HB],
                            zfin[:, s0 : s0 + HB],
                            Sigmoid,
                            bias=float(b2val),
                        )
                        nc.sync.dma_start(
                            out=out_d[:, s0 : s0 + HB], in_=zfin[:, s0 : s0 + HB]
                        )
    nc.finalize()
    return nc


def _sim_ns(nc):
    from concourse import bass_interp

    sim = bass_interp.CoreSim(nc, no_exec=True, publish_trace=False)
    sim.simulate()
    return int(sim.time)


def kernel(x, edge_index, W1, b1, W2, b2):
    global LAST_SIM_NS
    x = np.asarray(x, dtype=np.float32)
    edge_index = np.asarray(edge_index)
    k1_inputs, meta, (src, dst, dinv) = host_prep(x, edge_index, W1, b1, W2, b2)
    k2_inputs, meta2 = host_prep_k2(src, dst)
    b2val = float(np.asarray(b2, dtype=np.float32).reshape(-1)[0])
    nc = build_fused(meta, meta2, b2val)
    if MEASURE:
        LAST_SIM_NS = _sim_ns(nc)
    in_maps = [dict(k1_inputs[c], **k2_inputs[c]) for c in range(NCORES)]
    res = run_bass_kernel_spmd(nc, in_maps, list(range(NCORES)))
    out = np.zeros((N, 1), dtype=np.float32)
    for c in range(NCORES):
        out[c * NSH : (c + 1) * NSH, 0] = res.results[c]["out"][0, :NSH]
    return out


# revision 59
# speedup vs baseline: 2.8101x; 1.0065x over previous
"""2-layer GCN (PyG GCNConv x2 + sigmoid) on 8 TRN2 NeuronCores, single fused NEFF.

Sharding: dst-node ranges across the 8 cores (6250 nodes each); GCN weights
replicated; the layer-1->layer-2 halo exchange is an on-device AllGather of
each core's 6250 z'=W2^T h' values.

Design notes (cost-model driven):
- ap_gather costs max(table_cols, num_idxs)*0.833ns -> tables and gather
  chunks must be size-matched. 7 src-quarters (table=7144 cols) with 2
  ~8K-slot chunks each keeps L1 gathers slot-optimal (~0.84ns/edge).
- Edge segment sums via exact-degree ladders with layouts shared
  (max-over-core) so one SPMD program fits all cores; k=1 rows are Act
  copies, some k<=2/3 pair-adds go to GPSIMD to balance DVE; per-quarter
  partials are perm-gathered back to node order and accumulated in bf16
  (DVE 2x mode); assembly runs one quarter behind the gathers to keep
  GPSIMD saturated.
- Tables built by PE in bf16 (1 cyc/col); both dinv_dst multiplies fold
  into bf16 tensor ops; h'@W2 contracts on PE in bf16.
- Layer 2: the 8 GPSIMD 16-partition groups each own one SRC CORE RANGE so
  per-group z tables are 6256 wide (table-cost-minimal); self-loops are
  excluded from the edge stream (their term is zrow, added per chunk);
  cross-group partial sums contract on the PE via a stride-16 ones vector
  over the perm-gathered (bf16-converted) partials.
- Finalize and the last quarter's assembly are column-sliced so the
  z-row production chain into the collective stays pipelined.
"""

import sys

sys.path.insert(0, "/opt/trn_rl_repo")
import numpy as np
import ml_dtypes
from contextlib import ExitStack

from concourse import bacc, mybir
from concourse.tile import TileContext
from concourse.bass_utils import run_bass_kernel_spmd

MEASURE = False
LAST_SIM_NS = None

N = 50000
E = 800000
F = 128
P = 128
NCORES = 8
NSH = N // NCORES  # 6250
NQ = 7
QN = 7143  # nodes per quarter (last has 7142)
T = QN + 1  # 7144: [zero col, up to 7143 nodes]
NP_ = 6256  # padded per-core node count
MMCH = 512
XB = 2048


def _wrap16(idx_flat):
    n = idx_flat.shape[0]
    assert n % 16 == 0
    return np.ascontiguousarray(idx_flat.reshape(n // 16, 16).T)


def _pad16(n):
    return ((n + 15) // 16) * 16


def _concat_aranges(lens):
    if len(lens) == 0:
        return np.zeros(0, dtype=np.int64)
    total = int(lens.sum())
    out = np.ones(total, dtype=np.int64)
    ends = np.cumsum(lens)
    out[0] = 0
    out[ends[:-1]] = -(lens[:-1] - 1)
    return np.cumsum(out)


def _ladder_layout(kap_by_unit, n_chunks_cap, kdesc=False):
    """kap_by_unit: [n_units, n_nodes]. Shared exact-k ladder with row-aligned
    chunks, big k first (heavy reduces overlap the next chunk's gather).
    Returns (descr[(chunk, off, n_rows, k, col)], cols, kbase, chunk_sizes)."""
    kmax = int(kap_by_unit.max())
    budgets = {}
    for k in range(1, kmax + 1):
        nk = int((kap_by_unit == k).sum(axis=1).max())
        if nk > 0:
            budgets[k] = nk
    raw = sum(k * nk for k, nk in budgets.items())
    cap = raw + 64 if n_chunks_cap is None else (raw + n_chunks_cap - 1) // n_chunks_cap + 48
    descr, kbase = [], {}
    col = 1
    ch, off = 0, 0
    for k in sorted(budgets, reverse=kdesc):
        nk = budgets[k]
        kbase[k] = col
        left = nk
        while left > 0:
            fit = min(left, (cap - off) // k)
            if fit == 0:
                ch += 1
                off = 0
                fit = min(left, cap // k)
            descr.append((ch, off, fit, k, col))
            off += fit * k
            col += fit
            left -= fit
    chunk_sizes = {}
    for c, o, nr, k, _ in descr:
        chunk_sizes[c] = max(chunk_sizes.get(c, 0), o + nr * k)
    sizes = [_pad16(chunk_sizes[c]) for c in sorted(chunk_sizes)]
    return descr, col, kbase, sizes


def _pack_slots(kap, srcl_by_dst, dstl_by_dst, descr, kbase, cols, chunk_offs):
    """kap: [n_nodes] this unit's degrees; srcl/dstl: this unit's edges sorted
    by dst. Returns (slot_positions, slot_values, perm[node->accp col])."""
    nodes = np.nonzero(kap)[0]
    kn = kap[nodes]
    nd = np.lexsort((nodes, kn))
    nodes_s, kn_s = nodes[nd], kn[nd]
    rank = np.zeros(len(nodes_s), dtype=np.int64)
    colof = np.zeros(len(nodes_s), dtype=np.int64)
    for k in np.unique(kn_s):
        mk = kn_s == k
        rank[mk] = np.arange(mk.sum())
        colof[mk] = kbase[int(k)]
    node_col = colof + rank
    col2slot = np.full(cols, -1, dtype=np.int64)
    for ch, off, n_rows, k, col in descr:
        cc = np.arange(n_rows)
        col2slot[col + cc] = chunk_offs[ch] + off + cc * k
    starts = col2slot[node_col]
    eslots = np.repeat(starts, kn_s) + _concat_aranges(kn_s)
    # edge values in (k, node) order: stable sort of dst-sorted edges by k
    eo = np.argsort(kap[dstl_by_dst], kind="stable")
    ev = srcl_by_dst[eo]
    pm = np.zeros(len(kap), dtype=np.int16)
    pm[nodes_s] = node_col.astype(np.int16)
    return eslots, ev, pm


def host_prep(x, edge_index, W1, b1, W2, b2):
    src = np.concatenate([edge_index[0], np.arange(N, dtype=np.int64)]).astype(np.int32)
    dst = np.concatenate([edge_index[1], np.arange(N, dtype=np.int64)]).astype(np.int32)
    deg = np.bincount(dst, minlength=N).astype(np.float32)
    dinv = 1.0 / np.sqrt(np.maximum(deg, 1e-12))
    dinv[deg <= 0] = 0.0

    # Node -> table position. Stratified round-robin: nodes with identical
    # per-core in-degree vectors spread evenly over quarters, which tightens
    # the shared (max-over-core) ladder budgets vs a random permutation.
    degc = np.zeros((N, NCORES), dtype=np.int32)
    dst_t = np.concatenate([edge_index[1], np.arange(N, dtype=np.int64)])
    src_t = np.concatenate([edge_index[0], np.arange(N, dtype=np.int64)])
    np.add.at(degc, (src_t, dst_t // NSH), 1)
    okey = np.lexsort(tuple(degc[:, c] for c in range(NCORES)))
    rank = np.empty(N, dtype=np.int64)
    rank[okey] = np.arange(N)
    psrc = (rank % NQ) * QN + rank // NQ  # node -> table position
    assert psrc.max() < NQ * QN
    pinv = np.argsort(psrc)

    xtp = (x * dinv[:, None]).T.astype(np.float32)[:, pinv]  # [128, N] pos order
    xt = np.zeros((P, NQ * T), dtype=ml_dtypes.bfloat16)
    for q in range(NQ):
        qn = min(QN, N - q * QN)
        xt[:, q * T + 1 : q * T + 1 + qn] = xtp[:, q * QN : q * QN + qn].astype(
            ml_dtypes.bfloat16
        )

    core = dst // NSH
    dstl = (dst % NSH).astype(np.int64)
    pos = psrc[src]
    quarter = pos // QN
    srcl = (pos % QN).astype(np.int64) + 1

    flat = (core.astype(np.int64) * NQ + quarter) * NSH + dstl
    kap = np.bincount(flat, minlength=NCORES * NQ * NSH).reshape(NCORES, NQ, NSH)

    layouts = []
    for q in range(NQ):
        # last quarter: big-k first so its trailing chunk is reduce-light and
        # the final perm-gather (and the finalize chain) starts sooner
        descr, cols, kbase, sizes = _ladder_layout(kap[:, q, :], 2, kdesc=(q == NQ - 1))
        offs = np.concatenate([[0], np.cumsum(sizes)]).astype(np.int64)
        layouts.append((descr, cols, kbase, sizes, offs))
    SQ = [int(l[4][-1]) for l in layouts]
    PQ = _pad16(max(l[1] for l in layouts))
    G0 = max(max(l[3]) for l in layouts)

    order = np.lexsort((dstl, quarter, core))
    so, do_, qo, co = srcl[order], dstl[order], quarter[order], core[order]

    # combined per-quarter index stream: [SQ[q] slot idxs | NP_ perm idxs]
    qoff = np.concatenate([[0], np.cumsum([s + NP_ for s in SQ])]).astype(np.int64)
    qbase = np.concatenate([[0], np.cumsum(SQ)]).astype(np.int64)
    eidx = np.zeros((NCORES, int(qoff[-1])), dtype=np.int16)
    for c in range(NCORES):
        mc = co == c
        for q in range(NQ):
            m = mc & (qo == q)
            descr, cols, kbase, sizes, offs = layouts[q]
            eslots, ev, pm = _pack_slots(
                kap[c, q], so[m], do_[m], descr, kbase, cols, offs
            )
            eidx[c, qoff[q] + eslots] = ev.astype(np.int16)
            eidx[c, qoff[q] + SQ[q] : qoff[q] + SQ[q] + NSH] = pm

    eidx_w = np.zeros((NCORES, P, int(qoff[-1]) // 16), dtype=np.int16)
    for c in range(NCORES):
        eidx_w[c] = np.tile(_wrap16(eidx[c]), (8, 1))

    dinvb = np.zeros((NCORES, P, NP_), dtype=ml_dtypes.bfloat16)
    dinvrow = np.zeros((NCORES, 1, NP_), dtype=np.float32)
    for c in range(NCORES):
        dv = dinv[c * NSH : (c + 1) * NSH]
        dinvb[c, :, :NSH] = np.tile(dv.astype(ml_dtypes.bfloat16)[None, :], (P, 1))
        dinvrow[c, 0, :NSH] = dv

    meta = dict(layouts=layouts, SQ=SQ, PQ=PQ, G0=G0, qbase=qbase, qoff=qoff)
    k1_inputs = []
    for c in range(NCORES):
        k1_inputs.append(
            {
                "xt": xt,
                "w1": np.asarray(W1, dtype=ml_dtypes.bfloat16),
                "b1": np.asarray(b1, dtype=np.float32).reshape(P, 1),
                "w2": np.asarray(W2, dtype=ml_dtypes.bfloat16).reshape(P, 1),
                "eidx": np.ascontiguousarray(eidx_w[c]),
                "dinvb": np.ascontiguousarray(dinvb[c]),
                "dinvrow": np.ascontiguousarray(dinvrow[c]),
            }
        )
    return k1_inputs, meta, (src, dst, dinv)


def host_prep_k2(src, dst):
    """Layer 2: 8 GPSIMD groups = 8 src core ranges; self-loops excluded."""
    m = src != dst
    s2, d2 = src[m].astype(np.int64), dst[m].astype(np.int64)
    c2 = d2 // NSH
    g2 = s2 // NSH
    dstl = d2 % NSH
    srcl = s2 % NSH + 1

    flat = (c2 * NCORES + g2) * NSH + dstl
    kap2 = np.bincount(flat, minlength=NCORES * NCORES * NSH).reshape(
        NCORES * NCORES, NSH
    )
    descr2, cols2, kbase2, sizes2 = _ladder_layout(kap2, 2, kdesc=True)
    offs2 = np.concatenate([[0], np.cumsum(sizes2)]).astype(np.int64)
    slots2 = int(offs2[-1])
    P2 = _pad16(cols2)

    order = np.lexsort((dstl, g2, c2))
    so, do_, go, co = srcl[order], dstl[order], g2[order], c2[order]
    eidx2 = np.zeros((NCORES, NCORES, slots2), dtype=np.int16)
    perm2 = np.zeros((NCORES, NCORES, NP_), dtype=np.int16)
    for c in range(NCORES):
        mc = co == c
        for g in range(NCORES):
            mm = mc & (go == g)
            eslots, ev, pm = _pack_slots(
                kap2[c * NCORES + g], so[mm], do_[mm], descr2, kbase2, cols2, offs2
            )
            eidx2[c, g, eslots] = ev.astype(np.int16)
            perm2[c, g, :NSH] = pm

    eidx2_w = np.zeros((NCORES, P, slots2 // 16), dtype=np.int16)
    perm2_w = np.zeros((NCORES, P, NP_ // 16), dtype=np.int16)
    for c in range(NCORES):
        for g in range(NCORES):
            eidx2_w[c, g * 16 : (g + 1) * 16] = _wrap16(eidx2[c, g])
            perm2_w[c, g * 16 : (g + 1) * 16] = _wrap16(perm2[c, g])

    svec = np.zeros((P, 1), dtype=ml_dtypes.bfloat16)
    svec[0:P:16, 0] = 1.0  # sum the 8 group-partial rows

    meta2 = dict(descr2=descr2, P2=P2, slots2=slots2, sizes2=sizes2, offs2=offs2)
    k2_inputs = []
    for c in range(NCORES):
        k2_inputs.append(
            {
                "eidx2": np.ascontiguousarray(eidx2_w[c]),
                "perm2": np.ascontiguousarray(perm2_w[c]),
                "svec": svec,
            }
        )
    return k2_inputs, meta2


def build_fused(meta, meta2, b2val):
    layouts, SQ, PQ, G0, qbase, qoff = (
        meta["layouts"],
        meta["SQ"],
        meta["PQ"],
        meta["G0"],
        meta["qbase"],
        meta["qoff"],
    )
    ITW = (max(SQ) + NP_) // 16  # combined per-quarter idx tile width
    descr2, P2, slots2 = meta2["descr2"], meta2["P2"], meta2["slots2"]
    sizes2, offs2 = meta2["sizes2"], meta2["offs2"]
    G2 = max(sizes2)

    nc = bacc.Bacc(None, target_bir_lowering=False)
    f32, f32r, bf16, i16 = (
        mybir.dt.float32,
        mybir.dt.float32r,
        mybir.dt.bfloat16,
        mybir.dt.int16,
    )

    xt_d = nc.dram_tensor("xt", [P, NQ * T], bf16, kind="ExternalInput")
    w1_d = nc.dram_tensor("w1", [P, P], bf16, kind="ExternalInput")
    b1_d = nc.dram_tensor("b1", [P, 1], f32, kind="ExternalInput")
    w2_d = nc.dram_tensor("w2", [P, 1], bf16, kind="ExternalInput")
    eidx_d = nc.dram_tensor("eidx", [P, int(qoff[-1]) // 16], i16, kind="ExternalInput")
    dinvb_d = nc.dram_tensor("dinvb", [P, NP_], bf16, kind="ExternalInput")
    dinvrow_d = nc.dram_tensor("dinvrow", [1, NP_], f32, kind="ExternalInput")
    eidx2_d = nc.dram_tensor("eidx2", [P, slots2 // 16], i16, kind="ExternalInput")
    perm2_d = nc.dram_tensor("perm2", [P, NP_ // 16], i16, kind="ExternalInput")
    svec_d = nc.dram_tensor("svec", [P, 1], bf16, kind="ExternalInput")
    out_d = nc.dram_tensor("out", [1, NP_], f32, kind="ExternalOutput")

    zin = nc.dram_tensor("zin_cc", [1, NSH], f32, kind="Internal")
    zall = nc.dram_tensor(
        "zall_cc", [NCORES, NSH], f32, kind="Internal", addr_space="Shared"
    )

    Copy = mybir.ActivationFunctionType.Copy
    Sigmoid = mybir.ActivationFunctionType.Sigmoid
    ADD = mybir.AluOpType.add

    with ExitStack() as ctx:
        tc = ctx.enter_context(TileContext(nc))
        cpool = ctx.enter_context(tc.tile_pool(name="cpool", bufs=1))
        w1 = cpool.tile([P, P], bf16)
        b1 = cpool.tile([P, 1], f32)
        w2 = cpool.tile([P, 1], bf16)
        acc = cpool.tile([P, NP_], bf16)
        warm = cpool.tile([1, 16], f32)
        nc.sync.dma_start(out=w1[:], in_=w1_d[:])
        nc.sync.dma_start(out=b1[:], in_=b1_d[:])
        nc.sync.dma_start(out=w2[:], in_=w2_d[:])
        # preload the sigmoid activation table off the critical path
        nc.vector.memset(warm[:], 0.0)
        nc.scalar.activation(warm[:], warm[:], Sigmoid, bias=0.0)

        with (
            tc.tile_pool(name="tabs", bufs=2) as tabs,
            tc.tile_pool(name="xpool", bufs=2) as xpool,
            tc.tile_pool(name="gpool", bufs=2) as gpool,
            tc.tile_pool(name="tpool", bufs=1) as tpool,
            tc.tile_pool(name="accpool", bufs=2) as accpool,
            tc.tile_pool(name="epool", bufs=2) as epool,
            tc.tile_pool(name="pspool", bufs=2, space="PSUM") as pspool,
        ):
            accps = {}

            def assemble(q):
                # perm-gather quarter q's partials to node order and fold
                # into acc (emitted one quarter late to keep Pool saturated)
                accp_q, it_q = accps.pop(q)
                p0 = SQ[q] // 16
                tt = gpool.tile([P, G0], f32, tag="g")
                nc.gpsimd.ap_gather(
                    tt[:, :NP_],
                    accp_q[:],
                    it_q[:, p0 : p0 + NP_ // 16],
                    channels=P,
                    num_elems=PQ,
                    d=1,
                    num_idxs=NP_,
                )
                if q == 0:
                    nc.scalar.activation(acc[:], tt[:, :NP_], Copy)
                elif q < NQ - 1:
                    tb = tpool.tile([P, NP_], bf16, tag="tb")
                    nc.scalar.activation(tb[:], tt[:, :NP_], Copy)
                    nc.vector.tensor_add(acc[:], acc[:], tb[:])
                else:
                    # last quarter: slice so finalize can start per-slice
                    tb = tpool.tile([P, NP_], bf16, tag="tb")
                    for s0 in range(0, NP_, 1564):
                        sl = slice(s0, s0 + 1564)
                        nc.scalar.activation(tb[:, sl], tt[:, sl], Copy)
                        nc.vector.tensor_add(acc[:, sl], acc[:, sl], tb[:, sl])

            for q in range(NQ):
                descr, cols, kbase, sizes, offs = layouts[q]
                tab = tabs.tile([P, T], f32, tag="tab")
                chunks = []
                x0 = sum(chunks)
                while x0 < T:
                    chunks.append(min(XB, T - x0))
                    x0 += chunks[-1]
                x0 = 0
                for xw in chunks:
                    xc = xpool.tile([P, XB], bf16, tag="x")
                    nc.sync.dma_start(
                        out=xc[:, :xw], in_=xt_d[:, q * T + x0 : q * T + x0 + xw]
                    )
                    ps = pspool.tile([P, XB], f32, tag="ps")
                    for m0 in range(0, xw, MMCH):
                        mw = min(MMCH, xw - m0)
                        nc.tensor.matmul(
                            ps[:, m0 : m0 + mw],
                            w1[:],
                            xc[:, m0 : m0 + mw],
                            start=True,
                            stop=True,
                        )
                    nc.scalar.activation(tab[:, x0 : x0 + xw], ps[:, :xw], Copy)
                    x0 += xw
                accp = accpool.tile([P, PQ], f32, tag="accp")
                it = epool.tile([P, ITW], i16, tag="it")
                qw = (SQ[q] + NP_) // 16
                i0 = int(qoff[q]) // 16
                nc.sync.dma_start(out=it[:, :qw], in_=eidx_d[:, i0 : i0 + qw])
                accps[q] = (accp, it)
                nc.vector.memset(accp[:, 0:1], 0.0)
                by_chunk = {}
                for d_ in descr:
                    by_chunk.setdefault(d_[0], []).append(d_)
                for ci, ch in enumerate(sorted(by_chunk)):
                    sz = sizes[ch]
                    c0 = int(offs[ch]) // 16
                    g = gpool.tile([P, G0], f32, tag="g")
                    nc.gpsimd.ap_gather(
                        g[:, :sz],
                        tab[:],
                        it[:, c0 : c0 + sz // 16],
                        channels=P,
                        num_elems=T,
                        d=1,
                        num_idxs=sz,
                    )
                    if ci == 0 and q > 0:
                        assemble(q - 1)
                    for _, off, n_rows, k, col in by_chunk[ch]:
                        if k == 1 and ci == 0:
                            # Act handles chunk-0 k=1 rows; later chunks go to
                            # DVE so Act isn't blocked ahead of next tab build
                            nc.scalar.activation(
                                accp[:, col : col + n_rows],
                                g[:, off : off + n_rows],
                                Copy,
                            )
                        elif k == 1:
                            nc.vector.tensor_copy(
                                accp[:, col : col + n_rows], g[:, off : off + n_rows]
                            )
                        elif k == 2 and n_rows >= 48:
                            # rebalance: ~1/3 of pair-adds on GPSIMD
                            nh = _pad16(n_rows * 2 // 3)
                            pr = g[:, off : off + 2 * nh].rearrange(
                                "p (a b) -> p a b", a=nh, b=2
                            )
                            nc.gpsimd.tensor_add(
                                accp[:, col : col + nh], pr[:, :, 0], pr[:, :, 1]
                            )
                            nc.vector.tensor_reduce(
                                accp[:, col + nh : col + n_rows],
                                g[:, off + 2 * nh : off + 2 * n_rows].rearrange(
                                    "p (a b) -> p a b", a=n_rows - nh, b=2
                                ),
                                axis=mybir.AxisListType.X,
                                op=ADD,
                            )
                        else:
                            nc.vector.tensor_reduce(
                                accp[:, col : col + n_rows],
                                g[:, off : off + n_rows * k].rearrange(
                                    "p (a b) -> p a b", a=n_rows, b=k
                                ),
                                axis=mybir.AxisListType.X,
                                op=ADD,
                            )
                if q == NQ - 1:
                    assemble(q)

        # finalize (4 column slices): h' = dinv*sigmoid(dinv*acc+b1); z=W2^T h'
        with (
            tc.tile_pool(name="fin", bufs=1) as fin,
            tc.tile_pool(name="zps", bufs=2, space="PSUM") as zps,
        ):
            zrow = fin.tile([1, NP_], f32)
            dinvrow2 = fin.tile([1, NP_], f32)
            with tc.tile_pool(name="finb", bufs=1) as finb:
                dinvb = finb.tile([P, NP_], bf16)
                nc.sync.dma_start(out=dinvb[:], in_=dinvb_d[:])
                nc.sync.dma_start(out=dinvrow2[:], in_=dinvrow_d[:])
                bounds = [0, 1536, 3072, 4608, NP_]
                for si in range(4):
                    sl = slice(bounds[si], bounds[si + 1])
                    nc.vector.tensor_mul(acc[:, sl], acc[:, sl], dinvb[:, sl])
                    nc.scalar.activation(
                        acc[:, sl], acc[:, sl], Sigmoid, bias=b1[:, 0:1]
                    )
                    nc.vector.tensor_mul(acc[:, sl], acc[:, sl], dinvb[:, sl])
                    for m0 in range(bounds[si], bounds[si + 1], MMCH):
                        mw = min(MMCH, bounds[si + 1] - m0)
                        ps = zps.tile([1, MMCH], f32, tag="zp")
                        nc.tensor.matmul(
                            ps[:, :mw],
                            w2[:],
                            acc[:, m0 : m0 + mw],
                            start=True,
                            stop=True,
                        )
                        nc.scalar.activation(zrow[:, m0 : m0 + mw], ps[:, :mw], Copy)
                nc.sync.dma_start(out=zin[:, : NSH // 2], in_=zrow[:, : NSH // 2])
                nc.sync.dma_start(out=zin[:, NSH // 2 :], in_=zrow[:, NSH // 2 : NSH])

            nc.gpsimd.collective_compute(
                "AllGather",
                mybir.AluOpType.bypass,
                replica_groups=[list(range(NCORES))],
                ins=[zin[:].opt()],
                outs=[zall[:].opt()],
            )

            # ---- layer 2 ----
            with (
                tc.tile_pool(name="k2pool", bufs=1) as pool2,
                tc.tile_pool(name="zps2", bufs=2, space="PSUM") as zps2,
            ):
                eidx2 = pool2.tile([P, slots2 // 16], i16)
                perm2 = pool2.tile([P, NP_ // 16], i16)
                accp2 = pool2.tile([P, P2], f32)
                svec = pool2.tile([P, 1], bf16)
                zfin = pool2.tile([1, NP_], f32)
                nc.sync.dma_start(out=eidx2[:], in_=eidx2_d[:])
                nc.sync.dma_start(out=perm2[:], in_=perm2_d[:])
                nc.sync.dma_start(out=svec[:], in_=svec_d[:])
                nc.vector.memset(accp2[:, 0:1], 0.0)
                with tc.tile_pool(name="ztpool", bufs=1) as ztpool, tc.tile_pool(
                    name="g2pool", bufs=2
                ) as g2pool:
                    zt = ztpool.tile([P, NP_], f32)
                    nc.vector.memset(zt[:, 0:1], 0.0)
                    nc.sync.dma_start(out=zt[0:P:16, 1 : 1 + NSH], in_=zall[:, :])
                    by_chunk2 = {}
                    for d_ in descr2:
                        by_chunk2.setdefault(d_[0], []).append(d_)
                    for ch in sorted(by_chunk2):
                        sz = sizes2[ch]
                        g2 = g2pool.tile([P, G2], f32, tag="g2")
                        i0 = int(offs2[ch]) // 16
                        nc.gpsimd.ap_gather(
                            g2[:, :sz],
                            zt[:],
                            eidx2[:, i0 : i0 + sz // 16],
                            channels=P,
                            num_elems=NP_,
                            d=1,
                            num_idxs=sz,
                        )
                        for _, off, n_rows, k, col in by_chunk2[ch]:
                            if k == 1:
                                nc.scalar.activation(
                                    accp2[:, col : col + n_rows],
                                    g2[:, off : off + n_rows],
                                    Copy,
                                )
                            elif k == 2:
                                # pair-adds on GPSIMD (Pool idles post-gather)
                                pairs = g2[:, off : off + 2 * n_rows].rearrange(
                                    "p (a b) -> p a b", a=n_rows, b=2
                                )
                                nc.gpsimd.tensor_add(
                                    accp2[:, col : col + n_rows],
                                    pairs[:, :, 0],
                                    pairs[:, :, 1],
                                )
                            elif k == 3:
                                trip = g2[:, off : off + 3 * n_rows].rearrange(
                                    "p (a b) -> p a b", a=n_rows, b=3
                                )
                                nc.gpsimd.tensor_add(
                                    accp2[:, col : col + n_rows],
                                    trip[:, :, 0],
                                    trip[:, :, 1],
                                )
                                nc.vector.tensor_add(
                                    accp2[:, col : col + n_rows],
                                    accp2[:, col : col + n_rows],
                                    trip[:, :, 2],
                                )
                            else:
                                nc.vector.tensor_reduce(
                                    accp2[:, col : col + n_rows],
                                    g2[:, off : off + n_rows * k].rearrange(
                                        "p (a b) -> p a b", a=n_rows, b=k
                                    ),
                                    axis=mybir.AxisListType.X,
                                    op=ADD,
                                )
                with tc.tile_pool(name="gp2", bufs=1) as gp2pool:
                    g2p = gp2pool.tile([P, NP_], f32)
                    g2pb = gp2pool.tile([P, NP_], bf16)
                    nc.gpsimd.ap_gather(
                        g2p[:],
                        accp2[:],
                        perm2[:],
                        channels=P,
                        num_elems=P2,
                        d=1,
                        num_idxs=NP_,
                    )
                    HB = NP_ // 2
                    for s0 in (0, HB):
                        nc.scalar.activation(
                            g2pb[:, s0 : s0 + HB], g2p[:, s0 : s0 + HB], Copy
                        )
                        for m0 in range(s0, s0 + HB, MMCH):
                            mw = min(MMCH, s0 + HB - m0)
                            ps = zps2.tile([1, MMCH], f32, tag="zp2")
                            nc.tensor.matmul(
                                ps[:, :mw],
                                svec[:],
                                g2pb[:, m0 : m0 + mw],
                                start=True,
                                stop=True,
                            )
                            nc.scalar.activation(zfin[:, m0 : m0 + mw], ps[:, :mw], Copy)
                            # += self-loop term, then * dinv_dst
                            nc.vector.tensor_add(
                                zfin[:, m0 : m0 + mw],
                                zfin[:, m0 : m0 + mw],
                                zrow[:, m0 : m0 + mw],
                            )
                            nc.vector.tensor_mul(
                                zfin[:, m0 : m0 + mw],
                                zfin[:, m0 : m0 + mw],
                                dinvrow2[:, m0 : m0 + mw],
                            )
                        nc.scalar.activation(
                            zfin[:, s0 : s0 + HB],
                            zfin[:, s0 : s0 + HB],
                            Sigmoid,
                            bias=float(b2val),
                        )
                        nc.sync.dma_start(
                            out=out_d[:, s0 : s0 + HB], in_=zfin[:, s0 : s0 + HB]
                        )
    nc.finalize()
    return nc


def _sim_ns(nc):
    from concourse import bass_interp

    sim = bass_interp.CoreSim(nc, no_exec=True, publish_trace=False)
    sim.simulate()
    return int(sim.time)


def kernel(x, edge_index, W1, b1, W2, b2):
    global LAST_SIM_NS
    x = np.asarray(x, dtype=np.float32)
    edge_index = np.asarray(edge_index)
    k1_inputs, meta, (src, dst, dinv) = host_prep(x, edge_index, W1, b1, W2, b2)
    k2_inputs, meta2 = host_prep_k2(src, dst)
    b2val = float(np.asarray(b2, dtype=np.float32).reshape(-1)[0])
    nc = build_fused(meta, meta2, b2val)
    if MEASURE:
        LAST_SIM_NS = _sim_ns(nc)
    in_maps = [dict(k1_inputs[c], **k2_inputs[c]) for c in range(NCORES)]
    res = run_bass_kernel_spmd(nc, in_maps, list(range(NCORES)))
    out = np.zeros((N, 1), dtype=np.float32)
    for c in range(NCORES):
        out[c * NSH : (c + 1) * NSH, 0] = res.results[c]["out"][0, :NSH]
    return out


# revision 60
# speedup vs baseline: 2.8156x; 1.0019x over previous
"""2-layer GCN (PyG GCNConv x2 + sigmoid) on 8 TRN2 NeuronCores, single fused NEFF.

Sharding: dst-node ranges across the 8 cores (6250 nodes each); GCN weights
replicated; the layer-1->layer-2 halo exchange is an on-device AllGather of
each core's 6250 z'=W2^T h' values.

Design notes (cost-model driven):
- ap_gather costs max(table_cols, num_idxs)*0.833ns -> tables and gather
  chunks must be size-matched. 7 src-quarters (table=7144 cols) with 2
  ~8K-slot chunks each keeps L1 gathers slot-optimal (~0.84ns/edge).
- Edge segment sums via exact-degree ladders with layouts shared
  (max-over-core) so one SPMD program fits all cores; k=1 rows are Act
  copies, some k<=2/3 pair-adds go to GPSIMD to balance DVE; per-quarter
  partials are perm-gathered back to node order and accumulated in bf16
  (DVE 2x mode); assembly runs one quarter behind the gathers to keep
  GPSIMD saturated.
- Tables built by PE in bf16 (1 cyc/col); both dinv_dst multiplies fold
  into bf16 tensor ops; h'@W2 contracts on PE in bf16.
- Layer 2: the 8 GPSIMD 16-partition groups each own one SRC CORE RANGE so
  per-group z tables are 6256 wide (table-cost-minimal); self-loops are
  excluded from the edge stream (their term is zrow, added per chunk);
  cross-group partial sums contract on the PE via a stride-16 ones vector
  over the perm-gathered (bf16-converted) partials.
- Finalize and the last quarter's assembly are column-sliced so the
  z-row production chain into the collective stays pipelined.
"""

import sys

sys.path.insert(0, "/opt/trn_rl_repo")
import numpy as np
import ml_dtypes
from contextlib import ExitStack

from concourse import bacc, mybir
from concourse.tile import TileContext
from concourse.bass_utils import run_bass_kernel_spmd

MEASURE = False
LAST_SIM_NS = None

N = 50000
E = 800000
F = 128
P = 128
NCORES = 8
NSH = N // NCORES  # 6250
NQ = 7
QN = 7143  # nodes per quarter (last has 7142)
T = QN + 1  # 7144: [zero col, up to 7143 nodes]
NP_ = 6256  # padded per-core node count
MMCH = 512
XB = 2048


def _wrap16(idx_flat):
    n = idx_flat.shape[0]
    assert n % 16 == 0
    return np.ascontiguousarray(idx_flat.reshape(n // 16, 16).T)


def _pad16(n):
    return ((n + 15) // 16) * 16


def _concat_aranges(lens):
    if len(lens) == 0:
        return np.zeros(0, dtype=np.int64)
    total = int(lens.sum())
    out = np.ones(total, dtype=np.int64)
    ends = np.cumsum(lens)
    out[0] = 0
    out[ends[:-1]] = -(lens[:-1] - 1)
    return np.cumsum(out)


def _ladder_layout(kap_by_unit, n_chunks_cap, kdesc=False):
    """kap_by_unit: [n_units, n_nodes]. Shared exact-k ladder with row-aligned
    chunks, big k first (heavy reduces overlap the next chunk's gather).
    Returns (descr[(chunk, off, n_rows, k, col)], cols, kbase, chunk_sizes)."""
    kmax = int(kap_by_unit.max())
    budgets = {}
    for k in range(1, kmax + 1):
        nk = int((kap_by_unit == k).sum(axis=1).max())
        if nk > 0:
            budgets[k] = nk
    raw = sum(k * nk for k, nk in budgets.items())
    cap = raw + 64 if n_chunks_cap is None else (raw + n_chunks_cap - 1) // n_chunks_cap + 48
    descr, kbase = [], {}
    col = 1
    ch, off = 0, 0
    for k in sorted(budgets, reverse=kdesc):
        nk = budgets[k]
        kbase[k] = col
        left = nk
        while left > 0:
            fit = min(left, (cap - off) // k)
            if fit == 0:
                ch += 1
                off = 0
                fit = min(left, cap // k)
            descr.append((ch, off, fit, k, col))
            off += fit * k
            col += fit
            left -= fit
    chunk_sizes = {}
    for c, o, nr, k, _ in descr:
        chunk_sizes[c] = max(chunk_sizes.get(c, 0), o + nr * k)
    sizes = [_pad16(chunk_sizes[c]) for c in sorted(chunk_sizes)]
    return descr, col, kbase, sizes


def _pack_slots(kap, srcl_by_dst, dstl_by_dst, descr, kbase, cols, chunk_offs):
    """kap: [n_nodes] this unit's degrees; srcl/dstl: this unit's edges sorted
    by dst. Returns (slot_positions, slot_values, perm[node->accp col])."""
    nodes = np.nonzero(kap)[0]
    kn = kap[nodes]
    nd = np.lexsort((nodes, kn))
    nodes_s, kn_s = nodes[nd], kn[nd]
    rank = np.zeros(len(nodes_s), dtype=np.int64)
    colof = np.zeros(len(nodes_s), dtype=np.int64)
    for k in np.unique(kn_s):
        mk = kn_s == k
        rank[mk] = np.arange(mk.sum())
        colof[mk] = kbase[int(k)]
    node_col = colof + rank
    col2slot = np.full(cols, -1, dtype=np.int64)
    for ch, off, n_rows, k, col in descr:
        cc = np.arange(n_rows)
        col2slot[col + cc] = chunk_offs[ch] + off + cc * k
    starts = col2slot[node_col]
    eslots = np.repeat(starts, kn_s) + _concat_aranges(kn_s)
    # edge values in (k, node) order: stable sort of dst-sorted edges by k
    eo = np.argsort(kap[dstl_by_dst], kind="stable")
    ev = srcl_by_dst[eo]
    pm = np.zeros(len(kap), dtype=np.int16)
    pm[nodes_s] = node_col.astype(np.int16)
    return eslots, ev, pm


def host_prep(x, edge_index, W1, b1, W2, b2):
    src = np.concatenate([edge_index[0], np.arange(N, dtype=np.int64)]).astype(np.int32)
    dst = np.concatenate([edge_index[1], np.arange(N, dtype=np.int64)]).astype(np.int32)
    deg = np.bincount(dst, minlength=N).astype(np.float32)
    dinv = 1.0 / np.sqrt(np.maximum(deg, 1e-12))
    dinv[deg <= 0] = 0.0

    # Node -> table position. Stratified round-robin: nodes with identical
    # per-core in-degree vectors spread evenly over quarters, which tightens
    # the shared (max-over-core) ladder budgets vs a random permutation.
    degc = np.zeros((N, NCORES), dtype=np.int32)
    dst_t = np.concatenate([edge_index[1], np.arange(N, dtype=np.int64)])
    src_t = np.concatenate([edge_index[0], np.arange(N, dtype=np.int64)])
    np.add.at(degc, (src_t, dst_t // NSH), 1)
    okey = np.lexsort(tuple(degc[:, c] for c in range(NCORES)))
    rank = np.empty(N, dtype=np.int64)
    rank[okey] = np.arange(N)
    psrc = (rank % NQ) * QN + rank // NQ  # node -> table position
    assert psrc.max() < NQ * QN
    pinv = np.argsort(psrc)

    xtp = (x * dinv[:, None]).T.astype(np.float32)[:, pinv]  # [128, N] pos order
    xt = np.zeros((P, NQ * T), dtype=ml_dtypes.bfloat16)
    for q in range(NQ):
        qn = min(QN, N - q * QN)
        xt[:, q * T + 1 : q * T + 1 + qn] = xtp[:, q * QN : q * QN + qn].astype(
            ml_dtypes.bfloat16
        )

    core = dst // NSH
    dstl = (dst % NSH).astype(np.int64)
    pos = psrc[src]
    quarter = pos // QN
    srcl = (pos % QN).astype(np.int64) + 1

    flat = (core.astype(np.int64) * NQ + quarter) * NSH + dstl
    kap = np.bincount(flat, minlength=NCORES * NQ * NSH).reshape(NCORES, NQ, NSH)

    layouts = []
    for q in range(NQ):
        # last quarter: big-k first so its trailing chunk is reduce-light and
        # the final perm-gather (and the finalize chain) starts sooner
        descr, cols, kbase, sizes = _ladder_layout(kap[:, q, :], 2, kdesc=(q >= NQ - 2))
        offs = np.concatenate([[0], np.cumsum(sizes)]).astype(np.int64)
        layouts.append((descr, cols, kbase, sizes, offs))
    SQ = [int(l[4][-1]) for l in layouts]
    PQ = _pad16(max(l[1] for l in layouts))
    G0 = max(max(l[3]) for l in layouts)

    order = np.lexsort((dstl, quarter, core))
    so, do_, qo, co = srcl[order], dstl[order], quarter[order], core[order]

    # combined per-quarter index stream: [SQ[q] slot idxs | NP_ perm idxs]
    qoff = np.concatenate([[0], np.cumsum([s + NP_ for s in SQ])]).astype(np.int64)
    qbase = np.concatenate([[0], np.cumsum(SQ)]).astype(np.int64)
    eidx = np.zeros((NCORES, int(qoff[-1])), dtype=np.int16)
    for c in range(NCORES):
        mc = co == c
        for q in range(NQ):
            m = mc & (qo == q)
            descr, cols, kbase, sizes, offs = layouts[q]
            eslots, ev, pm = _pack_slots(
                kap[c, q], so[m], do_[m], descr, kbase, cols, offs
            )
            eidx[c, qoff[q] + eslots] = ev.astype(np.int16)
            eidx[c, qoff[q] + SQ[q] : qoff[q] + SQ[q] + NSH] = pm

    eidx_w = np.zeros((NCORES, P, int(qoff[-1]) // 16), dtype=np.int16)
    for c in range(NCORES):
        eidx_w[c] = np.tile(_wrap16(eidx[c]), (8, 1))

    dinvb = np.zeros((NCORES, P, NP_), dtype=ml_dtypes.bfloat16)
    dinvrow = np.zeros((NCORES, 1, NP_), dtype=np.float32)
    for c in range(NCORES):
        dv = dinv[c * NSH : (c + 1) * NSH]
        dinvb[c, :, :NSH] = np.tile(dv.astype(ml_dtypes.bfloat16)[None, :], (P, 1))
        dinvrow[c, 0, :NSH] = dv

    meta = dict(layouts=layouts, SQ=SQ, PQ=PQ, G0=G0, qbase=qbase, qoff=qoff)
    k1_inputs = []
    for c in range(NCORES):
        k1_inputs.append(
            {
                "xt": xt,
                "w1": np.asarray(W1, dtype=ml_dtypes.bfloat16),
                "b1": np.asarray(b1, dtype=np.float32).reshape(P, 1),
                "w2": np.asarray(W2, dtype=ml_dtypes.bfloat16).reshape(P, 1),
                "eidx": np.ascontiguousarray(eidx_w[c]),
                "dinvb": np.ascontiguousarray(dinvb[c]),
                "dinvrow": np.ascontiguousarray(dinvrow[c]),
            }
        )
    return k1_inputs, meta, (src, dst, dinv)


def host_prep_k2(src, dst):
    """Layer 2: 8 GPSIMD groups = 8 src core ranges; self-loops excluded."""
    m = src != dst
    s2, d2 = src[m].astype(np.int64), dst[m].astype(np.int64)
    c2 = d2 // NSH
    g2 = s2 // NSH
    dstl = d2 % NSH
    srcl = s2 % NSH + 1

    flat = (c2 * NCORES + g2) * NSH + dstl
    kap2 = np.bincount(flat, minlength=NCORES * NCORES * NSH).reshape(
        NCORES * NCORES, NSH
    )
    descr2, cols2, kbase2, sizes2 = _ladder_layout(kap2, 2, kdesc=True)
    offs2 = np.concatenate([[0], np.cumsum(sizes2)]).astype(np.int64)
    slots2 = int(offs2[-1])
    P2 = _pad16(cols2)

    order = np.lexsort((dstl, g2, c2))
    so, do_, go, co = srcl[order], dstl[order], g2[order], c2[order]
    eidx2 = np.zeros((NCORES, NCORES, slots2), dtype=np.int16)
    perm2 = np.zeros((NCORES, NCORES, NP_), dtype=np.int16)
    for c in range(NCORES):
        mc = co == c
        for g in range(NCORES):
            mm = mc & (go == g)
            eslots, ev, pm = _pack_slots(
                kap2[c * NCORES + g], so[mm], do_[mm], descr2, kbase2, cols2, offs2
            )
            eidx2[c, g, eslots] = ev.astype(np.int16)
            perm2[c, g, :NSH] = pm

    eidx2_w = np.zeros((NCORES, P, slots2 // 16), dtype=np.int16)
    perm2_w = np.zeros((NCORES, P, NP_ // 16), dtype=np.int16)
    for c in range(NCORES):
        for g in range(NCORES):
            eidx2_w[c, g * 16 : (g + 1) * 16] = _wrap16(eidx2[c, g])
            perm2_w[c, g * 16 : (g + 1) * 16] = _wrap16(perm2[c, g])

    svec = np.zeros((P, 1), dtype=ml_dtypes.bfloat16)
    svec[0:P:16, 0] = 1.0  # sum the 8 group-partial rows

    meta2 = dict(descr2=descr2, P2=P2, slots2=slots2, sizes2=sizes2, offs2=offs2)
    k2_inputs = []
    for c in range(NCORES):
        k2_inputs.append(
            {
                "eidx2": np.ascontiguousarray(eidx2_w[c]),
                "perm2": np.ascontiguousarray(perm2_w[c]),
                "svec": svec,
            }
        )
    return k2_inputs, meta2


def build_fused(meta, meta2, b2val):
    layouts, SQ, PQ, G0, qbase, qoff = (
        meta["layouts"],
        meta["SQ"],
        meta["PQ"],
        meta["G0"],
        meta["qbase"],
        meta["qoff"],
    )
    ITW = (max(SQ) + NP_) // 16  # combined per-quarter idx tile width
    descr2, P2, slots2 = meta2["descr2"], meta2["P2"], meta2["slots2"]
    sizes2, offs2 = meta2["sizes2"], meta2["offs2"]
    G2 = max(sizes2)

    nc = bacc.Bacc(None, target_bir_lowering=False)
    f32, f32r, bf16, i16 = (
        mybir.dt.float32,
        mybir.dt.float32r,
        mybir.dt.bfloat16,
        mybir.dt.int16,
    )

    xt_d = nc.dram_tensor("xt", [P, NQ * T], bf16, kind="ExternalInput")
    w1_d = nc.dram_tensor("w1", [P, P], bf16, kind="ExternalInput")
    b1_d = nc.dram_tensor("b1", [P, 1], f32, kind="ExternalInput")
    w2_d = nc.dram_tensor("w2", [P, 1], bf16, kind="ExternalInput")
    eidx_d = nc.dram_tensor("eidx", [P, int(qoff[-1]) // 16], i16, kind="ExternalInput")
    dinvb_d = nc.dram_tensor("dinvb", [P, NP_], bf16, kind="ExternalInput")
    dinvrow_d = nc.dram_tensor("dinvrow", [1, NP_], f32, kind="ExternalInput")
    eidx2_d = nc.dram_tensor("eidx2", [P, slots2 // 16], i16, kind="ExternalInput")
    perm2_d = nc.dram_tensor("perm2", [P, NP_ // 16], i16, kind="ExternalInput")
    svec_d = nc.dram_tensor("svec", [P, 1], bf16, kind="ExternalInput")
    out_d = nc.dram_tensor("out", [1, NP_], f32, kind="ExternalOutput")

    zin = nc.dram_tensor("zin_cc", [1, NSH], f32, kind="Internal")
    zall = nc.dram_tensor(
        "zall_cc", [NCORES, NSH], f32, kind="Internal", addr_space="Shared"
    )

    Copy = mybir.ActivationFunctionType.Copy
    Sigmoid = mybir.ActivationFunctionType.Sigmoid
    ADD = mybir.AluOpType.add

    with ExitStack() as ctx:
        tc = ctx.enter_context(TileContext(nc))
        cpool = ctx.enter_context(tc.tile_pool(name="cpool", bufs=1))
        w1 = cpool.tile([P, P], bf16)
        b1 = cpool.tile([P, 1], f32)
        w2 = cpool.tile([P, 1], bf16)
        acc = cpool.tile([P, NP_], bf16)
        warm = cpool.tile([1, 16], f32)
        nc.sync.dma_start(out=w1[:], in_=w1_d[:])
        nc.sync.dma_start(out=b1[:], in_=b1_d[:])
        nc.sync.dma_start(out=w2[:], in_=w2_d[:])
        # preload the sigmoid activation table off the critical path
        nc.vector.memset(warm[:], 0.0)
        nc.scalar.activation(warm[:], warm[:], Sigmoid, bias=0.0)

        with (
            tc.tile_pool(name="tabs", bufs=2) as tabs,
            tc.tile_pool(name="xpool", bufs=2) as xpool,
            tc.tile_pool(name="gpool", bufs=2) as gpool,
            tc.tile_pool(name="tpool", bufs=1) as tpool,
            tc.tile_pool(name="accpool", bufs=2) as accpool,
            tc.tile_pool(name="epool", bufs=2) as epool,
            tc.tile_pool(name="pspool", bufs=2, space="PSUM") as pspool,
        ):
            accps = {}

            def assemble(q):
                # perm-gather quarter q's partials to node order and fold
                # into acc (emitted one quarter late to keep Pool saturated)
                accp_q, it_q = accps.pop(q)
                p0 = SQ[q] // 16
                tt = gpool.tile([P, G0], f32, tag="g")
                nc.gpsimd.ap_gather(
                    tt[:, :NP_],
                    accp_q[:],
                    it_q[:, p0 : p0 + NP_ // 16],
                    channels=P,
                    num_elems=PQ,
                    d=1,
                    num_idxs=NP_,
                )
                if q == 0:
                    nc.scalar.activation(acc[:], tt[:, :NP_], Copy)
                elif q < NQ - 1:
                    tb = tpool.tile([P, NP_], bf16, tag="tb")
                    nc.scalar.activation(tb[:], tt[:, :NP_], Copy)
                    nc.vector.tensor_add(acc[:], acc[:], tb[:])
                else:
                    # last quarter: slice so finalize can start per-slice
                    tb = tpool.tile([P, NP_], bf16, tag="tb")
                    for s0 in range(0, NP_, 1564):
                        sl = slice(s0, s0 + 1564)
                        nc.scalar.activation(tb[:, sl], tt[:, sl], Copy)
                        nc.vector.tensor_add(acc[:, sl], acc[:, sl], tb[:, sl])

            for q in range(NQ):
                descr, cols, kbase, sizes, offs = layouts[q]
                tab = tabs.tile([P, T], f32, tag="tab")
                chunks = []
                x0 = sum(chunks)
                while x0 < T:
                    chunks.append(min(XB, T - x0))
                    x0 += chunks[-1]
                x0 = 0
                for xw in chunks:
                    xc = xpool.tile([P, XB], bf16, tag="x")
                    nc.sync.dma_start(
                        out=xc[:, :xw], in_=xt_d[:, q * T + x0 : q * T + x0 + xw]
                    )
                    ps = pspool.tile([P, XB], f32, tag="ps")
                    for m0 in range(0, xw, MMCH):
                        mw = min(MMCH, xw - m0)
                        nc.tensor.matmul(
                            ps[:, m0 : m0 + mw],
                            w1[:],
                            xc[:, m0 : m0 + mw],
                            start=True,
                            stop=True,
                        )
                    nc.scalar.activation(tab[:, x0 : x0 + xw], ps[:, :xw], Copy)
                    x0 += xw
                accp = accpool.tile([P, PQ], f32, tag="accp")
                it = epool.tile([P, ITW], i16, tag="it")
                qw = (SQ[q] + NP_) // 16
                i0 = int(qoff[q]) // 16
                nc.sync.dma_start(out=it[:, :qw], in_=eidx_d[:, i0 : i0 + qw])
                accps[q] = (accp, it)
                nc.vector.memset(accp[:, 0:1], 0.0)
                by_chunk = {}
                for d_ in descr:
                    by_chunk.setdefault(d_[0], []).append(d_)
                for ci, ch in enumerate(sorted(by_chunk)):
                    sz = sizes[ch]
                    c0 = int(offs[ch]) // 16
                    g = gpool.tile([P, G0], f32, tag="g")
                    nc.gpsimd.ap_gather(
                        g[:, :sz],
                        tab[:],
                        it[:, c0 : c0 + sz // 16],
                        channels=P,
                        num_elems=T,
                        d=1,
                        num_idxs=sz,
                    )
                    if ci == 0 and q > 0:
                        assemble(q - 1)
                    for _, off, n_rows, k, col in by_chunk[ch]:
                        if k == 1 and ci == 0:
                            # Act handles chunk-0 k=1 rows; later chunks go to
                            # DVE so Act isn't blocked ahead of next tab build
                            nc.scalar.activation(
                                accp[:, col : col + n_rows],
                                g[:, off : off + n_rows],
                                Copy,
                            )
                        elif k == 1:
                            nc.vector.tensor_copy(
                                accp[:, col : col + n_rows], g[:, off : off + n_rows]
                            )
                        elif k == 2 and n_rows >= 48:
                            # rebalance: ~1/3 of pair-adds on GPSIMD
                            nh = _pad16(n_rows * 2 // 3)
                            pr = g[:, off : off + 2 * nh].rearrange(
                                "p (a b) -> p a b", a=nh, b=2
                            )
                            nc.gpsimd.tensor_add(
                                accp[:, col : col + nh], pr[:, :, 0], pr[:, :, 1]
                            )
                            nc.vector.tensor_reduce(
                                accp[:, col + nh : col + n_rows],
                                g[:, off + 2 * nh : off + 2 * n_rows].rearrange(
                                    "p (a b) -> p a b", a=n_rows - nh, b=2
                                ),
                                axis=mybir.AxisListType.X,
                                op=ADD,
                            )
                        else:
                            nc.vector.tensor_reduce(
                                accp[:, col : col + n_rows],
                                g[:, off : off + n_rows * k].rearrange(
                                    "p (a b) -> p a b", a=n_rows, b=k
                                ),
                                axis=mybir.AxisListType.X,
                                op=ADD,
                            )
                if q == NQ - 1:
                    assemble(q)

        # finalize (4 column slices): h' = dinv*sigmoid(dinv*acc+b1); z=W2^T h'
        with (
            tc.tile_pool(name="fin", bufs=1) as fin,
            tc.tile_pool(name="zps", bufs=2, space="PSUM") as zps,
        ):
            zrow = fin.tile([1, NP_], f32)
            dinvrow2 = fin.tile([1, NP_], f32)
            with tc.tile_pool(name="finb", bufs=1) as finb:
                dinvb = finb.tile([P, NP_], bf16)
                nc.sync.dma_start(out=dinvb[:], in_=dinvb_d[:])
                nc.sync.dma_start(out=dinvrow2[:], in_=dinvrow_d[:])
                bounds = [0, 1536, 3072, 4608, NP_]
                for si in range(4):
                    sl = slice(bounds[si], bounds[si + 1])
                    nc.vector.tensor_mul(acc[:, sl], acc[:, sl], dinvb[:, sl])
                    nc.scalar.activation(
                        acc[:, sl], acc[:, sl], Sigmoid, bias=b1[:, 0:1]
                    )
                    nc.vector.tensor_mul(acc[:, sl], acc[:, sl], dinvb[:, sl])
                    for m0 in range(bounds[si], bounds[si + 1], MMCH):
                        mw = min(MMCH, bounds[si + 1] - m0)
                        ps = zps.tile([1, MMCH], f32, tag="zp")
                        nc.tensor.matmul(
                            ps[:, :mw],
                            w2[:],
                            acc[:, m0 : m0 + mw],
                            start=True,
                            stop=True,
                        )
                        nc.scalar.activation(zrow[:, m0 : m0 + mw], ps[:, :mw], Copy)
                nc.sync.dma_start(out=zin[:, : NSH // 2], in_=zrow[:, : NSH // 2])
                nc.sync.dma_start(out=zin[:, NSH // 2 :], in_=zrow[:, NSH // 2 : NSH])

            nc.gpsimd.collective_compute(
                "AllGather",
                mybir.AluOpType.bypass,
                replica_groups=[list(range(NCORES))],
                ins=[zin[:].opt()],
                outs=[zall[:].opt()],
            )

            # ---- layer 2 ----
            with (
                tc.tile_pool(name="k2pool", bufs=1) as pool2,
                tc.tile_pool(name="zps2", bufs=2, space="PSUM") as zps2,
            ):
                eidx2 = pool2.tile([P, slots2 // 16], i16)
                perm2 = pool2.tile([P, NP_ // 16], i16)
                accp2 = pool2.tile([P, P2], f32)
                svec = pool2.tile([P, 1], bf16)
                zfin = pool2.tile([1, NP_], f32)
                nc.sync.dma_start(out=eidx2[:], in_=eidx2_d[:])
                nc.sync.dma_start(out=perm2[:], in_=perm2_d[:])
                nc.sync.dma_start(out=svec[:], in_=svec_d[:])
                nc.vector.memset(accp2[:, 0:1], 0.0)
                with tc.tile_pool(name="ztpool", bufs=1) as ztpool, tc.tile_pool(
                    name="g2pool", bufs=2
                ) as g2pool:
                    zt = ztpool.tile([P, NP_], f32)
                    nc.vector.memset(zt[:, 0:1], 0.0)
                    nc.sync.dma_start(out=zt[0:P:16, 1 : 1 + NSH], in_=zall[:, :])
                    by_chunk2 = {}
                    for d_ in descr2:
                        by_chunk2.setdefault(d_[0], []).append(d_)
                    for ch in sorted(by_chunk2):
                        sz = sizes2[ch]
                        g2 = g2pool.tile([P, G2], f32, tag="g2")
                        i0 = int(offs2[ch]) // 16
                        nc.gpsimd.ap_gather(
                            g2[:, :sz],
                            zt[:],
                            eidx2[:, i0 : i0 + sz // 16],
                            channels=P,
                            num_elems=NP_,
                            d=1,
                            num_idxs=sz,
                        )
                        for _, off, n_rows, k, col in by_chunk2[ch]:
                            if k == 1:
                                nc.scalar.activation(
                                    accp2[:, col : col + n_rows],
                                    g2[:, off : off + n_rows],
                                    Copy,
                                )
                            elif k == 2:
                                # pair-adds on GPSIMD (Pool idles post-gather)
                                pairs = g2[:, off : off + 2 * n_rows].rearrange(
                                    "p (a b) -> p a b", a=n_rows, b=2
                                )
                                nc.gpsimd.tensor_add(
                                    accp2[:, col : col + n_rows],
                                    pairs[:, :, 0],
                                    pairs[:, :, 1],
                                )
                            elif k == 3:
                                trip = g2[:, off : off + 3 * n_rows].rearrange(
                                    "p (a b) -> p a b", a=n_rows, b=3
                                )
                                nc.gpsimd.tensor_add(
                                    accp2[:, col : col + n_rows],
                                    trip[:, :, 0],
                                    trip[:, :, 1],
                                )
                                nc.vector.tensor_add(
                                    accp2[:, col : col + n_rows],
                                    accp2[:, col : col + n_rows],
                                    trip[:, :, 2],
                                )
                            else:
                                nc.vector.tensor_reduce(
                                    accp2[:, col : col + n_rows],
                                    g2[:, off : off + n_rows * k].rearrange(
                                        "p (a b) -> p a b", a=n_rows, b=k
                                    ),
                                    axis=mybir.AxisListType.X,
                                    op=ADD,
                                )
                with tc.tile_pool(name="gp2", bufs=1) as gp2pool:
                    g2p = gp2pool.tile([P, NP_], f32)
                    g2pb = gp2pool.tile([P, NP_], bf16)
                    nc.gpsimd.ap_gather(
                        g2p[:],
                        accp2[:],
                        perm2[:],
                        channels=P,
                        num_elems=P2,
                        d=1,
                        num_idxs=NP_,
                    )
                    HB = NP_ // 2
                    for s0 in (0, HB):
                        nc.scalar.activation(
                            g2pb[:, s0 : s0 + HB], g2p[:, s0 : s0 + HB], Copy
                        )
                        for m0 in range(s0, s0 + HB, MMCH):
                            mw = min(MMCH, s0 + HB - m0)
                            ps = zps2.tile([1, MMCH], f32, tag="zp2")
                            nc.tensor.matmul(
                                ps[:, :mw],
                                svec[:],
                                g2pb[:, m0 : m0 + mw],
                                start=True,
                                stop=True,
                            )
                            nc.scalar.activation(zfin[:, m0 : m0 + mw], ps[:, :mw], Copy)
                            # += self-loop term, then * dinv_dst
                            nc.vector.tensor_add(
                                zfin[:, m0 : m0 + mw],
                                zfin[:, m0 : m0 + mw],
                                zrow[:, m0 : m0 + mw],
                            )
                            nc.vector.tensor_mul(
                                zfin[:, m0 : m0 + mw],
                                zfin[:, m0 : m0 + mw],
                                dinvrow2[:, m0 : m0 + mw],
                            )
                        nc.scalar.activation(
                            zfin[:, s0 : s0 + HB],
                            zfin[:, s0 : s0 + HB],
                            Sigmoid,
                            bias=float(b2val),
                        )
                        nc.sync.dma_start(
                            out=out_d[:, s0 : s0 + HB], in_=zfin[:, s0 : s0 + HB]
                        )
    nc.finalize()
    return nc


def _sim_ns(nc):
    from concourse import bass_interp

    sim = bass_interp.CoreSim(nc, no_exec=True, publish_trace=False)
    sim.simulate()
    return int(sim.time)


def kernel(x, edge_index, W1, b1, W2, b2):
    global LAST_SIM_NS
    x = np.asarray(x, dtype=np.float32)
    edge_index = np.asarray(edge_index)
    k1_inputs, meta, (src, dst, dinv) = host_prep(x, edge_index, W1, b1, W2, b2)
    k2_inputs, meta2 = host_prep_k2(src, dst)
    b2val = float(np.asarray(b2, dtype=np.float32).reshape(-1)[0])
    nc = build_fused(meta, meta2, b2val)
    if MEASURE:
        LAST_SIM_NS = _sim_ns(nc)
    in_maps = [dict(k1_inputs[c], **k2_inputs[c]) for c in range(NCORES)]
    res = run_bass_kernel_spmd(nc, in_maps, list(range(NCORES)))
    out = np.zeros((N, 1), dtype=np.float32)
    for c in range(NCORES):
        out[c * NSH : (c + 1) * NSH, 0] = res.results[c]["out"][0, :NSH]
    return out


# revision 61
# speedup vs baseline: 2.8201x; 1.0016x over previous
"""2-layer GCN (PyG GCNConv x2 + sigmoid) on 8 TRN2 NeuronCores, single fused NEFF.

Sharding: dst-node ranges across the 8 cores (6250 nodes each); GCN weights
replicated; the layer-1->layer-2 halo exchange is an on-device AllGather of
each core's 6250 z'=W2^T h' values.

Design notes (cost-model driven):
- ap_gather costs max(table_cols, num_idxs)*0.833ns -> tables and gather
  chunks must be size-matched. 7 src-quarters (table=7144 cols) with 2
  ~8K-slot chunks each keeps L1 gathers slot-optimal (~0.84ns/edge).
- Edge segment sums via exact-degree ladders with layouts shared
  (max-over-core) so one SPMD program fits all cores; k=1 rows are Act
  copies, some k<=2/3 pair-adds go to GPSIMD to balance DVE; per-quarter
  partials are perm-gathered back to node order and accumulated in bf16
  (DVE 2x mode); assembly runs one quarter behind the gathers to keep
  GPSIMD saturated.
- Tables built by PE in bf16 (1 cyc/col); both dinv_dst multiplies fold
  into bf16 tensor ops; h'@W2 contracts on PE in bf16.
- Layer 2: the 8 GPSIMD 16-partition groups each own one SRC CORE RANGE so
  per-group z tables are 6256 wide (table-cost-minimal); self-loops are
  excluded from the edge stream (their term is zrow, added per chunk);
  cross-group partial sums contract on the PE via a stride-16 ones vector
  over the perm-gathered (bf16-converted) partials.
- Finalize and the last quarter's assembly are column-sliced so the
  z-row production chain into the collective stays pipelined.
"""

import sys

sys.path.insert(0, "/opt/trn_rl_repo")
import numpy as np
import ml_dtypes
from contextlib import ExitStack

from concourse import bacc, mybir
from concourse.tile import TileContext
from concourse.bass_utils import run_bass_kernel_spmd

MEASURE = False
LAST_SIM_NS = None

N = 50000
E = 800000
F = 128
P = 128
NCORES = 8
NSH = N // NCORES  # 6250
NQ = 7
QN = 7143  # nodes per quarter (last has 7142)
T = QN + 1  # 7144: [zero col, up to 7143 nodes]
NP_ = 6256  # padded per-core node count
MMCH = 512
XB = 2048


def _wrap16(idx_flat):
    n = idx_flat.shape[0]
    assert n % 16 == 0
    return np.ascontiguousarray(idx_flat.reshape(n // 16, 16).T)


def _pad16(n):
    return ((n + 15) // 16) * 16


def _concat_aranges(lens):
    if len(lens) == 0:
        return np.zeros(0, dtype=np.int64)
    total = int(lens.sum())
    out = np.ones(total, dtype=np.int64)
    ends = np.cumsum(lens)
    out[0] = 0
    out[ends[:-1]] = -(lens[:-1] - 1)
    return np.cumsum(out)


def _ladder_layout(kap_by_unit, n_chunks_cap, kdesc=False):
    """kap_by_unit: [n_units, n_nodes]. Shared exact-k ladder with row-aligned
    chunks, big k first (heavy reduces overlap the next chunk's gather).
    Returns (descr[(chunk, off, n_rows, k, col)], cols, kbase, chunk_sizes)."""
    kmax = int(kap_by_unit.max())
    budgets = {}
    for k in range(1, kmax + 1):
        nk = int((kap_by_unit == k).sum(axis=1).max())
        if nk > 0:
            budgets[k] = nk
    raw = sum(k * nk for k, nk in budgets.items())
    cap = raw + 64 if n_chunks_cap is None else (raw + n_chunks_cap - 1) // n_chunks_cap + 48
    descr, kbase = [], {}
    col = 1
    ch, off = 0, 0
    for k in sorted(budgets, reverse=kdesc):
        nk = budgets[k]
        kbase[k] = col
        left = nk
        while left > 0:
            fit = min(left, (cap - off) // k)
            if fit == 0:
                ch += 1
                off = 0
                fit = min(left, cap // k)
            descr.append((ch, off, fit, k, col))
            off += fit * k
            col += fit
            left -= fit
    chunk_sizes = {}
    for c, o, nr, k, _ in descr:
        chunk_sizes[c] = max(chunk_sizes.get(c, 0), o + nr * k)
    sizes = [_pad16(chunk_sizes[c]) for c in sorted(chunk_sizes)]
    return descr, col, kbase, sizes


def _pack_slots(kap, srcl_by_dst, dstl_by_dst, descr, kbase, cols, chunk_offs):
    """kap: [n_nodes] this unit's degrees; srcl/dstl: this unit's edges sorted
    by dst. Returns (slot_positions, slot_values, perm[node->accp col])."""
    nodes = np.nonzero(kap)[0]
    kn = kap[nodes]
    nd = np.lexsort((nodes, kn))
    nodes_s, kn_s = nodes[nd], kn[nd]
    rank = np.zeros(len(nodes_s), dtype=np.int64)
    colof = np.zeros(len(nodes_s), dtype=np.int64)
    for k in np.unique(kn_s):
        mk = kn_s == k
        rank[mk] = np.arange(mk.sum())
        colof[mk] = kbase[int(k)]
    node_col = colof + rank
    col2slot = np.full(cols, -1, dtype=np.int64)
    for ch, off, n_rows, k, col in descr:
        cc = np.arange(n_rows)
        col2slot[col + cc] = chunk_offs[ch] + off + cc * k
    starts = col2slot[node_col]
    eslots = np.repeat(starts, kn_s) + _concat_aranges(kn_s)
    # edge values in (k, node) order: stable sort of dst-sorted edges by k
    eo = np.argsort(kap[dstl_by_dst], kind="stable")
    ev = srcl_by_dst[eo]
    pm = np.zeros(len(kap), dtype=np.int16)
    pm[nodes_s] = node_col.astype(np.int16)
    return eslots, ev, pm


def host_prep(x, edge_index, W1, b1, W2, b2):
    src = np.concatenate([edge_index[0], np.arange(N, dtype=np.int64)]).astype(np.int32)
    dst = np.concatenate([edge_index[1], np.arange(N, dtype=np.int64)]).astype(np.int32)
    deg = np.bincount(dst, minlength=N).astype(np.float32)
    dinv = 1.0 / np.sqrt(np.maximum(deg, 1e-12))
    dinv[deg <= 0] = 0.0

    # Node -> table position. Stratified round-robin: nodes with identical
    # per-core in-degree vectors spread evenly over quarters, which tightens
    # the shared (max-over-core) ladder budgets vs a random permutation.
    degc = np.zeros((N, NCORES), dtype=np.int32)
    dst_t = np.concatenate([edge_index[1], np.arange(N, dtype=np.int64)])
    src_t = np.concatenate([edge_index[0], np.arange(N, dtype=np.int64)])
    np.add.at(degc, (src_t, dst_t // NSH), 1)
    okey = np.lexsort(tuple(degc[:, c] for c in range(NCORES)))
    rank = np.empty(N, dtype=np.int64)
    rank[okey] = np.arange(N)
    psrc = (rank % NQ) * QN + rank // NQ  # node -> table position
    assert psrc.max() < NQ * QN
    pinv = np.argsort(psrc)

    xtp = (x * dinv[:, None]).T.astype(np.float32)[:, pinv]  # [128, N] pos order
    xt = np.zeros((P, NQ * T), dtype=ml_dtypes.bfloat16)
    for q in range(NQ):
        qn = min(QN, N - q * QN)
        xt[:, q * T + 1 : q * T + 1 + qn] = xtp[:, q * QN : q * QN + qn].astype(
            ml_dtypes.bfloat16
        )

    core = dst // NSH
    dstl = (dst % NSH).astype(np.int64)
    pos = psrc[src]
    quarter = pos // QN
    srcl = (pos % QN).astype(np.int64) + 1

    flat = (core.astype(np.int64) * NQ + quarter) * NSH + dstl
    kap = np.bincount(flat, minlength=NCORES * NQ * NSH).reshape(NCORES, NQ, NSH)

    layouts = []
    for q in range(NQ):
        # last quarter: big-k first so its trailing chunk is reduce-light and
        # the final perm-gather (and the finalize chain) starts sooner
        descr, cols, kbase, sizes = _ladder_layout(kap[:, q, :], 2, kdesc=(q >= NQ - 3))
        offs = np.concatenate([[0], np.cumsum(sizes)]).astype(np.int64)
        layouts.append((descr, cols, kbase, sizes, offs))
    SQ = [int(l[4][-1]) for l in layouts]
    PQ = _pad16(max(l[1] for l in layouts))
    G0 = max(max(l[3]) for l in layouts)

    order = np.lexsort((dstl, quarter, core))
    so, do_, qo, co = srcl[order], dstl[order], quarter[order], core[order]

    # combined per-quarter index stream: [SQ[q] slot idxs | NP_ perm idxs]
    qoff = np.concatenate([[0], np.cumsum([s + NP_ for s in SQ])]).astype(np.int64)
    qbase = np.concatenate([[0], np.cumsum(SQ)]).astype(np.int64)
    eidx = np.zeros((NCORES, int(qoff[-1])), dtype=np.int16)
    for c in range(NCORES):
        mc = co == c
        for q in range(NQ):
            m = mc & (qo == q)
            descr, cols, kbase, sizes, offs = layouts[q]
            eslots, ev, pm = _pack_slots(
                kap[c, q], so[m], do_[m], descr, kbase, cols, offs
            )
            eidx[c, qoff[q] + eslots] = ev.astype(np.int16)
            eidx[c, qoff[q] + SQ[q] : qoff[q] + SQ[q] + NSH] = pm

    eidx_w = np.zeros((NCORES, P, int(qoff[-1]) // 16), dtype=np.int16)
    for c in range(NCORES):
        eidx_w[c] = np.tile(_wrap16(eidx[c]), (8, 1))

    dinvb = np.zeros((NCORES, P, NP_), dtype=ml_dtypes.bfloat16)
    dinvrow = np.zeros((NCORES, 1, NP_), dtype=np.float32)
    for c in range(NCORES):
        dv = dinv[c * NSH : (c + 1) * NSH]
        dinvb[c, :, :NSH] = np.tile(dv.astype(ml_dtypes.bfloat16)[None, :], (P, 1))
        dinvrow[c, 0, :NSH] = dv

    meta = dict(layouts=layouts, SQ=SQ, PQ=PQ, G0=G0, qbase=qbase, qoff=qoff)
    k1_inputs = []
    for c in range(NCORES):
        k1_inputs.append(
            {
                "xt": xt,
                "w1": np.asarray(W1, dtype=ml_dtypes.bfloat16),
                "b1": np.asarray(b1, dtype=np.float32).reshape(P, 1),
                "w2": np.asarray(W2, dtype=ml_dtypes.bfloat16).reshape(P, 1),
                "eidx": np.ascontiguousarray(eidx_w[c]),
                "dinvb": np.ascontiguousarray(dinvb[c]),
                "dinvrow": np.ascontiguousarray(dinvrow[c]),
            }
        )
    return k1_inputs, meta, (src, dst, dinv)


def host_prep_k2(src, dst):
    """Layer 2: 8 GPSIMD groups = 8 src core ranges; self-loops excluded."""
    m = src != dst
    s2, d2 = src[m].astype(np.int64), dst[m].astype(np.int64)
    c2 = d2 // NSH
    g2 = s2 // NSH
    dstl = d2 % NSH
    srcl = s2 % NSH + 1

    flat = (c2 * NCORES + g2) * NSH + dstl
    kap2 = np.bincount(flat, minlength=NCORES * NCORES * NSH).reshape(
        NCORES * NCORES, NSH
    )
    descr2, cols2, kbase2, sizes2 = _ladder_layout(kap2, 2, kdesc=True)
    offs2 = np.concatenate([[0], np.cumsum(sizes2)]).astype(np.int64)
    slots2 = int(offs2[-1])
    P2 = _pad16(cols2)

    order = np.lexsort((dstl, g2, c2))
    so, do_, go, co = srcl[order], dstl[order], g2[order], c2[order]
    eidx2 = np.zeros((NCORES, NCORES, slots2), dtype=np.int16)
    perm2 = np.zeros((NCORES, NCORES, NP_), dtype=np.int16)
    for c in range(NCORES):
        mc = co == c
        for g in range(NCORES):
            mm = mc & (go == g)
            eslots, ev, pm = _pack_slots(
                kap2[c * NCORES + g], so[mm], do_[mm], descr2, kbase2, cols2, offs2
            )
            eidx2[c, g, eslots] = ev.astype(np.int16)
            perm2[c, g, :NSH] = pm

    eidx2_w = np.zeros((NCORES, P, slots2 // 16), dtype=np.int16)
    perm2_w = np.zeros((NCORES, P, NP_ // 16), dtype=np.int16)
    for c in range(NCORES):
        for g in range(NCORES):
            eidx2_w[c, g * 16 : (g + 1) * 16] = _wrap16(eidx2[c, g])
            perm2_w[c, g * 16 : (g + 1) * 16] = _wrap16(perm2[c, g])

    svec = np.zeros((P, 1), dtype=ml_dtypes.bfloat16)
    svec[0:P:16, 0] = 1.0  # sum the 8 group-partial rows

    meta2 = dict(descr2=descr2, P2=P2, slots2=slots2, sizes2=sizes2, offs2=offs2)
    k2_inputs = []
    for c in range(NCORES):
        k2_inputs.append(
            {
                "eidx2": np.ascontiguousarray(eidx2_w[c]),
                "perm2": np.ascontiguousarray(perm2_w[c]),
                "svec": svec,
            }
        )
    return k2_inputs, meta2


def build_fused(meta, meta2, b2val):
    layouts, SQ, PQ, G0, qbase, qoff = (
        meta["layouts"],
        meta["SQ"],
        meta["PQ"],
        meta["G0"],
        meta["qbase"],
        meta["qoff"],
    )
    ITW = (max(SQ) + NP_) // 16  # combined per-quarter idx tile width
    descr2, P2, slots2 = meta2["descr2"], meta2["P2"], meta2["slots2"]
    sizes2, offs2 = meta2["sizes2"], meta2["offs2"]
    G2 = max(sizes2)

    nc = bacc.Bacc(None, target_bir_lowering=False)
    f32, f32r, bf16, i16 = (
        mybir.dt.float32,
        mybir.dt.float32r,
        mybir.dt.bfloat16,
        mybir.dt.int16,
    )

    xt_d = nc.dram_tensor("xt", [P, NQ * T], bf16, kind="ExternalInput")
    w1_d = nc.dram_tensor("w1", [P, P], bf16, kind="ExternalInput")
    b1_d = nc.dram_tensor("b1", [P, 1], f32, kind="ExternalInput")
    w2_d = nc.dram_tensor("w2", [P, 1], bf16, kind="ExternalInput")
    eidx_d = nc.dram_tensor("eidx", [P, int(qoff[-1]) // 16], i16, kind="ExternalInput")
    dinvb_d = nc.dram_tensor("dinvb", [P, NP_], bf16, kind="ExternalInput")
    dinvrow_d = nc.dram_tensor("dinvrow", [1, NP_], f32, kind="ExternalInput")
    eidx2_d = nc.dram_tensor("eidx2", [P, slots2 // 16], i16, kind="ExternalInput")
    perm2_d = nc.dram_tensor("perm2", [P, NP_ // 16], i16, kind="ExternalInput")
    svec_d = nc.dram_tensor("svec", [P, 1], bf16, kind="ExternalInput")
    out_d = nc.dram_tensor("out", [1, NP_], f32, kind="ExternalOutput")

    zin = nc.dram_tensor("zin_cc", [1, NSH], f32, kind="Internal")
    zall = nc.dram_tensor(
        "zall_cc", [NCORES, NSH], f32, kind="Internal", addr_space="Shared"
    )

    Copy = mybir.ActivationFunctionType.Copy
    Sigmoid = mybir.ActivationFunctionType.Sigmoid
    ADD = mybir.AluOpType.add

    with ExitStack() as ctx:
        tc = ctx.enter_context(TileContext(nc))
        cpool = ctx.enter_context(tc.tile_pool(name="cpool", bufs=1))
        w1 = cpool.tile([P, P], bf16)
        b1 = cpool.tile([P, 1], f32)
        w2 = cpool.tile([P, 1], bf16)
        acc = cpool.tile([P, NP_], bf16)
        warm = cpool.tile([1, 16], f32)
        nc.sync.dma_start(out=w1[:], in_=w1_d[:])
        nc.sync.dma_start(out=b1[:], in_=b1_d[:])
        nc.sync.dma_start(out=w2[:], in_=w2_d[:])
        # preload the sigmoid activation table off the critical path
        nc.vector.memset(warm[:], 0.0)
        nc.scalar.activation(warm[:], warm[:], Sigmoid, bias=0.0)

        with (
            tc.tile_pool(name="tabs", bufs=2) as tabs,
            tc.tile_pool(name="xpool", bufs=2) as xpool,
            tc.tile_pool(name="gpool", bufs=2) as gpool,
            tc.tile_pool(name="tpool", bufs=1) as tpool,
            tc.tile_pool(name="accpool", bufs=2) as accpool,
            tc.tile_pool(name="epool", bufs=2) as epool,
            tc.tile_pool(name="pspool", bufs=2, space="PSUM") as pspool,
        ):
            accps = {}

            def assemble(q):
                # perm-gather quarter q's partials to node order and fold
                # into acc (emitted one quarter late to keep Pool saturated)
                accp_q, it_q = accps.pop(q)
                p0 = SQ[q] // 16
                tt = gpool.tile([P, G0], f32, tag="g")
                nc.gpsimd.ap_gather(
                    tt[:, :NP_],
                    accp_q[:],
                    it_q[:, p0 : p0 + NP_ // 16],
                    channels=P,
                    num_elems=PQ,
                    d=1,
                    num_idxs=NP_,
                )
                if q == 0:
                    nc.scalar.activation(acc[:], tt[:, :NP_], Copy)
                elif q < NQ - 1:
                    tb = tpool.tile([P, NP_], bf16, tag="tb")
                    nc.scalar.activation(tb[:], tt[:, :NP_], Copy)
                    nc.vector.tensor_add(acc[:], acc[:], tb[:])
                else:
                    # last quarter: slice so finalize can start per-slice
                    tb = tpool.tile([P, NP_], bf16, tag="tb")
                    for s0 in range(0, NP_, 1564):
                        sl = slice(s0, s0 + 1564)
                        nc.scalar.activation(tb[:, sl], tt[:, sl], Copy)
                        nc.vector.tensor_add(acc[:, sl], acc[:, sl], tb[:, sl])

            for q in range(NQ):
                descr, cols, kbase, sizes, offs = layouts[q]
                tab = tabs.tile([P, T], f32, tag="tab")
                chunks = []
                x0 = sum(chunks)
                while x0 < T:
                    chunks.append(min(XB, T - x0))
                    x0 += chunks[-1]
                x0 = 0
                for xw in chunks:
                    xc = xpool.tile([P, XB], bf16, tag="x")
                    nc.sync.dma_start(
                        out=xc[:, :xw], in_=xt_d[:, q * T + x0 : q * T + x0 + xw]
                    )
                    ps = pspool.tile([P, XB], f32, tag="ps")
                    for m0 in range(0, xw, MMCH):
                        mw = min(MMCH, xw - m0)
                        nc.tensor.matmul(
                            ps[:, m0 : m0 + mw],
                            w1[:],
                            xc[:, m0 : m0 + mw],
                            start=True,
                            stop=True,
                        )
                    nc.scalar.activation(tab[:, x0 : x0 + xw], ps[:, :xw], Copy)
                    x0 += xw
                accp = accpool.tile([P, PQ], f32, tag="accp")
                it = epool.tile([P, ITW], i16, tag="it")
                qw = (SQ[q] + NP_) // 16
                i0 = int(qoff[q]) // 16
                nc.sync.dma_start(out=it[:, :qw], in_=eidx_d[:, i0 : i0 + qw])
                accps[q] = (accp, it)
                nc.vector.memset(accp[:, 0:1], 0.0)
                by_chunk = {}
                for d_ in descr:
                    by_chunk.setdefault(d_[0], []).append(d_)
                for ci, ch in enumerate(sorted(by_chunk)):
                    sz = sizes[ch]
                    c0 = int(offs[ch]) // 16
                    g = gpool.tile([P, G0], f32, tag="g")
                    nc.gpsimd.ap_gather(
                        g[:, :sz],
                        tab[:],
                        it[:, c0 : c0 + sz // 16],
                        channels=P,
                        num_elems=T,
                        d=1,
                        num_idxs=sz,
                    )
                    if ci == 0 and q > 0:
                        assemble(q - 1)
                    for _, off, n_rows, k, col in by_chunk[ch]:
                        if k == 1 and ci == 0:
                            # Act handles chunk-0 k=1 rows; later chunks go to
                            # DVE so Act isn't blocked ahead of next tab build
                            nc.scalar.activation(
                                accp[:, col : col + n_rows],
                                g[:, off : off + n_rows],
                                Copy,
                            )
                        elif k == 1:
                            nc.vector.tensor_copy(
                                accp[:, col : col + n_rows], g[:, off : off + n_rows]
                            )
                        elif k == 2 and n_rows >= 48:
                            # rebalance: ~1/3 of pair-adds on GPSIMD
                            nh = _pad16(n_rows * 2 // 3)
                            pr = g[:, off : off + 2 * nh].rearrange(
                                "p (a b) -> p a b", a=nh, b=2
                            )
                            nc.gpsimd.tensor_add(
                                accp[:, col : col + nh], pr[:, :, 0], pr[:, :, 1]
                            )
                            nc.vector.tensor_reduce(
                                accp[:, col + nh : col + n_rows],
                                g[:, off + 2 * nh : off + 2 * n_rows].rearrange(
                                    "p (a b) -> p a b", a=n_rows - nh, b=2
                                ),
                                axis=mybir.AxisListType.X,
                                op=ADD,
                            )
                        else:
                            nc.vector.tensor_reduce(
                                accp[:, col : col + n_rows],
                                g[:, off : off + n_rows * k].rearrange(
                                    "p (a b) -> p a b", a=n_rows, b=k
                                ),
                                axis=mybir.AxisListType.X,
                                op=ADD,
                            )
                if q == NQ - 1:
                    assemble(q)

        # finalize (4 column slices): h' = dinv*sigmoid(dinv*acc+b1); z=W2^T h'
        with (
            tc.tile_pool(name="fin", bufs=1) as fin,
            tc.tile_pool(name="zps", bufs=2, space="PSUM") as zps,
        ):
            zrow = fin.tile([1, NP_], f32)
            dinvrow2 = fin.tile([1, NP_], f32)
            with tc.tile_pool(name="finb", bufs=1) as finb:
                dinvb = finb.tile([P, NP_], bf16)
                nc.sync.dma_start(out=dinvb[:], in_=dinvb_d[:])
                nc.sync.dma_start(out=dinvrow2[:], in_=dinvrow_d[:])
                bounds = [0, 1536, 3072, 4608, NP_]
                for si in range(4):
                    sl = slice(bounds[si], bounds[si + 1])
                    nc.vector.tensor_mul(acc[:, sl], acc[:, sl], dinvb[:, sl])
                    nc.scalar.activation(
                        acc[:, sl], acc[:, sl], Sigmoid, bias=b1[:, 0:1]
                    )
                    nc.vector.tensor_mul(acc[:, sl], acc[:, sl], dinvb[:, sl])
                    for m0 in range(bounds[si], bounds[si + 1], MMCH):
                        mw = min(MMCH, bounds[si + 1] - m0)
                        ps = zps.tile([1, MMCH], f32, tag="zp")
                        nc.tensor.matmul(
                            ps[:, :mw],
                            w2[:],
                            acc[:, m0 : m0 + mw],
                            start=True,
                            stop=True,
                        )
                        nc.scalar.activation(zrow[:, m0 : m0 + mw], ps[:, :mw], Copy)
                nc.sync.dma_start(out=zin[:, : NSH // 2], in_=zrow[:, : NSH // 2])
                nc.sync.dma_start(out=zin[:, NSH // 2 :], in_=zrow[:, NSH // 2 : NSH])

            nc.gpsimd.collective_compute(
                "AllGather",
                mybir.AluOpType.bypass,
                replica_groups=[list(range(NCORES))],
                ins=[zin[:].opt()],
                outs=[zall[:].opt()],
            )

            # ---- layer 2 ----
            with (
                tc.tile_pool(name="k2pool", bufs=1) as pool2,
                tc.tile_pool(name="zps2", bufs=2, space="PSUM") as zps2,
            ):
                eidx2 = pool2.tile([P, slots2 // 16], i16)
                perm2 = pool2.tile([P, NP_ // 16], i16)
                accp2 = pool2.tile([P, P2], f32)
                svec = pool2.tile([P, 1], bf16)
                zfin = pool2.tile([1, NP_], f32)
                nc.sync.dma_start(out=eidx2[:], in_=eidx2_d[:])
                nc.sync.dma_start(out=perm2[:], in_=perm2_d[:])
                nc.sync.dma_start(out=svec[:], in_=svec_d[:])
                nc.vector.memset(accp2[:, 0:1], 0.0)
                with tc.tile_pool(name="ztpool", bufs=1) as ztpool, tc.tile_pool(
                    name="g2pool", bufs=2
                ) as g2pool:
                    zt = ztpool.tile([P, NP_], f32)
                    nc.vector.memset(zt[:, 0:1], 0.0)
                    nc.sync.dma_start(out=zt[0:P:16, 1 : 1 + NSH], in_=zall[:, :])
                    by_chunk2 = {}
                    for d_ in descr2:
                        by_chunk2.setdefault(d_[0], []).append(d_)
                    for ch in sorted(by_chunk2):
                        sz = sizes2[ch]
                        g2 = g2pool.tile([P, G2], f32, tag="g2")
                        i0 = int(offs2[ch]) // 16
                        nc.gpsimd.ap_gather(
                            g2[:, :sz],
                            zt[:],
                            eidx2[:, i0 : i0 + sz // 16],
                            channels=P,
                            num_elems=NP_,
                            d=1,
                            num_idxs=sz,
                        )
                        for _, off, n_rows, k, col in by_chunk2[ch]:
                            if k == 1:
                                nc.scalar.activation(
                                    accp2[:, col : col + n_rows],
                                    g2[:, off : off + n_rows],
                                    Copy,
                                )
                            elif k == 2:
                                # pair-adds on GPSIMD (Pool idles post-gather)
                                pairs = g2[:, off : off + 2 * n_rows].rearrange(
                                    "p (a b) -> p a b", a=n_rows, b=2
                                )
                                nc.gpsimd.tensor_add(
                                    accp2[:, col : col + n_rows],
                                    pairs[:, :, 0],
                                    pairs[:, :, 1],
                                )
                            elif k == 3:
                                trip = g2[:, off : off + 3 * n_rows].rearrange(
                                    "p (a b) -> p a b", a=n_rows, b=3
                                )
                                nc.gpsimd.tensor_add(
                                    accp2[:, col : col + n_rows],
                                    trip[:, :, 0],
                                    trip[:, :, 1],
                                )
                                nc.vector.tensor_add(
                                    accp2[:, col : col + n_rows],
                                    accp2[:, col : col + n_rows],
                                    trip[:, :, 2],
                                )
                            else:
                                nc.vector.tensor_reduce(
                                    accp2[:, col : col + n_rows],
                                    g2[:, off : off + n_rows * k].rearrange(
                                        "p (a b) -> p a b", a=n_rows, b=k
                                    ),
                                    axis=mybir.AxisListType.X,
                                    op=ADD,
                                )
                with tc.tile_pool(name="gp2", bufs=1) as gp2pool:
                    g2p = gp2pool.tile([P, NP_], f32)
                    g2pb = gp2pool.tile([P, NP_], bf16)
                    nc.gpsimd.ap_gather(
                        g2p[:],
                        accp2[:],
                        perm2[:],
                        channels=P,
                        num_elems=P2,
                        d=1,
                        num_idxs=NP_,
                    )
                    HB = NP_ // 2
                    for s0 in (0, HB):
                        nc.scalar.activation(
                            g2pb[:, s0 : s0 + HB], g2p[:, s0 : s0 + HB], Copy
                        )
                        for m0 in range(s0, s0 + HB, MMCH):
                            mw = min(MMCH, s0 + HB - m0)
                            ps = zps2.tile([1, MMCH], f32, tag="zp2")
                            nc.tensor.matmul(
                                ps[:, :mw],
                                svec[:],
                                g2pb[:, m0 : m0 + mw],
                                start=True,
                                stop=True,
                            )
                            nc.scalar.activation(zfin[:, m0 : m0 + mw], ps[:, :mw], Copy)
                            # += self-loop term, then * dinv_dst
                            nc.vector.tensor_add(
                                zfin[:, m0 : m0 + mw],
                                zfin[:, m0 : m0 + mw],
                                zrow[:, m0 : m0 + mw],
                            )
                            nc.vector.tensor_mul(
                                zfin[:, m0 : m0 + mw],
                                zfin[:, m0 : m0 + mw],
                                dinvrow2[:, m0 : m0 + mw],
                            )
                        nc.scalar.activation(
                            zfin[:, s0 : s0 + HB],
                            zfin[:, s0 : s0 + HB],
                            Sigmoid,
                            bias=float(b2val),
                        )
                        nc.sync.dma_start(
                            out=out_d[:, s0 : s0 + HB], in_=zfin[:, s0 : s0 + HB]
                        )
    nc.finalize()
    return nc


def _sim_ns(nc):
    from concourse import bass_interp

    sim = bass_interp.CoreSim(nc, no_exec=True, publish_trace=False)
    sim.simulate()
    return int(sim.time)


def kernel(x, edge_index, W1, b1, W2, b2):
    global LAST_SIM_NS
    x = np.asarray(x, dtype=np.float32)
    edge_index = np.asarray(edge_index)
    k1_inputs, meta, (src, dst, dinv) = host_prep(x, edge_index, W1, b1, W2, b2)
    k2_inputs, meta2 = host_prep_k2(src, dst)
    b2val = float(np.asarray(b2, dtype=np.float32).reshape(-1)[0])
    nc = build_fused(meta, meta2, b2val)
    if MEASURE:
        LAST_SIM_NS = _sim_ns(nc)
    in_maps = [dict(k1_inputs[c], **k2_inputs[c]) for c in range(NCORES)]
    res = run_bass_kernel_spmd(nc, in_maps, list(range(NCORES)))
    out = np.zeros((N, 1), dtype=np.float32)
    for c in range(NCORES):
        out[c * NSH : (c + 1) * NSH, 0] = res.results[c]["out"][0, :NSH]
    return out
